# revision 20
# baseline (speedup 1.0000x reference)
"""Bass/Trainium2 kernel for nn_Block_14010183320003 (MST++-style block).

Sharding: 8 cores = 2 batches x 4 row-quarters (64 owned rows each, halo 6).
Chain packing: partitions 0:62 = x-derived chain A, 64:126 = y-derived chain B.
Attention is computed from 124-dim input Gram matrices (no q/k tensors);
per-core partial Grams are summed with two ~256KB AllReduces.
Pixel-major Gram stacks are built ON DEVICE with batched DMA xbar transposes
(few big DMA_TRANSPOSE ops instead of per-chunk ones).
Row layout: stride 258 per row (2 zero pad cols) so 3x3 convs are free-axis
shifted reads; depthwise convs run as PE diag-matmul accumulation.
"""
import sys

sys.path.insert(0, "/opt/trn_rl_repo")
import numpy as np
import ml_dtypes

import concourse.bass as bass
import concourse.mybir as mybir
import concourse.tile as tile
import concourse.bacc as bacc
from concourse.bass_utils import run_bass_kernel_spmd

BF16, F32 = mybir.dt.bfloat16, mybir.dt.float32
AF = mybir.ActivationFunctionType
ALU = mybir.AluOpType
bf = ml_dtypes.bfloat16

C = 62
W = 256
RS = 258  # row stride (2 zero pad cols)
OWN = 64
HALO = 6
R = 76
ROFF = 1  # lead pad row at tile row 0 (zero pad for shifted reads)
RA = 78  # lead pad + 76 slab rows + 1 trailing spare
FA = RA * RS  # 20124
NT = 512  # matmul moving chunk
NH, DH = 2, 31
GSPAN0 = (HALO + ROFF) * RS  # owned-pixel span start = 1806
NCK = 129  # owned span 64*258 = 129 chunks of 128

TAPS = [(dy, dx) for dy in (-1, 0, 1) for dx in (-1, 0, 1)]
T6 = [0, 2, 3, 5, 6, 8]  # taps with dx != 0 -> PE diag matmuls
TD = [1, 4, 7]  # dx == 0 taps (dy=-1,0,+1) -> DVE scalar chains
D6 = [TAPS[t][0] * RS + TAPS[t][1] for t in T6]


def exr(e):
    """free range of extent e (slab rows [6-e, 70+e)), incl lead-row offset."""
    return ((HALO - e + ROFF) * RS, (HALO + OWN + e + ROFF) * RS)


def _slab(full, row0):
    """full [C, 256, 256] -> [C, R, RS] zero-padded slab (rows row0..row0+R)."""
    out = np.zeros((C, R, RS), np.float32)
    lo, hi = max(0, row0), min(256, row0 + R)
    out[:, lo - row0 : hi - row0, :W] = full[:, lo:hi]
    return out


def _to_bf(a):
    return np.ascontiguousarray(a.astype(bf))


def _diag_taps6(kc):
    """kc: [128, 9] -> [128, 7*128] bf16: 6 dx!=0 tap diags + identity."""
    out = np.zeros((128, 7 * 128), np.float32)
    ar = np.arange(128)
    for j, t in enumerate(T6):
        out[:, j * 128 : (j + 1) * 128][ar, ar] = kc[:, t]
    out[:, 6 * 128 : 7 * 128][ar, ar] = 1.0
    return _to_bf(out)


def _pair128(a_block, b_block):
    out = np.zeros((128, 128), np.float32)
    out[0:62, 0:62] = a_block
    out[64:126, 64:126] = b_block
    return _to_bf(out)


def prep_host_inputs(inputs):
    """Build the 8 per-core input maps."""
    inp = {k: np.asarray(v, dtype=np.float32) for k, v in inputs.items()}
    wq, wk, wv = inp["attn_wq"], inp["attn_wk"], inp["attn_wv"]
    pw, pb = inp["attn_pw"], inp["attn_pb"]
    dw1, dw2 = inp["attn_dw1"], inp["attn_dw2"]
    resc = inp["attn_rescale"]

    shared = {}
    # v-projection pair weights [128,128].
    # wv01/wv45: block-diag (K rows 0:62 -> cols 0:62 even module, K 64:126 ->
    # cols 64:126 odd module). wv23: single-z, K rows 0:62 feed BOTH col blocks.
    shared["wv01"] = _pair128(wv[0], wv[1])
    shared["wv45"] = _pair128(wv[4], wv[5])
    wv23 = np.zeros((128, 128), np.float32)
    wv23[0:62, 0:62] = wv[2]
    wv23[0:62, 64:126] = wv[3]
    shared["wv23"] = _to_bf(wv23)
    for pair, (me, mo) in {"01": (0, 1), "23": (2, 3), "45": (4, 5)}.items():
        pbv = np.zeros((128, 1), np.float32)
        pbv[0:62, 0] = pb[me]
        pbv[64:126, 0] = pb[mo]
        shared[f"pb{pair}"] = pbv
        for cn, dwk in (("1", dw1), ("2", dw2)):
            kc = np.zeros((128, 9), np.float32)
            for t, (dy, dx) in enumerate(TAPS):
                kc[0:62, t] = dwk[me][:, dy + 1, dx + 1]
                kc[64:126, t] = dwk[mo][:, dy + 1, dx + 1]
            shared[f"m_dw{cn}_{pair}"] = _diag_taps6(kc)
            shared[f"c3_dw{cn}_{pair}"] = np.ascontiguousarray(kc[:, TD])
    # attn small weights: [128, 6*64] col block m at m*64, rows 0:62
    for nm, src in (("wqm", wq), ("wkm", wk), ("pwm", pw)):
        t = np.zeros((128, 6 * 64), np.float32)
        for m in range(6):
            t[0:62, m * 64 : m * 64 + 62] = src[m]
        shared[nm] = t
    rs = np.zeros((128, 6), np.float32)
    for m in range(6):
        rs[0:62, m] = np.repeat(resc[m], DH)
    shared["rsc"] = rs
    ones62 = np.zeros((128, 1), np.float32)
    ones62[0:62, 0] = 1.0
    shared["ones62"] = ones62
    hmA = np.zeros((128, 64), np.float32)
    hmB = np.full((128, 64), -1e4, np.float32)
    for h in range(NH):
        hmA[h * DH : (h + 1) * DH, h * DH : (h + 1) * DH] = 1.0
        hmB[h * DH : (h + 1) * DH, h * DH : (h + 1) * DH] = 0.0
    shared["hmA"] = hmA
    shared["hmB"] = hmB
    lnones = np.zeros((128, 2), np.float32)
    lnones[0:62, 0] = 1.0 / C
    lnones[64:126, 1] = 1.0 / C
    shared["lnones"] = _to_bf(lnones)

    # FF weights. chain A uses ff index 0, chain B index 1.
    for ci, cname in ((0, "a"), (1, "b")):
        g, b = inp["ln_g"][ci], inp["ln_b"][ci]
        w1 = inp["ff_w1"][ci]  # [62, 248]
        w2 = inp["ff_w2"][ci]  # [248, 62]
        w1p = g[:, None] * w1
        c2 = w1p.sum(axis=0)  # [248]
        c1 = b @ w1  # [248]
        dwk = inp["ff_dw"][ci]  # [248, 3, 3]
        r0 = 0 if ci == 0 else 64
        for h in (0, 1):
            sl = slice(h * 124, (h + 1) * 124)
            wt = np.zeros((128, 124), np.float32)
            wt[r0 : r0 + 62, :] = w1p[:, sl]
            wt[r0 + 62, :] = -c2[sl]
            shared[f"w1aug_{cname}{h}"] = _to_bf(wt)
            cb = np.zeros((128, 1), np.float32)
            cb[0:124, 0] = c1[sl]
            shared[f"c1b_{cname}{h}"] = cb
            kc = np.zeros((128, 9), np.float32)
            for t, (dy, dx) in enumerate(TAPS):
                kc[0:124, t] = dwk[sl, dy + 1, dx + 1]
            shared[f"m_ffdw_{cname}{h}"] = _diag_taps6(kc)
            shared[f"c3_ffdw_{cname}{h}"] = np.ascontiguousarray(kc[:, TD])
            w2t = np.zeros((128, 128), np.float32)
            w2t[0:124, 0:62] = w2[sl, :]
            shared[f"w2h_{cname}{h}"] = _to_bf(w2t)

    # fusion weights: fab = Wfu_t^T @ t12 + Wfu_x^T @ xaYb + fucb
    wt_ = np.zeros((128, 128), np.float32)
    wt_[0:62, 0:62] = inp["fuc1_w"][:, 0:62].T
    wt_[64:126, 64:126] = inp["fuc2_w"][:, 0:62].T
    shared["wfut"] = _to_bf(wt_)
    wx_ = np.zeros((128, 128), np.float32)
    wx_[0:62, 0:62] = inp["fuc1_w"][:, 62:124].T
    wx_[64:126, 64:126] = inp["fuc2_w"][:, 62:124].T
    shared["wfux"] = _to_bf(wx_)
    fucb = np.zeros((128, 1), np.float32)
    fucb[0:62, 0] = inp["fuc1_b"]
    fucb[64:126, 0] = inp["fuc2_b"]
    shared["fucb"] = fucb
    ow = np.zeros((128, 64), np.float32)
    ow[0:62, 0:62] = inp["outc_w"][:, 0:62].T
    ow[64:126, 0:62] = inp["outc_w"][:, 62:124].T
    shared["outw"] = _to_bf(ow)
    ob = np.zeros((128, 1), np.float32)
    ob[0:62, 0] = inp["outc_b"]
    shared["outb"] = ob
    fw = np.zeros((128, 9 * 64), np.float32)
    for t, (dy, dx) in enumerate(TAPS):
        fw[0:62, t * 64 : t * 64 + 62] = inp["final_w"][:, 0:62, dy + 1, dx + 1].T
        fw[64:126, t * 64 : t * 64 + 62] = inp["final_w"][:, 62:124, dy + 1, dx + 1].T
    shared["finw"] = _to_bf(fw)
    fb_ = np.zeros((128, 1), np.float32)
    fb_[0:62, 0] = inp["final_b"]
    shared["finb"] = fb_

    for k in list(shared.keys()):
        if shared[k].dtype == np.float32:
            shared[k] = np.ascontiguousarray(shared[k])

    in_maps = []
    for b in range(2):
        xb, yb_, zb = inp["x"][b], inp["y"][b], inp["z"][b]
        for s in range(4):
            row0 = 64 * s - HALO
            xs, ys, zs = _slab(xb, row0), _slab(yb_, row0), _slab(zb, row0)
            m = {}
            xy = np.zeros((128, R, RS), np.float32)
            xy[0:62], xy[64:126] = xs, ys
            m["xy"] = _to_bf(xy.reshape(128, R * RS))
            zs64 = np.zeros((64, R * RS), np.float32)
            zs64[0:62] = zs.reshape(C, R * RS)
            m["zs"] = _to_bf(zs64)
            m["zc"] = _to_bf(zs[:, 5:71].reshape(62, 66 * RS))
            msk = np.zeros((128, 4), np.float32)
            msk[:, 0] = 0.0 if s == 0 else 1.0
            msk[:, 1] = 0.0 if s == 3 else 1.0
            msk[:, 2] = 1.0 if b == 0 else 0.0
            msk[:, 3] = 1.0 if b == 1 else 0.0
            m["msk"] = msk
            m.update(shared)
            in_maps.append(m)
    return in_maps


def assemble_output(results):
    out = np.zeros((2, C, 256, 256), np.float32)
    for b in range(2):
        for s in range(4):
            r = results[b * 4 + s]["out"]  # [62, 64*256]
            out[b, :, 64 * s : 64 * (s + 1), :] = r.reshape(C, OWN, W)
    return out


# ---------------------------------------------------------------------------
# device IR
# ---------------------------------------------------------------------------

PAIRS = ["01", "23", "45"]
L2LEN = exr(2)[1] - exr(2)[0]  # 17544


def build_nc():
    nc = bacc.Bacc(None, target_bir_lowering=False, debug=False)

    P = {}
    P["xy"] = nc.declare_dram_parameter("xy", [128, R * RS], BF16, isOutput=False)
    P["zs"] = nc.declare_dram_parameter("zs", [64, R * RS], BF16, isOutput=False)
    P["zc"] = nc.declare_dram_parameter("zc", [C, 66 * RS], BF16, isOutput=False)
    P["msk"] = nc.declare_dram_parameter("msk", [128, 4], F32, isOutput=False)
    wnames_bf = (
        [f"wv{p}" for p in PAIRS]
        + ["lnones"]
        + [f"w1aug_{c}{h}" for c in "ab" for h in "01"]
        + [f"w2h_{c}{h}" for c in "ab" for h in "01"]
        + ["wfut", "wfux", "outw", "finw"]
    )
    wnames_f32 = (
        [f"pb{p}" for p in PAIRS]
        + ["rsc", "wqm", "wkm", "pwm", "ones62", "hmA", "hmB"]
        + [f"c1b_{c}{h}" for c in "ab" for h in "01"]
        + [f"c3_dw{c}_{p}" for c in "12" for p in PAIRS]
        + [f"c3_ffdw_{c}{h}" for c in "ab" for h in "01"]
        + ["fucb", "outb", "finb"]
    )
    WSHAPE = {
        "wqm": [128, 6 * 64], "wkm": [128, 6 * 64], "pwm": [128, 6 * 64],
        "ones62": [128, 1], "lnones": [128, 2], "rsc": [128, 6],
        "outw": [128, 64], "finw": [128, 9 * 64],
        "hmA": [128, 64], "hmB": [128, 64],
        "wfut": [128, 128], "wfux": [128, 128],
        "fucb": [128, 1], "outb": [128, 1], "finb": [128, 1],
    }
    for p in PAIRS:
        WSHAPE[f"wv{p}"] = [128, 128]
        WSHAPE[f"pb{p}"] = [128, 1]
        for c in "12":
            WSHAPE[f"m_dw{c}_{p}"] = [128, 7 * 128]
            WSHAPE[f"c3_dw{c}_{p}"] = [128, 3]
    for c in "ab":
        for h in "01":
            WSHAPE[f"w1aug_{c}{h}"] = [128, 124]
            WSHAPE[f"m_ffdw_{c}{h}"] = [128, 7 * 128]
            WSHAPE[f"c3_ffdw_{c}{h}"] = [128, 3]
            WSHAPE[f"c1b_{c}{h}"] = [128, 1]
            WSHAPE[f"w2h_{c}{h}"] = [128, 128]
    mnames = [f"m_dw{c}_{p}" for c in "12" for p in PAIRS] + [
        f"m_ffdw_{c}{h}" for c in "ab" for h in "01"
    ]
    for nm in wnames_bf + mnames:
        P[nm] = nc.declare_dram_parameter(nm, WSHAPE[nm], BF16, isOutput=False)
    for nm in wnames_f32:
        P[nm] = nc.declare_dram_parameter(nm, WSHAPE[nm], F32, isOutput=False)
    out_p = nc.declare_dram_parameter("out", [C, OWN * W], F32, isOutput=True)

    with tile.TileContext(nc, num_cores=8) as tc:
        with (
            tc.tile_pool(name="w", bufs=1) as wp,
            tc.tile_pool(name="small", bufs=1) as sp,
            tc.tile_pool(name="big", bufs=1) as bp,
            tc.tile_pool(name="ring", bufs=3) as rp,
            tc.tile_pool(name="psA", bufs=3, space="PSUM") as psA,
            tc.tile_pool(name="psC", bufs=2, space="PSUM") as psC,
            tc.tile_pool(name="dram", bufs=1, space="DRAM") as dp,
        ):
            # ---------- input slabs first (big DMAs lead the SP queue) ----------
            zz = bp.tile([128, FA], BF16, tag="s1")
            nc.sync.dma_start(zz[0:64, RS : RS + R * RS], P["zs"][:])
            xy = bp.tile([128, FA], BF16, tag="xy")
            nc.sync.dma_start(xy[0:64, RS : RS + R * RS], P["xy"][0:64, :])
            nc.sync.dma_start(xy[64:128, RS : RS + R * RS], P["xy"][64:128, :])

            def load_mdw(nm):
                t_ = rp.tile([128, 7 * 128], BF16, tag="mdw", bufs=2, name=nm + "_l")
                nc.scalar.dma_start(t_[:], P[nm][:])
                return t_

            def chunks(rng, step):
                lo, hi = rng
                out = []
                while lo < hi:
                    out.append((lo, min(lo + step, hi)))
                    lo += step
                return out

            def pad_zero(t, lo_row=0, hi_row=RA):
                v = t[:].rearrange("p (r s) -> p r s", s=RS)[:, lo_row:hi_row, W : W + 2]
                nc.vector.memset(v, 0.0)

            def mask_rows(t, e, dtype_rows=(0, 128)):
                if e <= 0:
                    return
                r0, r1 = dtype_rows
                top = t[r0:r1, (HALO - e + ROFF) * RS : (HALO + ROFF) * RS]
                bot = t[r0:r1, (HALO + OWN + ROFF) * RS : (HALO + OWN + e + ROFF) * RS]
                nc.vector.tensor_scalar_mul(top, top, MTOP[r0:r1])
                nc.vector.tensor_scalar_mul(bot, bot, MBOT[r0:r1])

            def stream_v(dst, wv_t, src, e, kp=128):
                """dst[:, rng] = (wv_t.T @ src)[:, rng] via psA, ACT copy evac."""
                for lo, hi in chunks(exr(e), 1024):
                    ps = psA.tile([128, 1024], F32, tag="psA")
                    for l2, h2 in chunks((lo, hi), NT):
                        nc.tensor.matmul(
                            ps[:, l2 - lo : h2 - lo], wv_t[0:kp, :], src[0:kp, l2:h2],
                            start=True, stop=True,
                        )
                    nc.scalar.copy(dst[:, lo:hi], ps[:, 0 : hi - lo])

            def dw_taps_split(ps, src, mats7, cv, lo, hi, start):
                """accumulate depthwise 3x3 of src into ps[:, 0:hi-lo]:
                6 dx!=0 taps as PE diag-MMs; 3 dx=0 taps on DVE into a bf16
                scratch merged by an identity diag-MM (block 6 of mats7)."""
                n = hi - lo
                scr = rp.tile([128, 1024], BF16, tag="scr", bufs=2)
                nc.vector.tensor_scalar_mul(
                    scr[:, 0:n], src[:, lo - RS : hi - RS], cv[:, 0:1]
                )
                nc.vector.scalar_tensor_tensor(
                    scr[:, 0:n], src[:, lo:hi], cv[:, 1:2], scr[:, 0:n],
                    op0=ALU.mult, op1=ALU.add,
                )
                nc.vector.scalar_tensor_tensor(
                    scr[:, 0:n], src[:, lo + RS : hi + RS], cv[:, 2:3], scr[:, 0:n],
                    op0=ALU.mult, op1=ALU.add,
                )
                for l2, h2 in chunks((lo, hi), NT):
                    for j, d in enumerate(D6):
                        nc.tensor.matmul(
                            ps[:, l2 - lo : h2 - lo],
                            mats7[:, j * 128 : (j + 1) * 128],
                            src[:, l2 + d : h2 + d],
                            start=(start and j == 0), stop=False,
                        )
                    nc.tensor.matmul(
                        ps[:, l2 - lo : h2 - lo],
                        mats7[:, 6 * 128 : 7 * 128],
                        scr[:, l2 - lo : h2 - lo],
                        start=False, stop=True,
                    )

            def conv_pe_gelu(dst, src, mats, cv, e_out):
                """dst = gelu(dwconv(src)) over extent e_out (split PE/DVE)."""
                for lo, hi in chunks(exr(e_out), 1024):
                    ps = psA.tile([128, 1024], F32, tag="psA")
                    dw_taps_split(ps, src, mats, cv, lo, hi, start=True)
                    nc.scalar.activation(dst[:, lo:hi], ps[:, 0 : hi - lo], AF.Gelu)

            def pair_tail(out_t, v_t, g_t, WoTp, pb_t, resid, pair, e_out):
                """out_t = WoT^T v + dw2(g) + pb + resid (split PE/DVE conv)."""
                mats = load_mdw(f"m_dw2_{pair}")
                cv = WT[f"c3_dw2_{pair}"]
                for lo, hi in chunks(exr(e_out), 1024):
                    ps = psA.tile([128, 1024], F32, tag="psA")
                    for l2, h2 in chunks((lo, hi), NT):
                        nc.tensor.matmul(
                            ps[:, l2 - lo : h2 - lo], WoTp[:], v_t[:, l2:h2],
                            start=True, stop=False,
                        )
                    dw_taps_split(ps, g_t, mats, cv, lo, hi, start=False)
                    nc.vector.scalar_tensor_tensor(
                        out_t[:, lo:hi], ps[:, 0 : hi - lo], pb_t[:],
                        resid[:, lo:hi], op0=ALU.add, op1=ALU.add,
                    )

            # ---------- gram helpers (batched xbar transposes) ----------
            KSECS = [(0, 33), (33, 65), (65, 97), (97, NCK)]

            def tp3v(tp_tile):
                return tp_tile[:].rearrange("p (k c) -> p k c", c=128)

            def gram_half(tp_tile, src, p0, c0, eng):
                """tp[p, k*128 + c0 + f] = src[p0+f, span k*128+p] via xbar
                transpose DMAs, sectioned so gram MMs pipeline per section."""
                tp3 = tp3v(tp_tile)
                for k0, k1 in KSECS:
                    eng.dma_start(
                        tp3[:, k0:k1, c0 : c0 + 64],
                        src[p0 : p0 + 64,
                            GSPAN0 + k0 * 128 : GSPAN0 + k1 * 128],
                        transpose=True,
                    )

            def gram_mms(gt, tp_tile):
                for ck in range(NCK):
                    op = tp_tile[:, ck * 128 : (ck + 1) * 128]
                    nc.tensor.matmul(
                        gt[:], op, op, start=(ck == 0), stop=(ck == NCK - 1)
                    )

            # =============== L1 Grams + AR1 ===============
            tpgx = bp.tile([128, NCK * 128], BF16, tag="s2")
            gram_half(tpgx, zz, 0, 0, nc.sync)
            gram_half(tpgx, xy, 0, 64, nc.scalar)
            tpgy = bp.tile([128, NCK * 128], BF16, tag="s3")
            for k0, k1 in KSECS:  # z-half copied, not re-transposed
                nc.vector.tensor_copy(
                    tp3v(tpgy)[:, k0:k1, 0:64], tp3v(tpgx)[:, k0:k1, 0:64]
                )
            gram_half(tpgy, xy, 64, 64, nc.scalar)

            # ---------- weights via ACT hwdge queue ----------
            WT = {}
            for nm in wnames_bf:
                WT[nm] = wp.tile(WSHAPE[nm], BF16, tag=nm, name=nm)
                nc.scalar.dma_start(WT[nm][:], P[nm][:])
            for nm in wnames_f32:
                WT[nm] = wp.tile(WSHAPE[nm], F32, tag=nm, name=nm)
                nc.scalar.dma_start(WT[nm][:], P[nm][:])
            msk = sp.tile([128, 4], F32, tag="msk")
            nc.scalar.dma_start(msk[:], P["msk"][:])
            SEL0, SEL1 = msk[:, 2:3], msk[:, 3:4]
            MTOP, MBOT = msk[:, 0:1], msk[:, 1:2]


            arin = dp.tile([128, 4 * 128], F32, tag="arin")
            arout = dp.tile([128, 4 * 128], F32, tag="arout", addr_space="Shared")
            arin_sb = sp.tile([128, 4 * 128], F32, tag="arin_sb")
            g1sb = sp.tile([128, 4 * 128], F32, tag="g1sb")

            for gi, tp_t in enumerate([tpgx, tpgy]):
                gps = psC.tile([128, 128], F32, tag="psC")
                gram_mms(gps, tp_t)
                nc.vector.tensor_scalar_mul(
                    arin_sb[:, gi * 128 : (gi + 1) * 128], gps[:], SEL0
                )
                nc.vector.tensor_scalar_mul(
                    arin_sb[:, (2 + gi) * 128 : (3 + gi) * 128], gps[:], SEL1
                )
            nc.sync.dma_start(arin[:], arin_sb[:])
            nc.gpsimd.collective_compute(
                "AllReduce", ALU.add, replica_groups=[list(range(8))],
                ins=[arin.opt()], outs=[arout.opt()],
            )
            nc.sync.dma_start(g1sb[:], arout[:])
            # per-batch Gram selection
            gmy = sp.tile([128, 2 * 128], F32, tag="gmy")
            for gi in range(2):
                nc.vector.tensor_scalar_mul(
                    gmy[:, gi * 128 : (gi + 1) * 128],
                    g1sb[:, gi * 128 : (gi + 1) * 128], SEL0,
                )
                nc.vector.scalar_tensor_tensor(
                    gmy[:, gi * 128 : (gi + 1) * 128],
                    g1sb[:, (2 + gi) * 128 : (3 + gi) * 128], SEL1,
                    gmy[:, gi * 128 : (gi + 1) * 128],
                    op0=ALU.mult, op1=ALU.add,
                )

            def attn_module(m, G, qblk, kblk, WoTp, odd):
                """emit small-attn for module m from stack-Gram G [128, 128];
                writes W_oT into WoTp rows/cols r0."""
                wq_s = WT["wqm"][0:62, m * 64 : m * 64 + 62]
                wk_s = WT["wkm"][0:62, m * 64 : m * 64 + 62]
                pw_s = WT["pwm"][0:62, m * 64 : m * 64 + 62]

                def blk(tag, rblk, cblk):
                    if rblk == 0:
                        return G[0:62, cblk * 64 : cblk * 64 + 62]
                    t_ = sp.tile([128, 64], F32, tag="gblk_" + tag)
                    nc.sync.dma_start(
                        t_[0:62, 0:62], G[64:126, cblk * 64 : cblk * 64 + 62]
                    )
                    return t_[0:62, 0:62]

                gqk = blk("qk", qblk, kblk)
                gqq = blk("qq", qblk, qblk)
                gkk = blk("kk", kblk, kblk)

                def mm62(lhs, rhs, tag):
                    pp = psC.tile([128, 64], F32, tag="psC")
                    nc.tensor.matmul(pp[0:62, 0:62], lhs, rhs, start=True, stop=True)
                    ss = sp.tile([128, 64], F32, tag="t_" + tag, name="t_" + tag)
                    nc.vector.tensor_copy(ss[0:62, 0:62], pp[0:62, 0:62])
                    return ss

                T1 = mm62(gqk, wq_s, "T1")
                T2 = mm62(gqq, wq_s, "T2")
                T3 = mm62(gkk, wk_s, "T3")
                SKQ = mm62(wk_s, T1[0:62, 0:62], "SKQ")
                u2 = sp.tile([128, 64], F32, tag="u2")
                nc.vector.tensor_tensor(
                    u2[0:62, 0:62], wq_s, T2[0:62, 0:62], op=ALU.mult
                )
                u3 = sp.tile([128, 64], F32, tag="u3")
                nc.vector.tensor_tensor(
                    u3[0:62, 0:62], wk_s, T3[0:62, 0:62], op=ALU.mult
                )
                pq = psC.tile([128, 64], F32, tag="psC")
                nc.tensor.matmul(
                    pq[0:1, 0:62], WT["ones62"][0:62, 0:1], u2[0:62, 0:62],
                    start=True, stop=True,
                )
                pk = psC.tile([128, 64], F32, tag="psC")
                nc.tensor.matmul(
                    pk[0:62, 0:1], u3[0:62, 0:62], WT["ones62"][0:62, 0:1],
                    start=True, stop=True,
                )
                ik = sp.tile([128, 2], F32, tag="ik")
                nc.scalar.activation(ik[0:62, 0:1], pk[0:62, 0:1], AF.Sqrt)
                nc.vector.tensor_scalar_max(ik[0:62, 0:1], ik[0:62, 0:1], 1e-12)
                nc.vector.reciprocal(ik[0:62, 1:2], ik[0:62, 0:1])
                scd = sp.tile([128, 1], F32, tag="scd")
                nc.vector.tensor_tensor(
                    scd[0:62, 0:1], ik[0:62, 1:2], WT["rsc"][0:62, m : m + 1],
                    op=ALU.mult,
                )
                iq = sp.tile([1, 128], F32, tag="iq")
                nc.scalar.activation(iq[0:1, 0:62], pq[0:1, 0:62], AF.Sqrt)
                nc.vector.tensor_scalar_max(iq[0:1, 0:62], iq[0:1, 0:62], 1e-12)
                nc.vector.reciprocal(iq[0:1, 64:126], iq[0:1, 0:62])
                iqb = sp.tile([128, 64], F32, tag="iqb")
                nc.gpsimd.partition_broadcast(iqb[0:62, 0:62], iq[0:1, 64:126])
                L = sp.tile([128, 64], F32, tag="L")
                nc.vector.tensor_scalar_mul(L[0:62, 0:62], SKQ[0:62, 0:62], scd[0:62, 0:1])
                nc.vector.tensor_tensor(
                    L[0:62, 0:62], L[0:62, 0:62], iqb[0:62, 0:62], op=ALU.mult
                )
                A = sp.tile([128, 64], F32, tag="A")
                nc.vector.memset(A[:], 0.0)
                nc.vector.tensor_tensor(
                    L[0:62, 0:62], L[0:62, 0:62], WT["hmA"][0:62, 0:62], op=ALU.mult
                )
                nc.vector.tensor_tensor(
                    L[0:62, 0:62], L[0:62, 0:62], WT["hmB"][0:62, 0:62], op=ALU.add
                )
                mx = sp.tile([128, 2], F32, tag="mx")
                nc.vector.tensor_reduce(
                    mx[0:62, 0:1], L[0:62, 0:62], op=ALU.max, axis=mybir.AxisListType.X
                )
                nc.vector.tensor_scalar_mul(mx[0:62, 1:2], mx[0:62, 0:1], -1.0)
                nc.scalar.activation(
                    A[0:62, 0:62], L[0:62, 0:62], AF.Exp, bias=mx[0:62, 1:2]
                )
                sm = sp.tile([128, 2], F32, tag="sm")
                nc.vector.tensor_reduce(
                    sm[0:62, 0:1], A[0:62, 0:62], op=ALU.add, axis=mybir.AxisListType.X
                )
                nc.vector.reciprocal(sm[0:62, 1:2], sm[0:62, 0:1])
                nc.vector.tensor_scalar_mul(A[0:62, 0:62], A[0:62, 0:62], sm[0:62, 1:2])
                wps = psC.tile([128, 64], F32, tag="psC")
                if odd:
                    nc.tensor.matmul(
                        wps[64:126, 0:62], A[0:62, 0:62], pw_s,
                        start=True, stop=True, tile_position=(0, 64),
                    )
                    nc.vector.tensor_copy(WoTp[64:126, 64:126], wps[64:126, 0:62])
                else:
                    nc.tensor.matmul(
                        wps[0:62, 0:62], A[0:62, 0:62], pw_s, start=True, stop=True
                    )
                    nc.vector.tensor_copy(WoTp[0:62, 0:62], wps[0:62, 0:62])

            # WoT pair tiles
            WoT = {}
            for p in PAIRS:
                WoT[p] = sp.tile([128, 128], BF16, tag=f"WoT{p}", name=f"WoT{p}")
                nc.vector.memset(WoT[p][:], 0.0)

            # =============== P23: modules 2,3 (v from z) ===============
            v23 = bp.tile([128, FA], BF16, tag="s2")
            stream_v(v23, WT["wv23"], zz, 6, kp=64)
            pad_zero(v23)
            g23 = bp.tile([128, FA], BF16, tag="s3")
            conv_pe_gelu(g23, v23, load_mdw("m_dw1_23"), WT["c3_dw1_23"], 5)
            pad_zero(g23)
            mask_rows(g23, 5)
            # attn for both L1 pairs (AR1 finished during the conv above);
            # modules 0,1 emitted after pt23 so their chain latency hides there
            attn_module(2, gmy[:, 0:128], 1, 0, WoT["23"], odd=False)
            attn_module(3, gmy[:, 128:256], 1, 0, WoT["23"], odd=True)
            fafb = bp.tile([128, FA], BF16, tag="s1")  # reuses zz slot
            pair_tail(fafb, v23, g23, WoT["23"], WT["pb23"], xy, "23", 4)
            pad_zero(fafb)
            mask_rows(fafb, 4)

            # =============== P01: modules 0,1 ===============
            attn_module(0, gmy[:, 0:128], 1, 1, WoT["01"], odd=False)
            attn_module(1, gmy[:, 128:256], 1, 1, WoT["01"], odd=True)
            v01 = bp.tile([128, FA], BF16, tag="s2")
            stream_v(v01, WT["wv01"], xy, 4)
            pad_zero(v01)
            g01 = bp.tile([128, FA], BF16, tag="s3")
            conv_pe_gelu(g01, v01, load_mdw("m_dw1_01"), WT["c3_dw1_01"], 3)
            pad_zero(g01)
            mask_rows(g01, 3)
            # in-place residual: xa|yb overwrites the xy slab (resid read and
            # output write are the same STT op, chunk by chunk)
            xaYb = xy
            pair_tail(xaYb, v01, g01, WoT["01"], WT["pb01"], xy, "01", 2)
            pad_zero(xaYb)
            mask_rows(xaYb, 2)

            # =============== L2 Grams + AR2 (all sources resident) ===============
            tpa = bp.tile([128, NCK * 128], BF16, tag="s2")  # v01 dead
            gram_half(tpa, xaYb, 0, 0, nc.sync)
            gram_half(tpa, fafb, 0, 64, nc.sync)
            tpb = bp.tile([128, NCK * 128], BF16, tag="s3")  # g01 dead
            gram_half(tpb, xaYb, 64, 0, nc.sync)
            gram_half(tpb, fafb, 64, 64, nc.sync)
            arin2_sb = sp.tile([128, 4 * 128], F32, tag="arin_sb")
            for gi, tp_t in enumerate([tpa, tpb]):
                gps = psC.tile([128, 128], F32, tag="psC")
                gram_mms(gps, tp_t)
                nc.vector.tensor_scalar_mul(
                    arin2_sb[:, gi * 128 : (gi + 1) * 128], gps[:], SEL0
                )
                nc.vector.tensor_scalar_mul(
                    arin2_sb[:, (2 + gi) * 128 : (3 + gi) * 128], gps[:], SEL1
                )
            arin2 = dp.tile([128, 4 * 128], F32, tag="arin2")
            arout2 = dp.tile([128, 4 * 128], F32, tag="arout2", addr_space="Shared")
            nc.sync.dma_start(arin2[:], arin2_sb[:])
            nc.gpsimd.collective_compute(
                "AllReduce", ALU.add, replica_groups=[list(range(8))],
                ins=[arin2.opt()], outs=[arout2.opt()],
            )
            g2sb = sp.tile([128, 4 * 128], F32, tag="g1sb")
            nc.sync.dma_start(g2sb[:], arout2[:])
            gmy2 = sp.tile([128, 2 * 128], F32, tag="gmy")
            for gi in range(2):
                nc.vector.tensor_scalar_mul(
                    gmy2[:, gi * 128 : (gi + 1) * 128],
                    g2sb[:, gi * 128 : (gi + 1) * 128], SEL0,
                )
                nc.vector.scalar_tensor_tensor(
                    gmy2[:, gi * 128 : (gi + 1) * 128],
                    g2sb[:, (2 + gi) * 128 : (3 + gi) * 128], SEL1,
                    gmy2[:, gi * 128 : (gi + 1) * 128],
                    op0=ALU.mult, op1=ALU.add,
                )

            # =============== P45 (v45/g45 overlap AR2 latency) ===============
            v45 = bp.tile([128, FA], BF16, tag="s2")  # tpa dead after gA MMs
            stream_v(v45, WT["wv45"], fafb, 4)
            pad_zero(v45)
            g45 = bp.tile([128, FA], BF16, tag="s3")  # tpb dead after gB MMs
            conv_pe_gelu(g45, v45, load_mdw("m_dw1_45"), WT["c3_dw1_45"], 3)
            pad_zero(g45)
            mask_rows(g45, 3)
            # stacks are [xa|fa]: q=xa blk0, k=fa blk1
            attn_module(4, gmy2[:, 0:128], 0, 1, WoT["45"], odd=False)
            attn_module(5, gmy2[:, 128:256], 0, 1, WoT["45"], odd=True)
            t12 = bp.tile([128, FA], BF16, tag="s1")  # fafb dead after v45/gram
            pair_tail(t12, v45, g45, WoT["45"], WT["pb45"], xaYb, "45", 2)
            pad_zero(t12)
            mask_rows(t12, 2)

            # =============== LN stats (both chains) ===============
            lo2, hi2 = exr(2)
            lo1, hi1 = exr(1)
            L2p = 64 * 275  # 17600 staging per stat row
            stat_all_d = dp.tile([2, 2 * L2p], F32, tag="stat_all_d")

            sq = bp.tile([128, FA], BF16, tag="s2")  # reuses v45 slot
            for lo, hi in chunks((lo2, hi2), 4096):
                nc.scalar.activation(sq[:, lo:hi], t12[:, lo:hi], AF.Square)
            for si, srct in enumerate([t12, sq]):
                base = si * L2LEN
                for lo, hi in chunks((lo2, hi2), 512):
                    ps = psA.tile([128, 1024], F32, tag="psA")
                    nc.tensor.matmul(
                        ps[0:2, 0 : hi - lo], WT["lnones"][:],
                        srct[:, lo:hi], start=True, stop=True,
                    )
                    stch = rp.tile([2, 512], F32, tag="stch", bufs=1)
                    nc.scalar.copy(stch[:, 0 : hi - lo], ps[0:2, 0 : hi - lo])
                    nc.sync.dma_start(
                        stat_all_d[:, base + lo - lo2 : base + hi - lo2],
                        stch[:, 0 : hi - lo],
                    )

            # per-pixel inv-std / mu*inv rows for both chains, upfront
            row_ds = {}
            for ci in (0, 1):
                row_dX = dp.tile([1, 2 * L2p], BF16, tag=f"row_d{ci}")
                mu64 = sp.tile([64, 275], F32, tag="mu64")
                ms64 = sp.tile([64, 275], F32, tag="ms64")
                nc.sync.dma_start(
                    mu64[:],
                    stat_all_d[ci : ci + 1, 0:L2p].rearrange(
                        "o (p f) -> (o p) f", p=64
                    ),
                )
                nc.sync.dma_start(
                    ms64[:],
                    stat_all_d[ci : ci + 1, L2LEN : L2LEN + L2p].rearrange(
                        "o (p f) -> (o p) f", p=64
                    ),
                )
                var = sp.tile([64, 275], F32, tag="var64")
                nc.vector.tensor_tensor(var[:], mu64[:], mu64[:], op=ALU.mult)
                nc.vector.tensor_tensor(var[:], ms64[:], var[:], op=ALU.subtract)
                sd = sp.tile([64, 275], F32, tag="sd64")
                nc.vector.tensor_scalar_add(var[:], var[:], 1e-5)
                nc.scalar.activation(sd[:], var[:], AF.Sqrt)
                inv = sp.tile([64, 275], F32, tag="inv64")
                nc.vector.reciprocal(inv[:], sd[:])
                invb = sp.tile([64, 275], BF16, tag="invb64")
                nc.vector.tensor_copy(invb[:], inv[:])
                musb = sp.tile([64, 275], BF16, tag="musb64")
                nc.vector.tensor_tensor(var[:], mu64[:], inv[:], op=ALU.mult)
                nc.vector.tensor_copy(musb[:], var[:])
                nc.sync.dma_start(
                    row_dX[0:1, 0:L2p].rearrange("o (p f) -> (o p) f", p=64),
                    invb[:],
                )
                nc.sync.dma_start(
                    row_dX[0:1, L2p : 2 * L2p].rearrange("o (p f) -> (o p) f", p=64),
                    musb[:],
                )
                row_ds[ci] = row_dX

            # =============== FF per chain (ring-buffered ts) ===============
            for ci, cn in ((0, "a"), (1, "b")):
                r0 = 0 if ci == 0 else 64
                row_dX = row_ds[ci]
                th = {}
                w1ts = [WT[f"w1aug_{cn}0"], WT[f"w1aug_{cn}1"]]
                for h, hn in ((0, "0"), (1, "1")):
                    tht = bp.tile(
                        [128, FA], BF16, tag=("s3" if h == 0 else "s2"),
                        name=f"th{h}_{cn}",
                    )
                    th[h] = tht
                # w1 for both halves per chunk; ts built on the fly in a ring
                for lo, hi in chunks((lo2, hi2), 1024):
                    n = hi - lo
                    sbr = rp.tile([128, 1024], BF16, tag="sbr", bufs=2)
                    nc.scalar.dma_start(
                        sbr[:, 0:n],
                        row_dX[0:1, lo - lo2 : hi - lo2].partition_broadcast(128),
                    )
                    tsr = rp.tile([128, 1024], BF16, tag="tsr", bufs=2)
                    nc.vector.tensor_tensor(
                        tsr[r0 : r0 + 62, 0:n], t12[r0 : r0 + 62, lo:hi],
                        sbr[r0 : r0 + 62, 0:n], op=ALU.mult,
                    )
                    nc.sync.dma_start(
                        tsr[r0 + 62 : r0 + 63, 0:n],
                        row_dX[0:1, L2p + lo - lo2 : L2p + hi - lo2],
                    )
                    for h in (0, 1):
                        ps = psA.tile([128, 1024], F32, tag="psA")
                        for l2, h2 in chunks((lo, hi), NT):
                            nc.tensor.matmul(
                                ps[0:124, l2 - lo : h2 - lo],
                                w1ts[h][r0 : r0 + 63, :],
                                tsr[r0 : r0 + 63, l2 - lo : h2 - lo],
                                start=True, stop=True,
                            )
                        nc.scalar.activation(
                            th[h][0:124, lo:hi], ps[0:124, 0 : hi - lo], AF.Gelu,
                            bias=WT[f"c1b_{cn}{'0' if h == 0 else '1'}"][0:124, :],
                        )
                for h in (0, 1):
                    pad_zero(th[h])
                    mask_rows(th[h], 2, (0, 124))
                # conv h0/h1 + gelu + w2, chunk-wise (convs on PE)
                for lo, hi in chunks((lo1, hi1), 1024):
                    wps = psA.tile([128, 1024], F32, tag="psA")
                    for h, hn in ((0, "0"), (1, "1")):
                        ghc = rp.tile([128, 1024], BF16, tag="ghc", bufs=2)
                        cps = psA.tile([128, 1024], F32, tag="psA")
                        mats = load_mdw(f"m_ffdw_{cn}{hn}")
                        dw_taps_split(
                            cps, th[h], mats, WT[f"c3_ffdw_{cn}{hn}"],
                            lo, hi, start=True,
                        )
                        nc.scalar.activation(
                            ghc[0:124, 0 : hi - lo], cps[0:124, 0 : hi - lo],
                            AF.Gelu,
                        )
                        w2t = WT[f"w2h_{cn}{hn}"]
                        for l2, h2 in chunks((lo, hi), NT):
                            nc.tensor.matmul(
                                wps[r0 : r0 + 62, l2 - lo : h2 - lo],
                                w2t[0:124, 0:62],
                                ghc[0:124, l2 - lo : h2 - lo],
                                start=(h == 0), stop=False,
                                tile_position=(0, 64) if ci == 1 else None,
                            )
                        if h == 1:
                            # old t12 folded into the psum accumulation via the
                            # identity diag block of mats (block 6)
                            for l2, h2 in chunks((lo, hi), NT):
                                nc.tensor.matmul(
                                    wps[r0 : r0 + 62, l2 - lo : h2 - lo],
                                    mats[r0 : r0 + 62,
                                         6 * 128 + r0 : 6 * 128 + r0 + 62],
                                    t12[r0 : r0 + 62, l2:h2],
                                    start=False, stop=True,
                                )
                    nc.scalar.copy(
                        t12[r0 : r0 + 62, lo:hi], wps[r0 : r0 + 62, 0 : hi - lo]
                    )

            # =============== fusions + final ===============
            fab = bp.tile([128, FA], BF16, tag="s2")
            for lo, hi in chunks((lo1, hi1), 1024):
                ps = psA.tile([128, 1024], F32, tag="psA")
                for l2, h2 in chunks((lo, hi), NT):
                    nc.tensor.matmul(
                        ps[:, l2 - lo : h2 - lo], WT["wfut"][:], t12[:, l2:h2],
                        start=True, stop=False,
                    )
                    nc.tensor.matmul(
                        ps[:, l2 - lo : h2 - lo], WT["wfux"][:], xaYb[:, l2:h2],
                        start=False, stop=True,
                    )
                nc.scalar.add(fab[:, lo:hi], ps[:, 0 : hi - lo], WT["fucb"][:])
            pad_zero(fab)
            mask_rows(fab, 1)
            fin = bp.tile([128, FA], BF16, tag="s3")
            for lo, hi in chunks((lo1, hi1), 1024):
                ps = psA.tile([128, 1024], F32, tag="psA")
                for l2, h2 in chunks((lo, hi), NT):
                    nc.tensor.matmul(
                        ps[0:64, l2 - lo : h2 - lo], WT["outw"][:], fab[:, l2:h2],
                        start=True, stop=True,
                    )
                nc.scalar.add(fin[0:62, lo:hi], ps[0:62, 0 : hi - lo], WT["outb"][0:62, :])
            nc.sync.dma_start(fin[64:126, lo1:hi1], P["zc"][0:62, :])
            pad_zero(fin)
            mask_rows(fin, 1, (0, 62))
            # final 3x3 conv, direct DMA out in 2-row (516 col) pieces.
            # each 258-col row-piece goes to its own PSUM bank (offsets 0, 512)
            lo0, hi0 = exr(0)
            out3 = out_p[:].rearrange("c (r w) -> c r w", w=W)
            for lo, hi in chunks((lo0, hi0), 516):
                ps = psA.tile([128, 1024], F32, tag="psA")
                for t in range(9):
                    dy, dx = TAPS[t]
                    d = dy * RS + dx
                    for bi, (l2, h2) in enumerate(((lo, lo + 258), (lo + 258, hi))):
                        nc.tensor.matmul(
                            ps[0:64, bi * 512 : bi * 512 + 258],
                            WT["finw"][:, t * 64 : (t + 1) * 64],
                            fin[:, l2 + d : h2 + d],
                            start=(t == 0), stop=(t == 8),
                        )
                och = rp.tile([62, 516], F32, tag="och", bufs=1)
                ps3 = ps[:].rearrange("p (b c) -> p b c", c=512)[0:62, :, 0:258]
                nc.scalar.add(
                    och[:].rearrange("c (r s) -> c r s", s=RS), ps3,
                    WT["finb"][0:62, :],
                )
                r0 = (lo - lo0) // RS
                nc.sync.dma_start(
                    out3[:, r0 : r0 + 2, :],
                    och[:].rearrange("c (r s) -> c r s", s=RS)[:, :, 0:W],
                )

    nc.finalize()
    return nc


_NC_CACHE = {}


def _run(inputs, trace=False):
    if "nc" not in _NC_CACHE:
        _NC_CACHE["nc"] = build_nc()
    nc = _NC_CACHE["nc"]
    names = {
        a.name.removesuffix("_set")
        for a in nc.m.functions[0].allocations
        if getattr(a, "kind", None) == "ExternalInput"
    }
    in_maps = prep_host_inputs(inputs)
    in_maps = [{k: v for k, v in m.items() if k in names} for m in in_maps]
    res = run_bass_kernel_spmd(
        nc, in_maps, core_ids=list(range(8)), trace=trace
    )
    return assemble_output(res.results), res


def kernel(**inputs):
    out, _ = _run(inputs, trace=False)
    return out


# revision 24
# speedup vs baseline: 1.0567x; 1.0567x over previous
"""Bass/Trainium2 kernel for nn_Block_14010183320003 (MST++-style block).

Sharding: 8 cores = 2 batches x 4 row-quarters (64 owned rows each, halo 6).
Chain packing: partitions 0:62 = x-derived chain A, 64:126 = y-derived chain B.
Attention is computed from 124-dim input Gram matrices (no q/k tensors);
per-core partial Grams are summed with two ~256KB AllReduces.
Pixel-major Gram stacks are built ON DEVICE with batched DMA xbar transposes
(few big DMA_TRANSPOSE ops instead of per-chunk ones).
Row layout: stride 258 per row (2 zero pad cols) so 3x3 convs are free-axis
shifted reads; depthwise convs run as PE diag-matmul accumulation.
"""
import sys

sys.path.insert(0, "/opt/trn_rl_repo")
import numpy as np
import ml_dtypes

import concourse.bass as bass
import concourse.mybir as mybir
import concourse.tile as tile
import concourse.bacc as bacc
from concourse.bass_utils import run_bass_kernel_spmd

BF16, F32 = mybir.dt.bfloat16, mybir.dt.float32
AF = mybir.ActivationFunctionType
ALU = mybir.AluOpType
bf = ml_dtypes.bfloat16

C = 62
W = 256
RS = 258  # row stride (2 zero pad cols)
OWN = 64
HALO = 6
R = 76
ROFF = 1  # lead pad row at tile row 0 (zero pad for shifted reads)
RA = 78  # lead pad + 76 slab rows + 1 trailing spare
FA = RA * RS  # 20124
NT = 512  # matmul moving chunk
NH, DH = 2, 31
GSPAN0 = (HALO + ROFF) * RS  # owned-pixel span start = 1806
NCK = 129  # owned span 64*258 = 129 chunks of 128

TAPS = [(dy, dx) for dy in (-1, 0, 1) for dx in (-1, 0, 1)]
T6 = [0, 2, 3, 5, 6, 8]  # taps with dx != 0 -> PE diag matmuls
TD = [1, 4, 7]  # dx == 0 taps (dy=-1,0,+1) -> DVE scalar chains
D6 = [TAPS[t][0] * RS + TAPS[t][1] for t in T6]


def exr(e):
    """free range of extent e (slab rows [6-e, 70+e)), incl lead-row offset."""
    return ((HALO - e + ROFF) * RS, (HALO + OWN + e + ROFF) * RS)


def _slab(full, row0):
    """full [C, 256, 256] -> [C, R, RS] zero-padded slab (rows row0..row0+R)."""
    out = np.zeros((C, R, RS), np.float32)
    lo, hi = max(0, row0), min(256, row0 + R)
    out[:, lo - row0 : hi - row0, :W] = full[:, lo:hi]
    return out


def _to_bf(a):
    return np.ascontiguousarray(a.astype(bf))


def _diag_taps6(kc):
    """kc: [128, 9] -> [128, 7*128] bf16: 6 dx!=0 tap diags + identity."""
    out = np.zeros((128, 7 * 128), np.float32)
    ar = np.arange(128)
    for j, t in enumerate(T6):
        out[:, j * 128 : (j + 1) * 128][ar, ar] = kc[:, t]
    out[:, 6 * 128 : 7 * 128][ar, ar] = 1.0
    return _to_bf(out)


def _pair128(a_block, b_block):
    out = np.zeros((128, 128), np.float32)
    out[0:62, 0:62] = a_block
    out[64:126, 64:126] = b_block
    return _to_bf(out)


def prep_host_inputs(inputs):
    """Build the 8 per-core input maps."""
    inp = {k: np.asarray(v, dtype=np.float32) for k, v in inputs.items()}
    wq, wk, wv = inp["attn_wq"], inp["attn_wk"], inp["attn_wv"]
    pw, pb = inp["attn_pw"], inp["attn_pb"]
    dw1, dw2 = inp["attn_dw1"], inp["attn_dw2"]
    resc = inp["attn_rescale"]

    shared = {}
    # v-projection pair weights [128,128].
    # wv01/wv45: block-diag (K rows 0:62 -> cols 0:62 even module, K 64:126 ->
    # cols 64:126 odd module). wv23: single-z, K rows 0:62 feed BOTH col blocks.
    shared["wv01"] = _pair128(wv[0], wv[1])
    shared["wv45"] = _pair128(wv[4], wv[5])
    wv23 = np.zeros((128, 128), np.float32)
    wv23[0:62, 0:62] = wv[2]
    wv23[0:62, 64:126] = wv[3]
    shared["wv23"] = _to_bf(wv23)
    for pair, (me, mo) in {"01": (0, 1), "23": (2, 3), "45": (4, 5)}.items():
        pbv = np.zeros((128, 1), np.float32)
        pbv[0:62, 0] = pb[me]
        pbv[64:126, 0] = pb[mo]
        shared[f"pb{pair}"] = pbv
        for cn, dwk in (("1", dw1), ("2", dw2)):
            kc = np.zeros((128, 9), np.float32)
            for t, (dy, dx) in enumerate(TAPS):
                kc[0:62, t] = dwk[me][:, dy + 1, dx + 1]
                kc[64:126, t] = dwk[mo][:, dy + 1, dx + 1]
            shared[f"m_dw{cn}_{pair}"] = _diag_taps6(kc)
            shared[f"c3_dw{cn}_{pair}"] = np.ascontiguousarray(kc[:, TD])
    # attn small weights: [128, 6*64] col block m at m*64, rows 0:62
    for nm, src in (("wqm", wq), ("wkm", wk), ("pwm", pw)):
        t = np.zeros((128, 6 * 64), np.float32)
        for m in range(6):
            t[0:62, m * 64 : m * 64 + 62] = src[m]
        shared[nm] = t
    rs = np.zeros((128, 6), np.float32)
    for m in range(6):
        rs[0:62, m] = np.repeat(resc[m], DH)
    shared["rsc"] = rs
    ones62 = np.zeros((128, 1), np.float32)
    ones62[0:62, 0] = 1.0
    shared["ones62"] = ones62
    hmA = np.zeros((128, 64), np.float32)
    hmB = np.full((128, 64), -1e4, np.float32)
    for h in range(NH):
        hmA[h * DH : (h + 1) * DH, h * DH : (h + 1) * DH] = 1.0
        hmB[h * DH : (h + 1) * DH, h * DH : (h + 1) * DH] = 0.0
    shared["hmA"] = hmA
    shared["hmB"] = hmB
    lnones = np.zeros((128, 2), np.float32)
    lnones[0:62, 0] = 1.0 / C
    lnones[64:126, 1] = 1.0 / C
    shared["lnones"] = _to_bf(lnones)

    # FF weights. chain A uses ff index 0, chain B index 1.
    for ci, cname in ((0, "a"), (1, "b")):
        g, b = inp["ln_g"][ci], inp["ln_b"][ci]
        w1 = inp["ff_w1"][ci]  # [62, 248]
        w2 = inp["ff_w2"][ci]  # [248, 62]
        w1p = g[:, None] * w1
        c2 = w1p.sum(axis=0)  # [248]
        c1 = b @ w1  # [248]
        dwk = inp["ff_dw"][ci]  # [248, 3, 3]
        r0 = 0 if ci == 0 else 64
        for h in (0, 1):
            sl = slice(h * 124, (h + 1) * 124)
            wt = np.zeros((128, 124), np.float32)
            wt[r0 : r0 + 62, :] = w1p[:, sl]
            wt[r0 + 62, :] = -c2[sl]
            shared[f"w1aug_{cname}{h}"] = _to_bf(wt)
            cb = np.zeros((128, 1), np.float32)
            cb[0:124, 0] = c1[sl]
            shared[f"c1b_{cname}{h}"] = cb
            kc = np.zeros((128, 9), np.float32)
            for t, (dy, dx) in enumerate(TAPS):
                kc[0:124, t] = dwk[sl, dy + 1, dx + 1]
            shared[f"m_ffdw_{cname}{h}"] = _diag_taps6(kc)
            shared[f"c3_ffdw_{cname}{h}"] = np.ascontiguousarray(kc[:, TD])
            w2t = np.zeros((128, 128), np.float32)
            w2t[0:124, 0:62] = w2[sl, :]
            shared[f"w2h_{cname}{h}"] = _to_bf(w2t)

    # fusion weights: fab = Wfu_t^T @ t12 + Wfu_x^T @ xaYb + fucb
    wt_ = np.zeros((128, 128), np.float32)
    wt_[0:62, 0:62] = inp["fuc1_w"][:, 0:62].T
    wt_[64:126, 64:126] = inp["fuc2_w"][:, 0:62].T
    shared["wfut"] = _to_bf(wt_)
    wx_ = np.zeros((128, 128), np.float32)
    wx_[0:62, 0:62] = inp["fuc1_w"][:, 62:124].T
    wx_[64:126, 64:126] = inp["fuc2_w"][:, 62:124].T
    shared["wfux"] = _to_bf(wx_)
    fucb = np.zeros((128, 1), np.float32)
    fucb[0:62, 0] = inp["fuc1_b"]
    fucb[64:126, 0] = inp["fuc2_b"]
    shared["fucb"] = fucb
    ow = np.zeros((128, 64), np.float32)
    ow[0:62, 0:62] = inp["outc_w"][:, 0:62].T
    ow[64:126, 0:62] = inp["outc_w"][:, 62:124].T
    shared["outw"] = _to_bf(ow)
    ob = np.zeros((128, 1), np.float32)
    ob[0:62, 0] = inp["outc_b"]
    shared["outb"] = ob
    fw = np.zeros((128, 9 * 64), np.float32)
    for t, (dy, dx) in enumerate(TAPS):
        fw[0:62, t * 64 : t * 64 + 62] = inp["final_w"][:, 0:62, dy + 1, dx + 1].T
        fw[64:126, t * 64 : t * 64 + 62] = inp["final_w"][:, 62:124, dy + 1, dx + 1].T
    shared["finw"] = _to_bf(fw)
    fb_ = np.zeros((128, 1), np.float32)
    fb_[0:62, 0] = inp["final_b"]
    shared["finb"] = fb_

    for k in list(shared.keys()):
        if shared[k].dtype == np.float32:
            shared[k] = np.ascontiguousarray(shared[k])

    in_maps = []
    for b in range(2):
        xb, yb_, zb = inp["x"][b], inp["y"][b], inp["z"][b]
        for s in range(4):
            row0 = 64 * s - HALO
            xs, ys, zs = _slab(xb, row0), _slab(yb_, row0), _slab(zb, row0)
            m = {}
            xy = np.zeros((128, R, RS), np.float32)
            xy[0:62], xy[64:126] = xs, ys
            m["xy"] = _to_bf(xy.reshape(128, R * RS))
            zs64 = np.zeros((64, R * RS), np.float32)
            zs64[0:62] = zs.reshape(C, R * RS)
            m["zs"] = _to_bf(zs64)
            m["zc"] = _to_bf(zs[:, 5:71].reshape(62, 66 * RS))
            def pixmaj(t1, t2):
                g = np.zeros((OWN * W, 128), np.float32)
                g[:, 0:62] = t1[:, HALO : HALO + OWN, :W].reshape(C, -1).T
                g[:, 64:126] = t2[:, HALO : HALO + OWN, :W].reshape(C, -1).T
                return _to_bf(g)
            m["gzx"] = pixmaj(zs, xs)
            m["gzy"] = pixmaj(zs, ys)
            msk = np.zeros((128, 4), np.float32)
            msk[:, 0] = 0.0 if s == 0 else 1.0
            msk[:, 1] = 0.0 if s == 3 else 1.0
            msk[:, 2] = 1.0 if b == 0 else 0.0
            msk[:, 3] = 1.0 if b == 1 else 0.0
            m["msk"] = msk
            m.update(shared)
            in_maps.append(m)
    return in_maps


def assemble_output(results):
    out = np.zeros((2, C, 256, 256), np.float32)
    for b in range(2):
        for s in range(4):
            r = results[b * 4 + s]["out"]  # [62, 64*256]
            out[b, :, 64 * s : 64 * (s + 1), :] = r.reshape(C, OWN, W)
    return out


# ---------------------------------------------------------------------------
# device IR
# ---------------------------------------------------------------------------

PAIRS = ["01", "23", "45"]
L2LEN = exr(2)[1] - exr(2)[0]  # 17544


def build_nc():
    nc = bacc.Bacc(None, target_bir_lowering=False, debug=False)

    P = {}
    P["xy"] = nc.declare_dram_parameter("xy", [128, R * RS], BF16, isOutput=False)
    P["zs"] = nc.declare_dram_parameter("zs", [64, R * RS], BF16, isOutput=False)
    P["zc"] = nc.declare_dram_parameter("zc", [C, 66 * RS], BF16, isOutput=False)
    P["gzx"] = nc.declare_dram_parameter("gzx", [OWN * W, 128], BF16, isOutput=False)
    P["gzy"] = nc.declare_dram_parameter("gzy", [OWN * W, 128], BF16, isOutput=False)
    P["msk"] = nc.declare_dram_parameter("msk", [128, 4], F32, isOutput=False)
    wnames_bf = (
        [f"wv{p}" for p in PAIRS]
        + ["lnones"]
        + [f"w1aug_{c}{h}" for c in "ab" for h in "01"]
        + [f"w2h_{c}{h}" for c in "ab" for h in "01"]
        + ["wfut", "wfux", "outw", "finw"]
    )
    wnames_f32 = (
        [f"pb{p}" for p in PAIRS]
        + ["rsc", "wqm", "wkm", "pwm", "ones62", "hmA", "hmB"]
        + [f"c1b_{c}{h}" for c in "ab" for h in "01"]
        + [f"c3_dw{c}_{p}" for c in "12" for p in PAIRS]
        + [f"c3_ffdw_{c}{h}" for c in "ab" for h in "01"]
        + ["fucb", "outb", "finb"]
    )
    WSHAPE = {
        "wqm": [128, 6 * 64], "wkm": [128, 6 * 64], "pwm": [128, 6 * 64],
        "ones62": [128, 1], "lnones": [128, 2], "rsc": [128, 6],
        "outw": [128, 64], "finw": [128, 9 * 64],
        "hmA": [128, 64], "hmB": [128, 64],
        "wfut": [128, 128], "wfux": [128, 128],
        "fucb": [128, 1], "outb": [128, 1], "finb": [128, 1],
    }
    for p in PAIRS:
        WSHAPE[f"wv{p}"] = [128, 128]
        WSHAPE[f"pb{p}"] = [128, 1]
        for c in "12":
            WSHAPE[f"m_dw{c}_{p}"] = [128, 7 * 128]
            WSHAPE[f"c3_dw{c}_{p}"] = [128, 3]
    for c in "ab":
        for h in "01":
            WSHAPE[f"w1aug_{c}{h}"] = [128, 124]
            WSHAPE[f"m_ffdw_{c}{h}"] = [128, 7 * 128]
            WSHAPE[f"c3_ffdw_{c}{h}"] = [128, 3]
            WSHAPE[f"c1b_{c}{h}"] = [128, 1]
            WSHAPE[f"w2h_{c}{h}"] = [128, 128]
    mnames = [f"m_dw{c}_{p}" for c in "12" for p in PAIRS] + [
        f"m_ffdw_{c}{h}" for c in "ab" for h in "01"
    ]
    for nm in wnames_bf + mnames:
        P[nm] = nc.declare_dram_parameter(nm, WSHAPE[nm], BF16, isOutput=False)
    for nm in wnames_f32:
        P[nm] = nc.declare_dram_parameter(nm, WSHAPE[nm], F32, isOutput=False)
    out_p = nc.declare_dram_parameter("out", [C, OWN * W], F32, isOutput=True)

    with tile.TileContext(nc, num_cores=8) as tc:
        with (
            tc.tile_pool(name="w", bufs=1) as wp,
            tc.tile_pool(name="small", bufs=1) as sp,
            tc.tile_pool(name="big", bufs=1) as bp,
            tc.tile_pool(name="ring", bufs=3) as rp,
            tc.tile_pool(name="psA", bufs=3, space="PSUM") as psA,
            tc.tile_pool(name="psC", bufs=2, space="PSUM") as psC,
            tc.tile_pool(name="dram", bufs=1, space="DRAM") as dp,
        ):
            # ---------- input slabs first (big DMAs lead the SP queue) ----------
            zz = bp.tile([128, FA], BF16, tag="s1")
            nc.sync.dma_start(zz[0:64, RS : RS + R * RS], P["zs"][:])
            xy = bp.tile([128, FA], BF16, tag="xy")
            nc.sync.dma_start(xy[0:64, RS : RS + R * RS], P["xy"][0:64, :])
            nc.sync.dma_start(xy[64:128, RS : RS + R * RS], P["xy"][64:128, :])

            def load_mdw(nm):
                t_ = rp.tile([128, 7 * 128], BF16, tag="mdw", bufs=2, name=nm + "_l")
                nc.scalar.dma_start(t_[:], P[nm][:])
                return t_

            def chunks(rng, step):
                lo, hi = rng
                out = []
                while lo < hi:
                    out.append((lo, min(lo + step, hi)))
                    lo += step
                return out

            def pad_zero(t, lo_row=0, hi_row=RA):
                v = t[:].rearrange("p (r s) -> p r s", s=RS)[:, lo_row:hi_row, W : W + 2]
                nc.vector.memset(v, 0.0)

            def mask_rows(t, e, dtype_rows=(0, 128)):
                if e <= 0:
                    return
                r0, r1 = dtype_rows
                top = t[r0:r1, (HALO - e + ROFF) * RS : (HALO + ROFF) * RS]
                bot = t[r0:r1, (HALO + OWN + ROFF) * RS : (HALO + OWN + e + ROFF) * RS]
                nc.vector.tensor_scalar_mul(top, top, MTOP[r0:r1])
                nc.vector.tensor_scalar_mul(bot, bot, MBOT[r0:r1])

            def stream_v(dst, wv_t, src, e, kp=128):
                """dst[:, rng] = (wv_t.T @ src)[:, rng] via psA, ACT copy evac."""
                for lo, hi in chunks(exr(e), 1024):
                    ps = psA.tile([128, 1024], F32, tag="psA")
                    for l2, h2 in chunks((lo, hi), NT):
                        nc.tensor.matmul(
                            ps[:, l2 - lo : h2 - lo], wv_t[0:kp, :], src[0:kp, l2:h2],
                            start=True, stop=True,
                        )
                    nc.scalar.copy(dst[:, lo:hi], ps[:, 0 : hi - lo])

            def dw_taps_split(ps, src, mats7, cv, lo, hi, start):
                """accumulate depthwise 3x3 of src into ps[:, 0:hi-lo]:
                6 dx!=0 taps as PE diag-MMs; 3 dx=0 taps on DVE into a bf16
                scratch merged by an identity diag-MM (block 6 of mats7)."""
                n = hi - lo
                scr = rp.tile([128, 1024], BF16, tag="scr", bufs=2)
                nc.vector.tensor_scalar_mul(
                    scr[:, 0:n], src[:, lo - RS : hi - RS], cv[:, 0:1]
                )
                nc.vector.scalar_tensor_tensor(
                    scr[:, 0:n], src[:, lo:hi], cv[:, 1:2], scr[:, 0:n],
                    op0=ALU.mult, op1=ALU.add,
                )
                nc.vector.scalar_tensor_tensor(
                    scr[:, 0:n], src[:, lo + RS : hi + RS], cv[:, 2:3], scr[:, 0:n],
                    op0=ALU.mult, op1=ALU.add,
                )
                for l2, h2 in chunks((lo, hi), NT):
                    for j, d in enumerate(D6):
                        nc.tensor.matmul(
                            ps[:, l2 - lo : h2 - lo],
                            mats7[:, j * 128 : (j + 1) * 128],
                            src[:, l2 + d : h2 + d],
                            start=(start and j == 0), stop=False,
                        )
                    nc.tensor.matmul(
                        ps[:, l2 - lo : h2 - lo],
                        mats7[:, 6 * 128 : 7 * 128],
                        scr[:, l2 - lo : h2 - lo],
                        start=False, stop=True,
                    )

            def conv_pe_gelu(dst, src, mats, cv, e_out, part=None):
                """dst = gelu(dwconv(src)) over extent e_out (split PE/DVE).
                part=(i, n) emits only the i-th of n chunk groups."""
                cl = chunks(exr(e_out), 1024)
                if part is not None:
                    i, n = part
                    cl = cl[i * len(cl) // n : (i + 1) * len(cl) // n]
                for lo, hi in cl:
                    ps = psA.tile([128, 1024], F32, tag="psA")
                    dw_taps_split(ps, src, mats, cv, lo, hi, start=True)
                    nc.scalar.activation(dst[:, lo:hi], ps[:, 0 : hi - lo], AF.Gelu)

            def pair_tail(out_t, v_t, g_t, WoTp, pb_t, resid, pair, e_out):
                """out_t = WoT^T v + dw2(g) + pb + resid (split PE/DVE conv)."""
                mats = load_mdw(f"m_dw2_{pair}")
                cv = WT[f"c3_dw2_{pair}"]
                for lo, hi in chunks(exr(e_out), 1024):
                    ps = psA.tile([128, 1024], F32, tag="psA")
                    for l2, h2 in chunks((lo, hi), NT):
                        nc.tensor.matmul(
                            ps[:, l2 - lo : h2 - lo], WoTp[:], v_t[:, l2:h2],
                            start=True, stop=False,
                        )
                    dw_taps_split(ps, g_t, mats, cv, lo, hi, start=False)
                    nc.vector.scalar_tensor_tensor(
                        out_t[:, lo:hi], ps[:, 0 : hi - lo], pb_t[:],
                        resid[:, lo:hi], op0=ALU.add, op1=ALU.add,
                    )

            # ---------- gram helpers (batched xbar transposes) ----------
            KSECS = [(0, 33), (33, 65), (65, 97), (97, NCK)]

            def tp3v(tp_tile):
                return tp_tile[:].rearrange("p (k c) -> p k c", c=128)

            def gram_half(tp_tile, src, p0, c0, eng):
                """tp[p, k*128 + c0 + f] = src[p0+f, span k*128+p] via xbar
                transpose DMAs, sectioned so gram MMs pipeline per section."""
                tp3 = tp3v(tp_tile)
                for k0, k1 in KSECS:
                    eng.dma_start(
                        tp3[:, k0:k1, c0 : c0 + 64],
                        src[p0 : p0 + 64,
                            GSPAN0 + k0 * 128 : GSPAN0 + k1 * 128],
                        transpose=True,
                    )

            def gram_mms(gt, tp_tile, nck=NCK):
                for ck in range(nck):
                    op = tp_tile[:, ck * 128 : (ck + 1) * 128]
                    nc.tensor.matmul(
                        gt[:], op, op, start=(ck == 0), stop=(ck == nck - 1)
                    )

            # =============== L1 Grams + AR1 ===============
            # host-prepacked pixel-major stacks, loaded with 2 big DMAs each
            NCK1 = OWN * W // 128  # 128 chunks
            tpgx = bp.tile([128, NCK1 * 128], BF16, tag="s2")
            tpgy = bp.tile([128, NCK1 * 128], BF16, tag="s3")
            for tp_t, pn, eng in ((tpgx, "gzx", nc.sync), (tpgy, "gzy", nc.scalar)):
                src3 = P[pn][:].rearrange("(k p) c -> p k c", p=128)
                dst3 = tp_t[:].rearrange("p (k c) -> p k c", c=128)
                hk = NCK1 // 2
                eng.dma_start(dst3[:, 0:hk, :], src3[:, 0:hk, :])
                eng.dma_start(dst3[:, hk:, :], src3[:, hk:, :])

            # ---------- weights via ACT hwdge queue ----------
            WT = {}
            for nm in wnames_bf:
                WT[nm] = wp.tile(WSHAPE[nm], BF16, tag=nm, name=nm)
                nc.scalar.dma_start(WT[nm][:], P[nm][:])
            for nm in wnames_f32:
                WT[nm] = wp.tile(WSHAPE[nm], F32, tag=nm, name=nm)
                nc.scalar.dma_start(WT[nm][:], P[nm][:])
            msk = sp.tile([128, 4], F32, tag="msk")
            nc.scalar.dma_start(msk[:], P["msk"][:])
            SEL0, SEL1 = msk[:, 2:3], msk[:, 3:4]
            MTOP, MBOT = msk[:, 0:1], msk[:, 1:2]


            arin = dp.tile([128, 4 * 128], F32, tag="arin")
            arout = dp.tile([128, 4 * 128], F32, tag="arout", addr_space="Shared")
            arin_sb = sp.tile([128, 4 * 128], F32, tag="arin_sb")
            g1sb = sp.tile([128, 4 * 128], F32, tag="g1sb")

            for gi, tp_t in enumerate([tpgx, tpgy]):
                gps = psC.tile([128, 128], F32, tag="psC")
                gram_mms(gps, tp_t, nck=NCK1)
                nc.vector.tensor_scalar_mul(
                    arin_sb[:, gi * 128 : (gi + 1) * 128], gps[:], SEL0
                )
                nc.vector.tensor_scalar_mul(
                    arin_sb[:, (2 + gi) * 128 : (3 + gi) * 128], gps[:], SEL1
                )
            nc.sync.dma_start(arin[:], arin_sb[:])
            nc.gpsimd.collective_compute(
                "AllReduce", ALU.add, replica_groups=[list(range(8))],
                ins=[arin.opt()], outs=[arout.opt()],
            )
            nc.sync.dma_start(g1sb[:], arout[:])
            # per-batch Gram selection
            gmy = sp.tile([128, 2 * 128], F32, tag="gmy")
            for gi in range(2):
                nc.vector.tensor_scalar_mul(
                    gmy[:, gi * 128 : (gi + 1) * 128],
                    g1sb[:, gi * 128 : (gi + 1) * 128], SEL0,
                )
                nc.vector.scalar_tensor_tensor(
                    gmy[:, gi * 128 : (gi + 1) * 128],
                    g1sb[:, (2 + gi) * 128 : (3 + gi) * 128], SEL1,
                    gmy[:, gi * 128 : (gi + 1) * 128],
                    op0=ALU.mult, op1=ALU.add,
                )

            def attn_module(m, G, qblk, kblk, WoTp, odd):
                """emit small-attn for module m from stack-Gram G [128, 128];
                writes W_oT into WoTp rows/cols r0."""
                wq_s = WT["wqm"][0:62, m * 64 : m * 64 + 62]
                wk_s = WT["wkm"][0:62, m * 64 : m * 64 + 62]
                pw_s = WT["pwm"][0:62, m * 64 : m * 64 + 62]

                def blk(tag, rblk, cblk):
                    if rblk == 0:
                        return G[0:62, cblk * 64 : cblk * 64 + 62]
                    t_ = sp.tile([128, 64], F32, tag="gblk_" + tag)
                    nc.sync.dma_start(
                        t_[0:62, 0:62], G[64:126, cblk * 64 : cblk * 64 + 62]
                    )
                    return t_[0:62, 0:62]

                gqk = blk("qk", qblk, kblk)
                gqq = blk("qq", qblk, qblk)
                gkk = blk("kk", kblk, kblk)

                def mm62(lhs, rhs, tag):
                    pp = psC.tile([128, 64], F32, tag="psC")
                    nc.tensor.matmul(pp[0:62, 0:62], lhs, rhs, start=True, stop=True)
                    ss = sp.tile([128, 64], F32, tag="t_" + tag, name="t_" + tag)
                    nc.vector.tensor_copy(ss[0:62, 0:62], pp[0:62, 0:62])
                    return ss

                T1 = mm62(gqk, wq_s, "T1")
                T2 = mm62(gqq, wq_s, "T2")
                T3 = mm62(gkk, wk_s, "T3")
                SKQ = mm62(wk_s, T1[0:62, 0:62], "SKQ")
                u2 = sp.tile([128, 64], F32, tag="u2")
                nc.vector.tensor_tensor(
                    u2[0:62, 0:62], wq_s, T2[0:62, 0:62], op=ALU.mult
                )
                u3 = sp.tile([128, 64], F32, tag="u3")
                nc.vector.tensor_tensor(
                    u3[0:62, 0:62], wk_s, T3[0:62, 0:62], op=ALU.mult
                )
                pq = psC.tile([128, 64], F32, tag="psC")
                nc.tensor.matmul(
                    pq[0:1, 0:62], WT["ones62"][0:62, 0:1], u2[0:62, 0:62],
                    start=True, stop=True,
                )
                pk = psC.tile([128, 64], F32, tag="psC")
                nc.tensor.matmul(
                    pk[0:62, 0:1], u3[0:62, 0:62], WT["ones62"][0:62, 0:1],
                    start=True, stop=True,
                )
                ik = sp.tile([128, 2], F32, tag="ik")
                nc.scalar.activation(ik[0:62, 0:1], pk[0:62, 0:1], AF.Sqrt)
                nc.vector.tensor_scalar_max(ik[0:62, 0:1], ik[0:62, 0:1], 1e-12)
                nc.vector.reciprocal(ik[0:62, 1:2], ik[0:62, 0:1])
                scd = sp.tile([128, 1], F32, tag="scd")
                nc.vector.tensor_tensor(
                    scd[0:62, 0:1], ik[0:62, 1:2], WT["rsc"][0:62, m : m + 1],
                    op=ALU.mult,
                )
                iq = sp.tile([1, 128], F32, tag="iq")
                nc.scalar.activation(iq[0:1, 0:62], pq[0:1, 0:62], AF.Sqrt)
                nc.vector.tensor_scalar_max(iq[0:1, 0:62], iq[0:1, 0:62], 1e-12)
                nc.vector.reciprocal(iq[0:1, 64:126], iq[0:1, 0:62])
                iqb = sp.tile([128, 64], F32, tag="iqb")
                nc.gpsimd.partition_broadcast(iqb[0:62, 0:62], iq[0:1, 64:126])
                L = sp.tile([128, 64], F32, tag="L")
                nc.vector.tensor_scalar_mul(L[0:62, 0:62], SKQ[0:62, 0:62], scd[0:62, 0:1])
                nc.vector.tensor_tensor(
                    L[0:62, 0:62], L[0:62, 0:62], iqb[0:62, 0:62], op=ALU.mult
                )
                A = sp.tile([128, 64], F32, tag="A")
                nc.vector.memset(A[:], 0.0)
                nc.vector.tensor_tensor(
                    L[0:62, 0:62], L[0:62, 0:62], WT["hmA"][0:62, 0:62], op=ALU.mult
                )
                nc.vector.tensor_tensor(
                    L[0:62, 0:62], L[0:62, 0:62], WT["hmB"][0:62, 0:62], op=ALU.add
                )
                mx = sp.tile([128, 2], F32, tag="mx")
                nc.vector.tensor_reduce(
                    mx[0:62, 0:1], L[0:62, 0:62], op=ALU.max, axis=mybir.AxisListType.X
                )
                nc.vector.tensor_scalar_mul(mx[0:62, 1:2], mx[0:62, 0:1], -1.0)
                nc.scalar.activation(
                    A[0:62, 0:62], L[0:62, 0:62], AF.Exp, bias=mx[0:62, 1:2]
                )
                sm = sp.tile([128, 2], F32, tag="sm")
                nc.vector.tensor_reduce(
                    sm[0:62, 0:1], A[0:62, 0:62], op=ALU.add, axis=mybir.AxisListType.X
                )
                nc.vector.reciprocal(sm[0:62, 1:2], sm[0:62, 0:1])
                nc.vector.tensor_scalar_mul(A[0:62, 0:62], A[0:62, 0:62], sm[0:62, 1:2])
                wps = psC.tile([128, 64], F32, tag="psC")
                if odd:
                    nc.tensor.matmul(
                        wps[64:126, 0:62], A[0:62, 0:62], pw_s,
                        start=True, stop=True, tile_position=(0, 64),
                    )
                    nc.vector.tensor_copy(WoTp[64:126, 64:126], wps[64:126, 0:62])
                else:
                    nc.tensor.matmul(
                        wps[0:62, 0:62], A[0:62, 0:62], pw_s, start=True, stop=True
                    )
                    nc.vector.tensor_copy(WoTp[0:62, 0:62], wps[0:62, 0:62])

            # WoT pair tiles
            WoT = {}
            for p in PAIRS:
                WoT[p] = sp.tile([128, 128], BF16, tag=f"WoT{p}", name=f"WoT{p}")
                nc.vector.memset(WoT[p][:], 0.0)

            # =============== P23: modules 2,3 (v from z) ===============
            v23 = bp.tile([128, FA], BF16, tag="s2")
            stream_v(v23, WT["wv23"], zz, 6, kp=64)
            pad_zero(v23)
            g23 = bp.tile([128, FA], BF16, tag="s3")
            conv_pe_gelu(g23, v23, load_mdw("m_dw1_23"), WT["c3_dw1_23"], 5)
            pad_zero(g23)
            mask_rows(g23, 5)
            # attn for both L1 pairs (AR1 finished during the conv above);
            # modules 0,1 emitted after pt23 so their chain latency hides there
            attn_module(2, gmy[:, 0:128], 1, 0, WoT["23"], odd=False)
            attn_module(3, gmy[:, 128:256], 1, 0, WoT["23"], odd=True)
            fafb = bp.tile([128, FA], BF16, tag="s1")  # reuses zz slot
            pair_tail(fafb, v23, g23, WoT["23"], WT["pb23"], xy, "23", 4)
            pad_zero(fafb)
            mask_rows(fafb, 4)

            # =============== P01: modules 0,1 ===============
            attn_module(0, gmy[:, 0:128], 1, 1, WoT["01"], odd=False)
            attn_module(1, gmy[:, 128:256], 1, 1, WoT["01"], odd=True)
            v01 = bp.tile([128, FA], BF16, tag="s2")
            stream_v(v01, WT["wv01"], xy, 4)
            pad_zero(v01)
            g01 = bp.tile([128, FA], BF16, tag="s3")
            conv_pe_gelu(g01, v01, load_mdw("m_dw1_01"), WT["c3_dw1_01"], 3)
            pad_zero(g01)
            mask_rows(g01, 3)
            # in-place residual: xa|yb overwrites the xy slab (resid read and
            # output write are the same STT op, chunk by chunk)
            xaYb = xy
            pair_tail(xaYb, v01, g01, WoT["01"], WT["pb01"], xy, "01", 2)
            pad_zero(xaYb)
            mask_rows(xaYb, 2)

            # =============== L2 Grams + AR2 (all sources resident) ===============
            tpa = bp.tile([128, NCK * 128], BF16, tag="s2")  # v01 dead
            gram_half(tpa, xaYb, 0, 0, nc.sync)
            gram_half(tpa, fafb, 0, 64, nc.scalar)
            tpb = bp.tile([128, NCK * 128], BF16, tag="s3")  # g01 dead
            gram_half(tpb, xaYb, 64, 0, nc.sync)
            gram_half(tpb, fafb, 64, 64, nc.scalar)
            arin2_sb = sp.tile([128, 4 * 128], F32, tag="arin_sb")
            for gi, tp_t in enumerate([tpa, tpb]):
                gps = psC.tile([128, 128], F32, tag="psC")
                gram_mms(gps, tp_t)
                nc.vector.tensor_scalar_mul(
                    arin2_sb[:, gi * 128 : (gi + 1) * 128], gps[:], SEL0
                )
                nc.vector.tensor_scalar_mul(
                    arin2_sb[:, (2 + gi) * 128 : (3 + gi) * 128], gps[:], SEL1
                )
            arin2 = dp.tile([128, 4 * 128], F32, tag="arin2")
            arout2 = dp.tile([128, 4 * 128], F32, tag="arout2", addr_space="Shared")
            nc.sync.dma_start(arin2[:], arin2_sb[:])
            nc.gpsimd.collective_compute(
                "AllReduce", ALU.add, replica_groups=[list(range(8))],
                ins=[arin2.opt()], outs=[arout2.opt()],
            )
            g2sb = sp.tile([128, 4 * 128], F32, tag="g1sb")
            nc.sync.dma_start(g2sb[:], arout2[:])
            gmy2 = sp.tile([128, 2 * 128], F32, tag="gmy")
            for gi in range(2):
                nc.vector.tensor_scalar_mul(
                    gmy2[:, gi * 128 : (gi + 1) * 128],
                    g2sb[:, gi * 128 : (gi + 1) * 128], SEL0,
                )
                nc.vector.scalar_tensor_tensor(
                    gmy2[:, gi * 128 : (gi + 1) * 128],
                    g2sb[:, (2 + gi) * 128 : (3 + gi) * 128], SEL1,
                    gmy2[:, gi * 128 : (gi + 1) * 128],
                    op0=ALU.mult, op1=ALU.add,
                )

            # =============== P45 (v45/g45 overlap AR2 latency) ===============
            v45 = bp.tile([128, FA], BF16, tag="s2")  # tpa dead after gA MMs
            stream_v(v45, WT["wv45"], fafb, 4)
            pad_zero(v45)
            g45 = bp.tile([128, FA], BF16, tag="s3")  # tpb dead after gB MMs
            mdw45 = load_mdw("m_dw1_45")
            conv_pe_gelu(g45, v45, mdw45, WT["c3_dw1_45"], 3, part=(0, 2))
            # stacks are [xa|fa]: q=xa blk0, k=fa blk1 (chain hides in conv 2nd half)
            attn_module(4, gmy2[:, 0:128], 0, 1, WoT["45"], odd=False)
            attn_module(5, gmy2[:, 128:256], 0, 1, WoT["45"], odd=True)
            conv_pe_gelu(g45, v45, mdw45, WT["c3_dw1_45"], 3, part=(1, 2))
            pad_zero(g45)
            mask_rows(g45, 3)
            t12 = bp.tile([128, FA], BF16, tag="s1")  # fafb dead after v45/gram
            pair_tail(t12, v45, g45, WoT["45"], WT["pb45"], xaYb, "45", 2)
            pad_zero(t12)
            mask_rows(t12, 2)

            # =============== LN stats (both chains) ===============
            lo2, hi2 = exr(2)
            lo1, hi1 = exr(1)
            L2p = 64 * 275  # 17600 staging per stat row
            stat_all_d = dp.tile([2, 2 * L2p], F32, tag="stat_all_d")

            sq = bp.tile([128, FA], BF16, tag="s2")  # reuses v45 slot
            for lo, hi in chunks((lo2, hi2), 4096):
                nc.scalar.activation(sq[:, lo:hi], t12[:, lo:hi], AF.Square)
            for si, srct in enumerate([t12, sq]):
                base = si * L2LEN
                for lo, hi in chunks((lo2, hi2), 512):
                    ps = psA.tile([128, 1024], F32, tag="psA")
                    nc.tensor.matmul(
                        ps[0:2, 0 : hi - lo], WT["lnones"][:],
                        srct[:, lo:hi], start=True, stop=True,
                    )
                    stch = rp.tile([2, 512], F32, tag="stch", bufs=1)
                    nc.scalar.copy(stch[:, 0 : hi - lo], ps[0:2, 0 : hi - lo])
                    nc.sync.dma_start(
                        stat_all_d[:, base + lo - lo2 : base + hi - lo2],
                        stch[:, 0 : hi - lo],
                    )

            # per-pixel inv-std / mu*inv rows for both chains, upfront
            row_ds = {}
            for ci in (0, 1):
                row_dX = dp.tile([1, 2 * L2p], BF16, tag=f"row_d{ci}")
                mu64 = sp.tile([64, 275], F32, tag="mu64")
                ms64 = sp.tile([64, 275], F32, tag="ms64")
                nc.sync.dma_start(
                    mu64[:],
                    stat_all_d[ci : ci + 1, 0:L2p].rearrange(
                        "o (p f) -> (o p) f", p=64
                    ),
                )
                nc.sync.dma_start(
                    ms64[:],
                    stat_all_d[ci : ci + 1, L2LEN : L2LEN + L2p].rearrange(
                        "o (p f) -> (o p) f", p=64
                    ),
                )
                var = sp.tile([64, 275], F32, tag="var64")
                nc.vector.tensor_tensor(var[:], mu64[:], mu64[:], op=ALU.mult)
                nc.vector.tensor_tensor(var[:], ms64[:], var[:], op=ALU.subtract)
                sd = sp.tile([64, 275], F32, tag="sd64")
                nc.vector.tensor_scalar_add(var[:], var[:], 1e-5)
                nc.scalar.activation(sd[:], var[:], AF.Sqrt)
                inv = sp.tile([64, 275], F32, tag="inv64")
                nc.vector.reciprocal(inv[:], sd[:])
                invb = sp.tile([64, 275], BF16, tag="invb64")
                nc.vector.tensor_copy(invb[:], inv[:])
                musb = sp.tile([64, 275], BF16, tag="musb64")
                nc.vector.tensor_tensor(var[:], mu64[:], inv[:], op=ALU.mult)
                nc.vector.tensor_copy(musb[:], var[:])
                nc.sync.dma_start(
                    row_dX[0:1, 0:L2p].rearrange("o (p f) -> (o p) f", p=64),
                    invb[:],
                )
                nc.sync.dma_start(
                    row_dX[0:1, L2p : 2 * L2p].rearrange("o (p f) -> (o p) f", p=64),
                    musb[:],
                )
                row_ds[ci] = row_dX

            # =============== FF per chain (ring-buffered ts) ===============
            for ci, cn in ((0, "a"), (1, "b")):
                r0 = 0 if ci == 0 else 64
                row_dX = row_ds[ci]
                th = {}
                w1ts = [WT[f"w1aug_{cn}0"], WT[f"w1aug_{cn}1"]]
                for h, hn in ((0, "0"), (1, "1")):
                    tht = bp.tile(
                        [128, FA], BF16, tag=("s3" if h == 0 else "s2"),
                        name=f"th{h}_{cn}",
                    )
                    th[h] = tht
                # w1 for both halves per chunk; ts built on the fly in a ring
                for lo, hi in chunks((lo2, hi2), 1024):
                    n = hi - lo
                    sbr = rp.tile([128, 1024], BF16, tag="sbr", bufs=1)
                    nc.scalar.dma_start(
                        sbr[:, 0:n],
                        row_dX[0:1, lo - lo2 : hi - lo2].partition_broadcast(128),
                    )
                    tsr = rp.tile([128, 1024], BF16, tag="tsr", bufs=2)
                    nc.vector.tensor_tensor(
                        tsr[r0 : r0 + 62, 0:n], t12[r0 : r0 + 62, lo:hi],
                        sbr[r0 : r0 + 62, 0:n], op=ALU.mult,
                    )
                    nc.sync.dma_start(
                        tsr[r0 + 62 : r0 + 63, 0:n],
                        row_dX[0:1, L2p + lo - lo2 : L2p + hi - lo2],
                    )
                    for h in (0, 1):
                        ps = psA.tile([128, 1024], F32, tag="psA")
                        for l2, h2 in chunks((lo, hi), NT):
                            nc.tensor.matmul(
                                ps[0:124, l2 - lo : h2 - lo],
                                w1ts[h][r0 : r0 + 63, :],
                                tsr[r0 : r0 + 63, l2 - lo : h2 - lo],
                                start=True, stop=True,
                            )
                        nc.scalar.activation(
                            th[h][0:124, lo:hi], ps[0:124, 0 : hi - lo], AF.Gelu,
                            bias=WT[f"c1b_{cn}{'0' if h == 0 else '1'}"][0:124, :],
                        )
                for h in (0, 1):
                    pad_zero(th[h])
                    mask_rows(th[h], 2, (0, 124))
                # conv h0/h1 + gelu + w2, chunk-wise (convs on PE)
                for lo, hi in chunks((lo1, hi1), 1024):
                    wps = psA.tile([128, 1024], F32, tag="psA")
                    for h, hn in ((0, "0"), (1, "1")):
                        ghc = rp.tile([128, 1024], BF16, tag="ghc", bufs=2)
                        cps = psA.tile([128, 1024], F32, tag="psA")
                        mats = load_mdw(f"m_ffdw_{cn}{hn}")
                        dw_taps_split(
                            cps, th[h], mats, WT[f"c3_ffdw_{cn}{hn}"],
                            lo, hi, start=True,
                        )
                        nc.scalar.activation(
                            ghc[0:124, 0 : hi - lo], cps[0:124, 0 : hi - lo],
                            AF.Gelu,
                        )
                        w2t = WT[f"w2h_{cn}{hn}"]
                        for l2, h2 in chunks((lo, hi), NT):
                            nc.tensor.matmul(
                                wps[r0 : r0 + 62, l2 - lo : h2 - lo],
                                w2t[0:124, 0:62],
                                ghc[0:124, l2 - lo : h2 - lo],
                                start=(h == 0), stop=(h == 1),
                                tile_position=(0, 64) if ci == 1 else None,
                            )
                    # t12 += w2 out (in place, one rounding)
                    nc.vector.scalar_tensor_tensor(
                        t12[r0 : r0 + 62, lo:hi], wps[r0 : r0 + 62, 0 : hi - lo],
                        1.0, t12[r0 : r0 + 62, lo:hi], op0=ALU.mult, op1=ALU.add,
                    )

            # =============== fusions + final ===============
            fab = bp.tile([128, FA], BF16, tag="s2")
            for lo, hi in chunks((lo1, hi1), 1024):
                ps = psA.tile([128, 1024], F32, tag="psA")
                for l2, h2 in chunks((lo, hi), NT):
                    nc.tensor.matmul(
                        ps[:, l2 - lo : h2 - lo], WT["wfut"][:], t12[:, l2:h2],
                        start=True, stop=False,
                    )
                    nc.tensor.matmul(
                        ps[:, l2 - lo : h2 - lo], WT["wfux"][:], xaYb[:, l2:h2],
                        start=False, stop=True,
                    )
                nc.scalar.add(fab[:, lo:hi], ps[:, 0 : hi - lo], WT["fucb"][:])
            pad_zero(fab)
            mask_rows(fab, 1)
            fin = bp.tile([128, FA], BF16, tag="s3")
            for lo, hi in chunks((lo1, hi1), 1024):
                ps = psA.tile([128, 1024], F32, tag="psA")
                for l2, h2 in chunks((lo, hi), NT):
                    nc.tensor.matmul(
                        ps[0:64, l2 - lo : h2 - lo], WT["outw"][:], fab[:, l2:h2],
                        start=True, stop=True,
                    )
                nc.scalar.add(fin[0:62, lo:hi], ps[0:62, 0 : hi - lo], WT["outb"][0:62, :])
            nc.sync.dma_start(fin[64:126, lo1:hi1], P["zc"][0:62, :])
            pad_zero(fin)
            mask_rows(fin, 1, (0, 62))
            # final 3x3 conv, direct DMA out in 2-row (516 col) pieces.
            # each 258-col row-piece goes to its own PSUM bank (offsets 0, 512)
            lo0, hi0 = exr(0)
            out3 = out_p[:].rearrange("c (r w) -> c r w", w=W)
            for lo, hi in chunks((lo0, hi0), 516):
                ps = psA.tile([128, 1024], F32, tag="psA")
                for t in range(9):
                    dy, dx = TAPS[t]
                    d = dy * RS + dx
                    for bi, (l2, h2) in enumerate(((lo, lo + 258), (lo + 258, hi))):
                        nc.tensor.matmul(
                            ps[0:64, bi * 512 : bi * 512 + 258],
                            WT["finw"][:, t * 64 : (t + 1) * 64],
                            fin[:, l2 + d : h2 + d],
                            start=(t == 0), stop=(t == 8),
                        )
                och = rp.tile([62, 516], F32, tag="och", bufs=2)
                ps3 = ps[:].rearrange("p (b c) -> p b c", c=512)[0:62, :, 0:258]
                nc.scalar.add(
                    och[:].rearrange("c (r s) -> c r s", s=RS), ps3,
                    WT["finb"][0:62, :],
                )
                r0 = (lo - lo0) // RS
                nc.sync.dma_start(
                    out3[:, r0 : r0 + 2, :],
                    och[:].rearrange("c (r s) -> c r s", s=RS)[:, :, 0:W],
                )

    nc.finalize()
    return nc


_NC_CACHE = {}


def _run(inputs, trace=False):
    if "nc" not in _NC_CACHE:
        _NC_CACHE["nc"] = build_nc()
    nc = _NC_CACHE["nc"]
    names = {
        a.name.removesuffix("_set")
        for a in nc.m.functions[0].allocations
        if getattr(a, "kind", None) == "ExternalInput"
    }
    in_maps = prep_host_inputs(inputs)
    in_maps = [{k: v for k, v in m.items() if k in names} for m in in_maps]
    res = run_bass_kernel_spmd(
        nc, in_maps, core_ids=list(range(8)), trace=trace
    )
    return assemble_output(res.results), res


def kernel(**inputs):
    out, _ = _run(inputs, trace=False)
    return out


# revision 25
# speedup vs baseline: 1.0715x; 1.0141x over previous
"""Bass/Trainium2 kernel for nn_Block_14010183320003 (MST++-style block).

Sharding: 8 cores = 2 batches x 4 row-quarters (64 owned rows each, halo 6).
Chain packing: partitions 0:62 = x-derived chain A, 64:126 = y-derived chain B.
Attention is computed from 124-dim input Gram matrices (no q/k tensors);
per-core partial Grams are summed with two ~256KB AllReduces.
Pixel-major Gram stacks are built ON DEVICE with batched DMA xbar transposes
(few big DMA_TRANSPOSE ops instead of per-chunk ones).
Row layout: stride 258 per row (2 zero pad cols) so 3x3 convs are free-axis
shifted reads; depthwise convs run as PE diag-matmul accumulation.
"""
import sys

sys.path.insert(0, "/opt/trn_rl_repo")
import numpy as np
import ml_dtypes

import concourse.bass as bass
import concourse.mybir as mybir
import concourse.tile as tile
import concourse.bacc as bacc
from concourse.bass_utils import run_bass_kernel_spmd

BF16, F32 = mybir.dt.bfloat16, mybir.dt.float32
AF = mybir.ActivationFunctionType
ALU = mybir.AluOpType
bf = ml_dtypes.bfloat16

C = 62
W = 256
RS = 258  # row stride (2 zero pad cols)
OWN = 64
HALO = 6
R = 76
ROFF = 1  # lead pad row at tile row 0 (zero pad for shifted reads)
RA = 78  # lead pad + 76 slab rows + 1 trailing spare
FA = RA * RS  # 20124
NT = 512  # matmul moving chunk
NH, DH = 2, 31
GSPAN0 = (HALO + ROFF) * RS  # owned-pixel span start = 1806
NCK = 129  # owned span 64*258 = 129 chunks of 128

TAPS = [(dy, dx) for dy in (-1, 0, 1) for dx in (-1, 0, 1)]
T6 = [0, 2, 3, 5, 6, 8]  # taps with dx != 0 -> PE diag matmuls
TD = [1, 4, 7]  # dx == 0 taps (dy=-1,0,+1) -> DVE scalar chains
D6 = [TAPS[t][0] * RS + TAPS[t][1] for t in T6]


def exr(e):
    """free range of extent e (slab rows [6-e, 70+e)), incl lead-row offset."""
    return ((HALO - e + ROFF) * RS, (HALO + OWN + e + ROFF) * RS)


def _slab(full, row0):
    """full [C, 256, 256] -> [C, R, RS] zero-padded slab (rows row0..row0+R)."""
    out = np.zeros((C, R, RS), np.float32)
    lo, hi = max(0, row0), min(256, row0 + R)
    out[:, lo - row0 : hi - row0, :W] = full[:, lo:hi]
    return out


def _to_bf(a):
    return np.ascontiguousarray(a.astype(bf))


def _diag_taps6(kc):
    """kc: [128, 9] -> [128, 7*128] bf16: 6 dx!=0 tap diags + identity."""
    out = np.zeros((128, 7 * 128), np.float32)
    ar = np.arange(128)
    for j, t in enumerate(T6):
        out[:, j * 128 : (j + 1) * 128][ar, ar] = kc[:, t]
    out[:, 6 * 128 : 7 * 128][ar, ar] = 1.0
    return _to_bf(out)


def _pair128(a_block, b_block):
    out = np.zeros((128, 128), np.float32)
    out[0:62, 0:62] = a_block
    out[64:126, 64:126] = b_block
    return _to_bf(out)


def prep_host_inputs(inputs):
    """Build the 8 per-core input maps."""
    inp = {k: np.asarray(v, dtype=np.float32) for k, v in inputs.items()}
    wq, wk, wv = inp["attn_wq"], inp["attn_wk"], inp["attn_wv"]
    pw, pb = inp["attn_pw"], inp["attn_pb"]
    dw1, dw2 = inp["attn_dw1"], inp["attn_dw2"]
    resc = inp["attn_rescale"]

    shared = {}
    # v-projection pair weights [128,128].
    # wv01/wv45: block-diag (K rows 0:62 -> cols 0:62 even module, K 64:126 ->
    # cols 64:126 odd module). wv23: single-z, K rows 0:62 feed BOTH col blocks.
    shared["wv01"] = _pair128(wv[0], wv[1])
    shared["wv45"] = _pair128(wv[4], wv[5])
    wv23 = np.zeros((128, 128), np.float32)
    wv23[0:62, 0:62] = wv[2]
    wv23[0:62, 64:126] = wv[3]
    shared["wv23"] = _to_bf(wv23)
    for pair, (me, mo) in {"01": (0, 1), "23": (2, 3), "45": (4, 5)}.items():
        pbv = np.zeros((128, 1), np.float32)
        pbv[0:62, 0] = pb[me]
        pbv[64:126, 0] = pb[mo]
        shared[f"pb{pair}"] = pbv
        for cn, dwk in (("1", dw1), ("2", dw2)):
            kc = np.zeros((128, 9), np.float32)
            for t, (dy, dx) in enumerate(TAPS):
                kc[0:62, t] = dwk[me][:, dy + 1, dx + 1]
                kc[64:126, t] = dwk[mo][:, dy + 1, dx + 1]
            shared[f"m_dw{cn}_{pair}"] = _diag_taps6(kc)
            shared[f"c3_dw{cn}_{pair}"] = np.ascontiguousarray(kc[:, TD])
    # attn small weights: [128, 6*64] col block m at m*64, rows 0:62
    for nm, src in (("wqm", wq), ("wkm", wk), ("pwm", pw)):
        t = np.zeros((128, 6 * 64), np.float32)
        for m in range(6):
            t[0:62, m * 64 : m * 64 + 62] = src[m]
        shared[nm] = t
    rs = np.zeros((128, 6), np.float32)
    for m in range(6):
        rs[0:62, m] = np.repeat(resc[m], DH)
    shared["rsc"] = rs
    ones62 = np.zeros((128, 1), np.float32)
    ones62[0:62, 0] = 1.0
    shared["ones62"] = ones62
    hmA = np.zeros((128, 64), np.float32)
    hmB = np.full((128, 64), -1e4, np.float32)
    for h in range(NH):
        hmA[h * DH : (h + 1) * DH, h * DH : (h + 1) * DH] = 1.0
        hmB[h * DH : (h + 1) * DH, h * DH : (h + 1) * DH] = 0.0
    shared["hmA"] = hmA
    shared["hmB"] = hmB
    lnones = np.zeros((128, 2), np.float32)
    lnones[0:62, 0] = 1.0 / C
    lnones[64:126, 1] = 1.0 / C
    shared["lnones"] = _to_bf(lnones)

    # FF weights. chain A uses ff index 0, chain B index 1.
    for ci, cname in ((0, "a"), (1, "b")):
        g, b = inp["ln_g"][ci], inp["ln_b"][ci]
        w1 = inp["ff_w1"][ci]  # [62, 248]
        w2 = inp["ff_w2"][ci]  # [248, 62]
        w1p = g[:, None] * w1
        c2 = w1p.sum(axis=0)  # [248]
        c1 = b @ w1  # [248]
        dwk = inp["ff_dw"][ci]  # [248, 3, 3]
        r0 = 0 if ci == 0 else 64
        for h in (0, 1):
            sl = slice(h * 124, (h + 1) * 124)
            wt = np.zeros((128, 124), np.float32)
            wt[r0 : r0 + 62, :] = w1p[:, sl]
            wt[r0 + 62, :] = -c2[sl]
            shared[f"w1aug_{cname}{h}"] = _to_bf(wt)
            cb = np.zeros((128, 1), np.float32)
            cb[0:124, 0] = c1[sl]
            shared[f"c1b_{cname}{h}"] = cb
            kc = np.zeros((128, 9), np.float32)
            for t, (dy, dx) in enumerate(TAPS):
                kc[0:124, t] = dwk[sl, dy + 1, dx + 1]
            shared[f"m_ffdw_{cname}{h}"] = _diag_taps6(kc)
            shared[f"c3_ffdw_{cname}{h}"] = np.ascontiguousarray(kc[:, TD])
            w2t = np.zeros((128, 128), np.float32)
            w2t[0:124, 0:62] = w2[sl, :]
            shared[f"w2h_{cname}{h}"] = _to_bf(w2t)

    # fusion weights: fab = Wfu_t^T @ t12 + Wfu_x^T @ xaYb + fucb
    wt_ = np.zeros((128, 128), np.float32)
    wt_[0:62, 0:62] = inp["fuc1_w"][:, 0:62].T
    wt_[64:126, 64:126] = inp["fuc2_w"][:, 0:62].T
    shared["wfut"] = _to_bf(wt_)
    wx_ = np.zeros((128, 128), np.float32)
    wx_[0:62, 0:62] = inp["fuc1_w"][:, 62:124].T
    wx_[64:126, 64:126] = inp["fuc2_w"][:, 62:124].T
    shared["wfux"] = _to_bf(wx_)
    fucb = np.zeros((128, 1), np.float32)
    fucb[0:62, 0] = inp["fuc1_b"]
    fucb[64:126, 0] = inp["fuc2_b"]
    shared["fucb"] = fucb
    ow = np.zeros((128, 64), np.float32)
    ow[0:62, 0:62] = inp["outc_w"][:, 0:62].T
    ow[64:126, 0:62] = inp["outc_w"][:, 62:124].T
    shared["outw"] = _to_bf(ow)
    ob = np.zeros((128, 1), np.float32)
    ob[0:62, 0] = inp["outc_b"]
    shared["outb"] = ob
    fw = np.zeros((128, 9 * 64), np.float32)
    for t, (dy, dx) in enumerate(TAPS):
        fw[0:62, t * 64 : t * 64 + 62] = inp["final_w"][:, 0:62, dy + 1, dx + 1].T
        fw[64:126, t * 64 : t * 64 + 62] = inp["final_w"][:, 62:124, dy + 1, dx + 1].T
    shared["finw"] = _to_bf(fw)
    fb_ = np.zeros((128, 1), np.float32)
    fb_[0:62, 0] = inp["final_b"]
    shared["finb"] = fb_

    for k in list(shared.keys()):
        if shared[k].dtype == np.float32:
            shared[k] = np.ascontiguousarray(shared[k])

    in_maps = []
    for b in range(2):
        xb, yb_, zb = inp["x"][b], inp["y"][b], inp["z"][b]
        for s in range(4):
            row0 = 64 * s - HALO
            xs, ys, zs = _slab(xb, row0), _slab(yb_, row0), _slab(zb, row0)
            m = {}
            xy = np.zeros((128, R, RS), np.float32)
            xy[0:62], xy[64:126] = xs, ys
            m["xy"] = _to_bf(xy.reshape(128, R * RS))
            zs64 = np.zeros((64, R * RS), np.float32)
            zs64[0:62] = zs.reshape(C, R * RS)
            m["zs"] = _to_bf(zs64)
            m["zc"] = _to_bf(zs[:, 5:71].reshape(62, 66 * RS))
            def pixmaj(t1, t2):
                g = np.zeros((OWN * W, 128), np.float32)
                g[:, 0:62] = t1[:, HALO : HALO + OWN, :W].reshape(C, -1).T
                g[:, 64:126] = t2[:, HALO : HALO + OWN, :W].reshape(C, -1).T
                # -> [p, k, c] flattened: partition-major chunks for direct load
                return _to_bf(
                    np.ascontiguousarray(
                        g.reshape(128, 128, 128).transpose(1, 0, 2)
                    ).reshape(128, 128 * 128)
                )
            m["gzx"] = pixmaj(zs, xs)
            m["gzy"] = pixmaj(zs, ys)
            msk = np.zeros((128, 4), np.float32)
            msk[:, 0] = 0.0 if s == 0 else 1.0
            msk[:, 1] = 0.0 if s == 3 else 1.0
            msk[:, 2] = 1.0 if b == 0 else 0.0
            msk[:, 3] = 1.0 if b == 1 else 0.0
            m["msk"] = msk
            m.update(shared)
            in_maps.append(m)
    return in_maps


def assemble_output(results):
    out = np.zeros((2, C, 256, 256), np.float32)
    for b in range(2):
        for s in range(4):
            r = results[b * 4 + s]["out"]  # [62, 64*256]
            out[b, :, 64 * s : 64 * (s + 1), :] = r.reshape(C, OWN, W)
    return out


# ---------------------------------------------------------------------------
# device IR
# ---------------------------------------------------------------------------

PAIRS = ["01", "23", "45"]
L2LEN = exr(2)[1] - exr(2)[0]  # 17544


def build_nc():
    nc = bacc.Bacc(None, target_bir_lowering=False, debug=False)

    P = {}
    P["xy"] = nc.declare_dram_parameter("xy", [128, R * RS], BF16, isOutput=False)
    P["zs"] = nc.declare_dram_parameter("zs", [64, R * RS], BF16, isOutput=False)
    P["zc"] = nc.declare_dram_parameter("zc", [C, 66 * RS], BF16, isOutput=False)
    P["gzx"] = nc.declare_dram_parameter("gzx", [128, OWN * W], BF16, isOutput=False)
    P["gzy"] = nc.declare_dram_parameter("gzy", [128, OWN * W], BF16, isOutput=False)
    P["msk"] = nc.declare_dram_parameter("msk", [128, 4], F32, isOutput=False)
    wnames_bf = (
        [f"wv{p}" for p in PAIRS]
        + ["lnones"]
        + [f"w1aug_{c}{h}" for c in "ab" for h in "01"]
        + [f"w2h_{c}{h}" for c in "ab" for h in "01"]
        + ["wfut", "wfux", "outw", "finw"]
    )
    wnames_f32 = (
        [f"pb{p}" for p in PAIRS]
        + ["rsc", "wqm", "wkm", "pwm", "ones62", "hmA", "hmB"]
        + [f"c1b_{c}{h}" for c in "ab" for h in "01"]
        + [f"c3_dw{c}_{p}" for c in "12" for p in PAIRS]
        + [f"c3_ffdw_{c}{h}" for c in "ab" for h in "01"]
        + ["fucb", "outb", "finb"]
    )
    WSHAPE = {
        "wqm": [128, 6 * 64], "wkm": [128, 6 * 64], "pwm": [128, 6 * 64],
        "ones62": [128, 1], "lnones": [128, 2], "rsc": [128, 6],
        "outw": [128, 64], "finw": [128, 9 * 64],
        "hmA": [128, 64], "hmB": [128, 64],
        "wfut": [128, 128], "wfux": [128, 128],
        "fucb": [128, 1], "outb": [128, 1], "finb": [128, 1],
    }
    for p in PAIRS:
        WSHAPE[f"wv{p}"] = [128, 128]
        WSHAPE[f"pb{p}"] = [128, 1]
        for c in "12":
            WSHAPE[f"m_dw{c}_{p}"] = [128, 7 * 128]
            WSHAPE[f"c3_dw{c}_{p}"] = [128, 3]
    for c in "ab":
        for h in "01":
            WSHAPE[f"w1aug_{c}{h}"] = [128, 124]
            WSHAPE[f"m_ffdw_{c}{h}"] = [128, 7 * 128]
            WSHAPE[f"c3_ffdw_{c}{h}"] = [128, 3]
            WSHAPE[f"c1b_{c}{h}"] = [128, 1]
            WSHAPE[f"w2h_{c}{h}"] = [128, 128]
    mnames = [f"m_dw{c}_{p}" for c in "12" for p in PAIRS] + [
        f"m_ffdw_{c}{h}" for c in "ab" for h in "01"
    ]
    for nm in wnames_bf + mnames:
        P[nm] = nc.declare_dram_parameter(nm, WSHAPE[nm], BF16, isOutput=False)
    for nm in wnames_f32:
        P[nm] = nc.declare_dram_parameter(nm, WSHAPE[nm], F32, isOutput=False)
    out_p = nc.declare_dram_parameter("out", [C, OWN * W], F32, isOutput=True)

    with tile.TileContext(nc, num_cores=8) as tc:
        with (
            tc.tile_pool(name="w", bufs=1) as wp,
            tc.tile_pool(name="small", bufs=1) as sp,
            tc.tile_pool(name="big", bufs=1) as bp,
            tc.tile_pool(name="ring", bufs=3) as rp,
            tc.tile_pool(name="psA", bufs=3, space="PSUM") as psA,
            tc.tile_pool(name="psC", bufs=2, space="PSUM") as psC,
            tc.tile_pool(name="dram", bufs=1, space="DRAM") as dp,
        ):
            # ---------- input slabs first (big DMAs lead the SP queue) ----------
            zz = bp.tile([128, FA], BF16, tag="s1")
            nc.sync.dma_start(zz[0:64, RS : RS + R * RS], P["zs"][:])
            xy = bp.tile([128, FA], BF16, tag="xy")
            nc.sync.dma_start(xy[0:64, RS : RS + R * RS], P["xy"][0:64, :])
            nc.sync.dma_start(xy[64:128, RS : RS + R * RS], P["xy"][64:128, :])

            def load_mdw(nm):
                t_ = rp.tile([128, 7 * 128], BF16, tag="mdw", bufs=2, name=nm + "_l")
                nc.scalar.dma_start(t_[:], P[nm][:])
                return t_

            def chunks(rng, step):
                lo, hi = rng
                out = []
                while lo < hi:
                    out.append((lo, min(lo + step, hi)))
                    lo += step
                return out

            def pad_zero(t, lo_row=0, hi_row=RA):
                v = t[:].rearrange("p (r s) -> p r s", s=RS)[:, lo_row:hi_row, W : W + 2]
                nc.vector.memset(v, 0.0)

            def mask_rows(t, e, dtype_rows=(0, 128)):
                if e <= 0:
                    return
                r0, r1 = dtype_rows
                top = t[r0:r1, (HALO - e + ROFF) * RS : (HALO + ROFF) * RS]
                bot = t[r0:r1, (HALO + OWN + ROFF) * RS : (HALO + OWN + e + ROFF) * RS]
                nc.vector.tensor_scalar_mul(top, top, MTOP[r0:r1])
                nc.vector.tensor_scalar_mul(bot, bot, MBOT[r0:r1])

            def stream_v(dst, wv_t, src, e, kp=128):
                """dst[:, rng] = (wv_t.T @ src)[:, rng] via psA, ACT copy evac."""
                for lo, hi in chunks(exr(e), 1024):
                    ps = psA.tile([128, 1024], F32, tag="psA")
                    for l2, h2 in chunks((lo, hi), NT):
                        nc.tensor.matmul(
                            ps[:, l2 - lo : h2 - lo], wv_t[0:kp, :], src[0:kp, l2:h2],
                            start=True, stop=True,
                        )
                    nc.scalar.copy(dst[:, lo:hi], ps[:, 0 : hi - lo])

            def dw_taps_split(ps, src, mats7, cv, lo, hi, start):
                """accumulate depthwise 3x3 of src into ps[:, 0:hi-lo]:
                6 dx!=0 taps as PE diag-MMs; 3 dx=0 taps on DVE into a bf16
                scratch merged by an identity diag-MM (block 6 of mats7)."""
                n = hi - lo
                scr = rp.tile([128, 1024], BF16, tag="scr", bufs=2)
                nc.vector.tensor_scalar_mul(
                    scr[:, 0:n], src[:, lo - RS : hi - RS], cv[:, 0:1]
                )
                nc.vector.scalar_tensor_tensor(
                    scr[:, 0:n], src[:, lo:hi], cv[:, 1:2], scr[:, 0:n],
                    op0=ALU.mult, op1=ALU.add,
                )
                nc.vector.scalar_tensor_tensor(
                    scr[:, 0:n], src[:, lo + RS : hi + RS], cv[:, 2:3], scr[:, 0:n],
                    op0=ALU.mult, op1=ALU.add,
                )
                for l2, h2 in chunks((lo, hi), NT):
                    for j, d in enumerate(D6):
                        nc.tensor.matmul(
                            ps[:, l2 - lo : h2 - lo],
                            mats7[:, j * 128 : (j + 1) * 128],
                            src[:, l2 + d : h2 + d],
                            start=(start and j == 0), stop=False,
                        )
                    nc.tensor.matmul(
                        ps[:, l2 - lo : h2 - lo],
                        mats7[:, 6 * 128 : 7 * 128],
                        scr[:, l2 - lo : h2 - lo],
                        start=False, stop=True,
                    )

            def conv_pe_gelu(dst, src, mats, cv, e_out, part=None):
                """dst = gelu(dwconv(src)) over extent e_out (split PE/DVE).
                part=(i, n) emits only the i-th of n chunk groups."""
                cl = chunks(exr(e_out), 1024)
                if part is not None:
                    i, n = part
                    cl = cl[i * len(cl) // n : (i + 1) * len(cl) // n]
                for lo, hi in cl:
                    ps = psA.tile([128, 1024], F32, tag="psA")
                    dw_taps_split(ps, src, mats, cv, lo, hi, start=True)
                    nc.scalar.activation(dst[:, lo:hi], ps[:, 0 : hi - lo], AF.Gelu)

            def pair_tail(out_t, v_t, g_t, WoTp, pb_t, resid, pair, e_out):
                """out_t = WoT^T v + dw2(g) + pb + resid (split PE/DVE conv)."""
                mats = load_mdw(f"m_dw2_{pair}")
                cv = WT[f"c3_dw2_{pair}"]
                for lo, hi in chunks(exr(e_out), 1024):
                    ps = psA.tile([128, 1024], F32, tag="psA")
                    for l2, h2 in chunks((lo, hi), NT):
                        nc.tensor.matmul(
                            ps[:, l2 - lo : h2 - lo], WoTp[:], v_t[:, l2:h2],
                            start=True, stop=False,
                        )
                    dw_taps_split(ps, g_t, mats, cv, lo, hi, start=False)
                    nc.vector.scalar_tensor_tensor(
                        out_t[:, lo:hi], ps[:, 0 : hi - lo], pb_t[:],
                        resid[:, lo:hi], op0=ALU.add, op1=ALU.add,
                    )

            # ---------- gram helpers (batched xbar transposes) ----------
            KSECS = [(0, 33), (33, 65), (65, 97), (97, NCK)]

            def tp3v(tp_tile):
                return tp_tile[:].rearrange("p (k c) -> p k c", c=128)

            def gram_half(tp_tile, src, p0, c0, eng):
                """tp[p, k*128 + c0 + f] = src[p0+f, span k*128+p] via xbar
                transpose DMAs, sectioned so gram MMs pipeline per section."""
                tp3 = tp3v(tp_tile)
                for k0, k1 in KSECS:
                    eng.dma_start(
                        tp3[:, k0:k1, c0 : c0 + 64],
                        src[p0 : p0 + 64,
                            GSPAN0 + k0 * 128 : GSPAN0 + k1 * 128],
                        transpose=True,
                    )

            def gram_mms(gt, tp_tile, nck=NCK):
                for ck in range(nck):
                    op = tp_tile[:, ck * 128 : (ck + 1) * 128]
                    nc.tensor.matmul(
                        gt[:], op, op, start=(ck == 0), stop=(ck == nck - 1)
                    )

            # =============== L1 Grams + AR1 ===============
            # host-prepacked pixel-major stacks, loaded with 2 big DMAs each
            NCK1 = OWN * W // 128  # 128 chunks
            tpgx = bp.tile([128, NCK1 * 128], BF16, tag="s2")
            tpgy = bp.tile([128, NCK1 * 128], BF16, tag="s3")
            for tp_t, pn, eng in ((tpgx, "gzx", nc.sync), (tpgy, "gzy", nc.scalar)):
                halfc = NCK1 * 128 // 2
                eng.dma_start(tp_t[:, 0:halfc], P[pn][:, 0:halfc])
                eng.dma_start(tp_t[:, halfc:], P[pn][:, halfc:])

            # ---------- weights via ACT hwdge queue ----------
            WT = {}
            for nm in wnames_bf:
                WT[nm] = wp.tile(WSHAPE[nm], BF16, tag=nm, name=nm)
                nc.scalar.dma_start(WT[nm][:], P[nm][:])
            for nm in wnames_f32:
                WT[nm] = wp.tile(WSHAPE[nm], F32, tag=nm, name=nm)
                nc.scalar.dma_start(WT[nm][:], P[nm][:])
            msk = sp.tile([128, 4], F32, tag="msk")
            nc.scalar.dma_start(msk[:], P["msk"][:])
            SEL0, SEL1 = msk[:, 2:3], msk[:, 3:4]
            MTOP, MBOT = msk[:, 0:1], msk[:, 1:2]


            arin = dp.tile([128, 4 * 128], F32, tag="arin")
            arout = dp.tile([128, 4 * 128], F32, tag="arout", addr_space="Shared")
            arin_sb = sp.tile([128, 4 * 128], F32, tag="arin_sb")
            g1sb = sp.tile([128, 4 * 128], F32, tag="g1sb")

            for gi, tp_t in enumerate([tpgx, tpgy]):
                gps = psC.tile([128, 128], F32, tag="psC")
                gram_mms(gps, tp_t, nck=NCK1)
                nc.vector.tensor_scalar_mul(
                    arin_sb[:, gi * 128 : (gi + 1) * 128], gps[:], SEL0
                )
                nc.vector.tensor_scalar_mul(
                    arin_sb[:, (2 + gi) * 128 : (3 + gi) * 128], gps[:], SEL1
                )
            nc.sync.dma_start(arin[:], arin_sb[:])
            nc.gpsimd.collective_compute(
                "AllReduce", ALU.add, replica_groups=[list(range(8))],
                ins=[arin.opt()], outs=[arout.opt()],
            )
            nc.sync.dma_start(g1sb[:], arout[:])
            # per-batch Gram selection
            gmy = sp.tile([128, 2 * 128], F32, tag="gmy")
            for gi in range(2):
                nc.vector.tensor_scalar_mul(
                    gmy[:, gi * 128 : (gi + 1) * 128],
                    g1sb[:, gi * 128 : (gi + 1) * 128], SEL0,
                )
                nc.vector.scalar_tensor_tensor(
                    gmy[:, gi * 128 : (gi + 1) * 128],
                    g1sb[:, (2 + gi) * 128 : (3 + gi) * 128], SEL1,
                    gmy[:, gi * 128 : (gi + 1) * 128],
                    op0=ALU.mult, op1=ALU.add,
                )

            def attn_module(m, G, qblk, kblk, WoTp, odd):
                """emit small-attn for module m from stack-Gram G [128, 128];
                writes W_oT into WoTp rows/cols r0."""
                wq_s = WT["wqm"][0:62, m * 64 : m * 64 + 62]
                wk_s = WT["wkm"][0:62, m * 64 : m * 64 + 62]
                pw_s = WT["pwm"][0:62, m * 64 : m * 64 + 62]

                def blk(tag, rblk, cblk):
                    if rblk == 0:
                        return G[0:62, cblk * 64 : cblk * 64 + 62]
                    t_ = sp.tile([128, 64], F32, tag="gblk_" + tag)
                    nc.sync.dma_start(
                        t_[0:62, 0:62], G[64:126, cblk * 64 : cblk * 64 + 62]
                    )
                    return t_[0:62, 0:62]

                gqk = blk("qk", qblk, kblk)
                gqq = blk("qq", qblk, qblk)
                gkk = blk("kk", kblk, kblk)

                def mm62(lhs, rhs, tag):
                    pp = psC.tile([128, 64], F32, tag="psC")
                    nc.tensor.matmul(pp[0:62, 0:62], lhs, rhs, start=True, stop=True)
                    ss = sp.tile([128, 64], F32, tag="t_" + tag, name="t_" + tag)
                    nc.vector.tensor_copy(ss[0:62, 0:62], pp[0:62, 0:62])
                    return ss

                T1 = mm62(gqk, wq_s, "T1")
                T2 = mm62(gqq, wq_s, "T2")
                T3 = mm62(gkk, wk_s, "T3")
                SKQ = mm62(wk_s, T1[0:62, 0:62], "SKQ")
                u2 = sp.tile([128, 64], F32, tag="u2")
                nc.vector.tensor_tensor(
                    u2[0:62, 0:62], wq_s, T2[0:62, 0:62], op=ALU.mult
                )
                u3 = sp.tile([128, 64], F32, tag="u3")
                nc.vector.tensor_tensor(
                    u3[0:62, 0:62], wk_s, T3[0:62, 0:62], op=ALU.mult
                )
                pq = psC.tile([128, 64], F32, tag="psC")
                nc.tensor.matmul(
                    pq[0:1, 0:62], WT["ones62"][0:62, 0:1], u2[0:62, 0:62],
                    start=True, stop=True,
                )
                pk = psC.tile([128, 64], F32, tag="psC")
                nc.tensor.matmul(
                    pk[0:62, 0:1], u3[0:62, 0:62], WT["ones62"][0:62, 0:1],
                    start=True, stop=True,
                )
                ik = sp.tile([128, 2], F32, tag="ik")
                nc.scalar.activation(ik[0:62, 0:1], pk[0:62, 0:1], AF.Sqrt)
                nc.vector.tensor_scalar_max(ik[0:62, 0:1], ik[0:62, 0:1], 1e-12)
                nc.vector.reciprocal(ik[0:62, 1:2], ik[0:62, 0:1])
                scd = sp.tile([128, 1], F32, tag="scd")
                nc.vector.tensor_tensor(
                    scd[0:62, 0:1], ik[0:62, 1:2], WT["rsc"][0:62, m : m + 1],
                    op=ALU.mult,
                )
                iq = sp.tile([1, 128], F32, tag="iq")
                nc.scalar.activation(iq[0:1, 0:62], pq[0:1, 0:62], AF.Sqrt)
                nc.vector.tensor_scalar_max(iq[0:1, 0:62], iq[0:1, 0:62], 1e-12)
                nc.vector.reciprocal(iq[0:1, 64:126], iq[0:1, 0:62])
                iqb = sp.tile([128, 64], F32, tag="iqb")
                nc.gpsimd.partition_broadcast(iqb[0:62, 0:62], iq[0:1, 64:126])
                L = sp.tile([128, 64], F32, tag="L")
                nc.vector.tensor_scalar_mul(L[0:62, 0:62], SKQ[0:62, 0:62], scd[0:62, 0:1])
                nc.vector.tensor_tensor(
                    L[0:62, 0:62], L[0:62, 0:62], iqb[0:62, 0:62], op=ALU.mult
                )
                A = sp.tile([128, 64], F32, tag="A")
                nc.vector.memset(A[:], 0.0)
                nc.vector.tensor_tensor(
                    L[0:62, 0:62], L[0:62, 0:62], WT["hmA"][0:62, 0:62], op=ALU.mult
                )
                nc.vector.tensor_tensor(
                    L[0:62, 0:62], L[0:62, 0:62], WT["hmB"][0:62, 0:62], op=ALU.add
                )
                mx = sp.tile([128, 2], F32, tag="mx")
                nc.vector.tensor_reduce(
                    mx[0:62, 0:1], L[0:62, 0:62], op=ALU.max, axis=mybir.AxisListType.X
                )
                nc.vector.tensor_scalar_mul(mx[0:62, 1:2], mx[0:62, 0:1], -1.0)
                nc.scalar.activation(
                    A[0:62, 0:62], L[0:62, 0:62], AF.Exp, bias=mx[0:62, 1:2]
                )
                sm = sp.tile([128, 2], F32, tag="sm")
                nc.vector.tensor_reduce(
                    sm[0:62, 0:1], A[0:62, 0:62], op=ALU.add, axis=mybir.AxisListType.X
                )
                nc.vector.reciprocal(sm[0:62, 1:2], sm[0:62, 0:1])
                nc.vector.tensor_scalar_mul(A[0:62, 0:62], A[0:62, 0:62], sm[0:62, 1:2])
                wps = psC.tile([128, 64], F32, tag="psC")
                if odd:
                    nc.tensor.matmul(
                        wps[64:126, 0:62], A[0:62, 0:62], pw_s,
                        start=True, stop=True, tile_position=(0, 64),
                    )
                    nc.vector.tensor_copy(WoTp[64:126, 64:126], wps[64:126, 0:62])
                else:
                    nc.tensor.matmul(
                        wps[0:62, 0:62], A[0:62, 0:62], pw_s, start=True, stop=True
                    )
                    nc.vector.tensor_copy(WoTp[0:62, 0:62], wps[0:62, 0:62])

            # WoT pair tiles
            WoT = {}
            for p in PAIRS:
                WoT[p] = sp.tile([128, 128], BF16, tag=f"WoT{p}", name=f"WoT{p}")
                nc.vector.memset(WoT[p][:], 0.0)

            # =============== P23: modules 2,3 (v from z) ===============
            v23 = bp.tile([128, FA], BF16, tag="s2")
            stream_v(v23, WT["wv23"], zz, 6, kp=64)
            pad_zero(v23)
            g23 = bp.tile([128, FA], BF16, tag="s3")
            conv_pe_gelu(g23, v23, load_mdw("m_dw1_23"), WT["c3_dw1_23"], 5)
            pad_zero(g23)
            mask_rows(g23, 5)
            # attn for both L1 pairs (AR1 finished during the conv above);
            # modules 0,1 emitted after pt23 so their chain latency hides there
            attn_module(2, gmy[:, 0:128], 1, 0, WoT["23"], odd=False)
            attn_module(3, gmy[:, 128:256], 1, 0, WoT["23"], odd=True)
            fafb = bp.tile([128, FA], BF16, tag="s1")  # reuses zz slot
            pair_tail(fafb, v23, g23, WoT["23"], WT["pb23"], xy, "23", 4)
            pad_zero(fafb)
            mask_rows(fafb, 4)

            # =============== P01: modules 0,1 ===============
            attn_module(0, gmy[:, 0:128], 1, 1, WoT["01"], odd=False)
            attn_module(1, gmy[:, 128:256], 1, 1, WoT["01"], odd=True)
            v01 = bp.tile([128, FA], BF16, tag="s2")
            stream_v(v01, WT["wv01"], xy, 4)
            pad_zero(v01)
            g01 = bp.tile([128, FA], BF16, tag="s3")
            conv_pe_gelu(g01, v01, load_mdw("m_dw1_01"), WT["c3_dw1_01"], 3)
            pad_zero(g01)
            mask_rows(g01, 3)
            # in-place residual: xa|yb overwrites the xy slab (resid read and
            # output write are the same STT op, chunk by chunk)
            xaYb = xy
            pair_tail(xaYb, v01, g01, WoT["01"], WT["pb01"], xy, "01", 2)
            pad_zero(xaYb)
            mask_rows(xaYb, 2)

            # =============== L2 Grams + AR2 (all sources resident) ===============
            tpa = bp.tile([128, NCK * 128], BF16, tag="s2")  # v01 dead
            gram_half(tpa, xaYb, 0, 0, nc.sync)
            gram_half(tpa, fafb, 0, 64, nc.scalar)
            tpb = bp.tile([128, NCK * 128], BF16, tag="s3")  # g01 dead
            gram_half(tpb, xaYb, 64, 0, nc.sync)
            gram_half(tpb, fafb, 64, 64, nc.scalar)
            arin2_sb = sp.tile([128, 4 * 128], F32, tag="arin_sb")
            for gi, tp_t in enumerate([tpa, tpb]):
                gps = psC.tile([128, 128], F32, tag="psC")
                gram_mms(gps, tp_t)
                nc.vector.tensor_scalar_mul(
                    arin2_sb[:, gi * 128 : (gi + 1) * 128], gps[:], SEL0
                )
                nc.vector.tensor_scalar_mul(
                    arin2_sb[:, (2 + gi) * 128 : (3 + gi) * 128], gps[:], SEL1
                )
            arin2 = dp.tile([128, 4 * 128], F32, tag="arin2")
            arout2 = dp.tile([128, 4 * 128], F32, tag="arout2", addr_space="Shared")
            nc.sync.dma_start(arin2[:], arin2_sb[:])
            nc.gpsimd.collective_compute(
                "AllReduce", ALU.add, replica_groups=[list(range(8))],
                ins=[arin2.opt()], outs=[arout2.opt()],
            )
            g2sb = sp.tile([128, 4 * 128], F32, tag="g1sb")
            nc.sync.dma_start(g2sb[:], arout2[:])
            gmy2 = sp.tile([128, 2 * 128], F32, tag="gmy")
            for gi in range(2):
                nc.vector.tensor_scalar_mul(
                    gmy2[:, gi * 128 : (gi + 1) * 128],
                    g2sb[:, gi * 128 : (gi + 1) * 128], SEL0,
                )
                nc.vector.scalar_tensor_tensor(
                    gmy2[:, gi * 128 : (gi + 1) * 128],
                    g2sb[:, (2 + gi) * 128 : (3 + gi) * 128], SEL1,
                    gmy2[:, gi * 128 : (gi + 1) * 128],
                    op0=ALU.mult, op1=ALU.add,
                )

            # =============== P45 (v45/g45 overlap AR2 latency) ===============
            v45 = bp.tile([128, FA], BF16, tag="s2")  # tpa dead after gA MMs
            stream_v(v45, WT["wv45"], fafb, 4)
            pad_zero(v45)
            g45 = bp.tile([128, FA], BF16, tag="s3")  # tpb dead after gB MMs
            mdw45 = load_mdw("m_dw1_45")
            conv_pe_gelu(g45, v45, mdw45, WT["c3_dw1_45"], 3, part=(0, 2))
            # stacks are [xa|fa]: q=xa blk0, k=fa blk1 (chain hides in conv 2nd half)
            attn_module(4, gmy2[:, 0:128], 0, 1, WoT["45"], odd=False)
            attn_module(5, gmy2[:, 128:256], 0, 1, WoT["45"], odd=True)
            conv_pe_gelu(g45, v45, mdw45, WT["c3_dw1_45"], 3, part=(1, 2))
            pad_zero(g45)
            mask_rows(g45, 3)
            t12 = bp.tile([128, FA], BF16, tag="s1")  # fafb dead after v45/gram
            pair_tail(t12, v45, g45, WoT["45"], WT["pb45"], xaYb, "45", 2)
            pad_zero(t12)
            mask_rows(t12, 2)

            # =============== LN stats (both chains) ===============
            lo2, hi2 = exr(2)
            lo1, hi1 = exr(1)
            L2p = 64 * 275  # 17600 staging per stat row
            stat_all_d = dp.tile([2, 2 * L2p], F32, tag="stat_all_d")

            sq = bp.tile([128, FA], BF16, tag="s2")  # reuses v45 slot
            for lo, hi in chunks((lo2, hi2), 4096):
                nc.scalar.activation(sq[:, lo:hi], t12[:, lo:hi], AF.Square)
            for si, srct in enumerate([t12, sq]):
                base = si * L2LEN
                for lo, hi in chunks((lo2, hi2), 512):
                    ps = psA.tile([128, 1024], F32, tag="psA")
                    nc.tensor.matmul(
                        ps[0:2, 0 : hi - lo], WT["lnones"][:],
                        srct[:, lo:hi], start=True, stop=True,
                    )
                    stch = rp.tile([2, 512], F32, tag="stch", bufs=1)
                    nc.scalar.copy(stch[:, 0 : hi - lo], ps[0:2, 0 : hi - lo])
                    nc.sync.dma_start(
                        stat_all_d[:, base + lo - lo2 : base + hi - lo2],
                        stch[:, 0 : hi - lo],
                    )

            # per-pixel inv-std / mu*inv rows for both chains, upfront
            row_ds = {}
            for ci in (0, 1):
                row_dX = dp.tile([1, 2 * L2p], BF16, tag=f"row_d{ci}")
                mu64 = sp.tile([64, 275], F32, tag="mu64")
                ms64 = sp.tile([64, 275], F32, tag="ms64")
                nc.sync.dma_start(
                    mu64[:],
                    stat_all_d[ci : ci + 1, 0:L2p].rearrange(
                        "o (p f) -> (o p) f", p=64
                    ),
                )
                nc.sync.dma_start(
                    ms64[:],
                    stat_all_d[ci : ci + 1, L2LEN : L2LEN + L2p].rearrange(
                        "o (p f) -> (o p) f", p=64
                    ),
                )
                var = sp.tile([64, 275], F32, tag="var64")
                nc.vector.tensor_tensor(var[:], mu64[:], mu64[:], op=ALU.mult)
                nc.vector.tensor_tensor(var[:], ms64[:], var[:], op=ALU.subtract)
                sd = sp.tile([64, 275], F32, tag="sd64")
                nc.vector.tensor_scalar_add(var[:], var[:], 1e-5)
                nc.scalar.activation(sd[:], var[:], AF.Sqrt)
                inv = sp.tile([64, 275], F32, tag="inv64")
                nc.vector.reciprocal(inv[:], sd[:])
                invb = sp.tile([64, 275], BF16, tag="invb64")
                nc.vector.tensor_copy(invb[:], inv[:])
                musb = sp.tile([64, 275], BF16, tag="musb64")
                nc.vector.tensor_tensor(var[:], mu64[:], inv[:], op=ALU.mult)
                nc.vector.tensor_copy(musb[:], var[:])
                nc.sync.dma_start(
                    row_dX[0:1, 0:L2p].rearrange("o (p f) -> (o p) f", p=64),
                    invb[:],
                )
                nc.sync.dma_start(
                    row_dX[0:1, L2p : 2 * L2p].rearrange("o (p f) -> (o p) f", p=64),
                    musb[:],
                )
                row_ds[ci] = row_dX

            # =============== FF per chain (ring-buffered ts) ===============
            for ci, cn in ((0, "a"), (1, "b")):
                r0 = 0 if ci == 0 else 64
                row_dX = row_ds[ci]
                th = {}
                w1ts = [WT[f"w1aug_{cn}0"], WT[f"w1aug_{cn}1"]]
                for h, hn in ((0, "0"), (1, "1")):
                    tht = bp.tile(
                        [128, FA], BF16, tag=("s3" if h == 0 else "s2"),
                        name=f"th{h}_{cn}",
                    )
                    th[h] = tht
                # w1 for both halves per chunk; ts built on the fly in a ring
                for lo, hi in chunks((lo2, hi2), 1024):
                    n = hi - lo
                    sbr = rp.tile([128, 1024], BF16, tag="sbr", bufs=1)
                    nc.scalar.dma_start(
                        sbr[:, 0:n],
                        row_dX[0:1, lo - lo2 : hi - lo2].partition_broadcast(128),
                    )
                    tsr = rp.tile([128, 1024], BF16, tag="tsr", bufs=2)
                    nc.vector.tensor_tensor(
                        tsr[r0 : r0 + 62, 0:n], t12[r0 : r0 + 62, lo:hi],
                        sbr[r0 : r0 + 62, 0:n], op=ALU.mult,
                    )
                    nc.sync.dma_start(
                        tsr[r0 + 62 : r0 + 63, 0:n],
                        row_dX[0:1, L2p + lo - lo2 : L2p + hi - lo2],
                    )
                    for h in (0, 1):
                        ps = psA.tile([128, 1024], F32, tag="psA")
                        for l2, h2 in chunks((lo, hi), NT):
                            nc.tensor.matmul(
                                ps[0:124, l2 - lo : h2 - lo],
                                w1ts[h][r0 : r0 + 63, :],
                                tsr[r0 : r0 + 63, l2 - lo : h2 - lo],
                                start=True, stop=True,
                            )
                        nc.scalar.activation(
                            th[h][0:124, lo:hi], ps[0:124, 0 : hi - lo], AF.Gelu,
                            bias=WT[f"c1b_{cn}{'0' if h == 0 else '1'}"][0:124, :],
                        )
                for h in (0, 1):
                    pad_zero(th[h])
                    mask_rows(th[h], 2, (0, 124))
                # conv h0/h1 + gelu + w2, chunk-wise (convs on PE)
                for lo, hi in chunks((lo1, hi1), 1024):
                    wps = psA.tile([128, 1024], F32, tag="psA")
                    for h, hn in ((0, "0"), (1, "1")):
                        ghc = rp.tile([128, 1024], BF16, tag="ghc", bufs=2)
                        cps = psA.tile([128, 1024], F32, tag="psA")
                        mats = load_mdw(f"m_ffdw_{cn}{hn}")
                        dw_taps_split(
                            cps, th[h], mats, WT[f"c3_ffdw_{cn}{hn}"],
                            lo, hi, start=True,
                        )
                        nc.scalar.activation(
                            ghc[0:124, 0 : hi - lo], cps[0:124, 0 : hi - lo],
                            AF.Gelu,
                        )
                        w2t = WT[f"w2h_{cn}{hn}"]
                        for l2, h2 in chunks((lo, hi), NT):
                            nc.tensor.matmul(
                                wps[r0 : r0 + 62, l2 - lo : h2 - lo],
                                w2t[0:124, 0:62],
                                ghc[0:124, l2 - lo : h2 - lo],
                                start=(h == 0), stop=(h == 1),
                                tile_position=(0, 64) if ci == 1 else None,
                            )
                    # t12 += w2 out (in place, one rounding)
                    nc.vector.scalar_tensor_tensor(
                        t12[r0 : r0 + 62, lo:hi], wps[r0 : r0 + 62, 0 : hi - lo],
                        1.0, t12[r0 : r0 + 62, lo:hi], op0=ALU.mult, op1=ALU.add,
                    )

            # =============== fusions + final ===============
            fin = bp.tile([128, FA], BF16, tag="s3")
            nc.sync.dma_start(fin[64:126, lo1:hi1], P["zc"][0:62, :])
            fab = bp.tile([128, FA], BF16, tag="s2")
            for lo, hi in chunks((lo1, hi1), 1024):
                ps = psA.tile([128, 1024], F32, tag="psA")
                for l2, h2 in chunks((lo, hi), NT):
                    nc.tensor.matmul(
                        ps[:, l2 - lo : h2 - lo], WT["wfut"][:], t12[:, l2:h2],
                        start=True, stop=False,
                    )
                    nc.tensor.matmul(
                        ps[:, l2 - lo : h2 - lo], WT["wfux"][:], xaYb[:, l2:h2],
                        start=False, stop=True,
                    )
                nc.scalar.add(fab[:, lo:hi], ps[:, 0 : hi - lo], WT["fucb"][:])
            pad_zero(fab)
            mask_rows(fab, 1)
            for lo, hi in chunks((lo1, hi1), 1024):
                ps = psA.tile([128, 1024], F32, tag="psA")
                for l2, h2 in chunks((lo, hi), NT):
                    nc.tensor.matmul(
                        ps[0:64, l2 - lo : h2 - lo], WT["outw"][:], fab[:, l2:h2],
                        start=True, stop=True,
                    )
                nc.scalar.add(fin[0:62, lo:hi], ps[0:62, 0 : hi - lo], WT["outb"][0:62, :])
            pad_zero(fin)
            mask_rows(fin, 1, (0, 62))
            # final 3x3 conv, direct DMA out in 2-row (516 col) pieces.
            # each 258-col row-piece goes to its own PSUM bank (offsets 0, 512)
            lo0, hi0 = exr(0)
            out3 = out_p[:].rearrange("c (r w) -> c r w", w=W)
            for lo, hi in chunks((lo0, hi0), 516):
                ps = psA.tile([128, 1024], F32, tag="psA")
                for t in range(9):
                    dy, dx = TAPS[t]
                    d = dy * RS + dx
                    for bi, (l2, h2) in enumerate(((lo, lo + 258), (lo + 258, hi))):
                        nc.tensor.matmul(
                            ps[0:64, bi * 512 : bi * 512 + 258],
                            WT["finw"][:, t * 64 : (t + 1) * 64],
                            fin[:, l2 + d : h2 + d],
                            start=(t == 0), stop=(t == 8),
                        )
                och = rp.tile([62, 516], F32, tag="och", bufs=2)
                ps3 = ps[:].rearrange("p (b c) -> p b c", c=512)[0:62, :, 0:258]
                nc.scalar.add(
                    och[:].rearrange("c (r s) -> c r s", s=RS), ps3,
                    WT["finb"][0:62, :],
                )
                r0 = (lo - lo0) // RS
                nc.sync.dma_start(
                    out3[:, r0 : r0 + 2, :],
                    och[:].rearrange("c (r s) -> c r s", s=RS)[:, :, 0:W],
                )

    nc.finalize()
    return nc


_NC_CACHE = {}


def _run(inputs, trace=False):
    if "nc" not in _NC_CACHE:
        _NC_CACHE["nc"] = build_nc()
    nc = _NC_CACHE["nc"]
    names = {
        a.name.removesuffix("_set")
        for a in nc.m.functions[0].allocations
        if getattr(a, "kind", None) == "ExternalInput"
    }
    in_maps = prep_host_inputs(inputs)
    in_maps = [{k: v for k, v in m.items() if k in names} for m in in_maps]
    res = run_bass_kernel_spmd(
        nc, in_maps, core_ids=list(range(8)), trace=trace
    )
    return assemble_output(res.results), res


def kernel(**inputs):
    out, _ = _run(inputs, trace=False)
    return out


# revision 26
# speedup vs baseline: 1.0925x; 1.0196x over previous
"""Bass/Trainium2 kernel for nn_Block_14010183320003 (MST++-style block).

Sharding: 8 cores = 2 batches x 4 row-quarters (64 owned rows each, halo 6).
Chain packing: partitions 0:62 = x-derived chain A, 64:126 = y-derived chain B.
Attention is computed from 124-dim input Gram matrices (no q/k tensors);
per-core partial Grams are summed with two ~256KB AllReduces.
Pixel-major Gram stacks are built ON DEVICE with batched DMA xbar transposes
(few big DMA_TRANSPOSE ops instead of per-chunk ones).
Row layout: stride 258 per row (2 zero pad cols) so 3x3 convs are free-axis
shifted reads; depthwise convs run as PE diag-matmul accumulation.
"""
import sys

sys.path.insert(0, "/opt/trn_rl_repo")
import numpy as np
import ml_dtypes

import concourse.bass as bass
import concourse.mybir as mybir
import concourse.tile as tile
import concourse.bacc as bacc
from concourse.bass_utils import run_bass_kernel_spmd

BF16, F32 = mybir.dt.bfloat16, mybir.dt.float32
AF = mybir.ActivationFunctionType
ALU = mybir.AluOpType
bf = ml_dtypes.bfloat16

C = 62
W = 256
RS = 258  # row stride (2 zero pad cols)
OWN = 64
HALO = 6
R = 76
ROFF = 1  # lead pad row at tile row 0 (zero pad for shifted reads)
RA = 78  # lead pad + 76 slab rows + 1 trailing spare
FA = RA * RS  # 20124
NT = 512  # matmul moving chunk
NH, DH = 2, 31
GSPAN0 = (HALO + ROFF) * RS  # owned-pixel span start = 1806
NCK = 129  # owned span 64*258 = 129 chunks of 128

TAPS = [(dy, dx) for dy in (-1, 0, 1) for dx in (-1, 0, 1)]
T6 = [0, 2, 3, 5, 6, 8]  # taps with dx != 0 -> PE diag matmuls
TD = [1, 4, 7]  # dx == 0 taps (dy=-1,0,+1) -> DVE scalar chains
D6 = [TAPS[t][0] * RS + TAPS[t][1] for t in T6]


def exr(e):
    """free range of extent e (slab rows [6-e, 70+e)), incl lead-row offset."""
    return ((HALO - e + ROFF) * RS, (HALO + OWN + e + ROFF) * RS)


def _slab(full, row0):
    """full [C, 256, 256] -> [C, R, RS] zero-padded slab (rows row0..row0+R)."""
    out = np.zeros((C, R, RS), np.float32)
    lo, hi = max(0, row0), min(256, row0 + R)
    out[:, lo - row0 : hi - row0, :W] = full[:, lo:hi]
    return out


def _to_bf(a):
    return np.ascontiguousarray(a.astype(bf))


def _diag_taps6(kc):
    """kc: [128, 9] -> [128, 7*128] bf16: 6 dx!=0 tap diags + identity."""
    out = np.zeros((128, 7 * 128), np.float32)
    ar = np.arange(128)
    for j, t in enumerate(T6):
        out[:, j * 128 : (j + 1) * 128][ar, ar] = kc[:, t]
    out[:, 6 * 128 : 7 * 128][ar, ar] = 1.0
    return _to_bf(out)


def _pair128(a_block, b_block):
    out = np.zeros((128, 128), np.float32)
    out[0:62, 0:62] = a_block
    out[64:126, 64:126] = b_block
    return _to_bf(out)


def prep_host_inputs(inputs):
    """Build the 8 per-core input maps."""
    inp = {k: np.asarray(v, dtype=np.float32) for k, v in inputs.items()}
    wq, wk, wv = inp["attn_wq"], inp["attn_wk"], inp["attn_wv"]
    pw, pb = inp["attn_pw"], inp["attn_pb"]
    dw1, dw2 = inp["attn_dw1"], inp["attn_dw2"]
    resc = inp["attn_rescale"]

    shared = {}
    # v-projection pair weights [128,128].
    # wv01/wv45: block-diag (K rows 0:62 -> cols 0:62 even module, K 64:126 ->
    # cols 64:126 odd module). wv23: single-z, K rows 0:62 feed BOTH col blocks.
    shared["wv01"] = _pair128(wv[0], wv[1])
    shared["wv45"] = _pair128(wv[4], wv[5])
    wv23 = np.zeros((128, 128), np.float32)
    wv23[0:62, 0:62] = wv[2]
    wv23[0:62, 64:126] = wv[3]
    shared["wv23"] = _to_bf(wv23)
    for pair, (me, mo) in {"01": (0, 1), "23": (2, 3), "45": (4, 5)}.items():
        pbv = np.zeros((128, 1), np.float32)
        pbv[0:62, 0] = pb[me]
        pbv[64:126, 0] = pb[mo]
        shared[f"pb{pair}"] = pbv
        for cn, dwk in (("1", dw1), ("2", dw2)):
            kc = np.zeros((128, 9), np.float32)
            for t, (dy, dx) in enumerate(TAPS):
                kc[0:62, t] = dwk[me][:, dy + 1, dx + 1]
                kc[64:126, t] = dwk[mo][:, dy + 1, dx + 1]
            shared[f"m_dw{cn}_{pair}"] = _diag_taps6(kc)
            shared[f"c3_dw{cn}_{pair}"] = np.ascontiguousarray(kc[:, TD])
    # attn small weights: [128, 6*64] col block m at m*64, rows 0:62
    for nm, src in (("wqm", wq), ("wkm", wk), ("pwm", pw)):
        t = np.zeros((128, 6 * 64), np.float32)
        for m in range(6):
            t[0:62, m * 64 : m * 64 + 62] = src[m]
        shared[nm] = t
    rs = np.zeros((128, 6), np.float32)
    for m in range(6):
        rs[0:62, m] = np.repeat(resc[m], DH)
    shared["rsc"] = rs
    ones62 = np.zeros((128, 1), np.float32)
    ones62[0:62, 0] = 1.0
    shared["ones62"] = ones62
    hmA = np.zeros((128, 64), np.float32)
    hmB = np.full((128, 64), -1e4, np.float32)
    for h in range(NH):
        hmA[h * DH : (h + 1) * DH, h * DH : (h + 1) * DH] = 1.0
        hmB[h * DH : (h + 1) * DH, h * DH : (h + 1) * DH] = 0.0
    shared["hmA"] = hmA
    shared["hmB"] = hmB
    lnones = np.zeros((128, 2), np.float32)
    lnones[0:62, 0] = 1.0 / C
    lnones[64:126, 1] = 1.0 / C
    shared["lnones"] = _to_bf(lnones)

    # FF weights. chain A uses ff index 0, chain B index 1.
    for ci, cname in ((0, "a"), (1, "b")):
        g, b = inp["ln_g"][ci], inp["ln_b"][ci]
        w1 = inp["ff_w1"][ci]  # [62, 248]
        w2 = inp["ff_w2"][ci]  # [248, 62]
        w1p = g[:, None] * w1
        c2 = w1p.sum(axis=0)  # [248]
        c1 = b @ w1  # [248]
        dwk = inp["ff_dw"][ci]  # [248, 3, 3]
        r0 = 0 if ci == 0 else 64
        for h in (0, 1):
            sl = slice(h * 124, (h + 1) * 124)
            wt = np.zeros((128, 124), np.float32)
            wt[r0 : r0 + 62, :] = w1p[:, sl]
            wt[r0 + 62, :] = -c2[sl]
            shared[f"w1aug_{cname}{h}"] = _to_bf(wt)
            cb = np.zeros((128, 1), np.float32)
            cb[0:124, 0] = c1[sl]
            shared[f"c1b_{cname}{h}"] = cb
            kc = np.zeros((128, 9), np.float32)
            for t, (dy, dx) in enumerate(TAPS):
                kc[0:124, t] = dwk[sl, dy + 1, dx + 1]
            shared[f"m_ffdw_{cname}{h}"] = _diag_taps6(kc)
            shared[f"c3_ffdw_{cname}{h}"] = np.ascontiguousarray(kc[:, TD])
            w2t = np.zeros((128, 128), np.float32)
            w2t[0:124, 0:62] = w2[sl, :]
            shared[f"w2h_{cname}{h}"] = _to_bf(w2t)

    # fusion weights: fab = Wfu_t^T @ t12 + Wfu_x^T @ xaYb + fucb
    wt_ = np.zeros((128, 128), np.float32)
    wt_[0:62, 0:62] = inp["fuc1_w"][:, 0:62].T
    wt_[64:126, 64:126] = inp["fuc2_w"][:, 0:62].T
    shared["wfut"] = _to_bf(wt_)
    wx_ = np.zeros((128, 128), np.float32)
    wx_[0:62, 0:62] = inp["fuc1_w"][:, 62:124].T
    wx_[64:126, 64:126] = inp["fuc2_w"][:, 62:124].T
    shared["wfux"] = _to_bf(wx_)
    fucb = np.zeros((128, 1), np.float32)
    fucb[0:62, 0] = inp["fuc1_b"]
    fucb[64:126, 0] = inp["fuc2_b"]
    shared["fucb"] = fucb
    ow = np.zeros((128, 64), np.float32)
    ow[0:62, 0:62] = inp["outc_w"][:, 0:62].T
    ow[64:126, 0:62] = inp["outc_w"][:, 62:124].T
    shared["outw"] = _to_bf(ow)
    ob = np.zeros((128, 1), np.float32)
    ob[0:62, 0] = inp["outc_b"]
    shared["outb"] = ob
    fw = np.zeros((128, 9 * 64), np.float32)
    for t, (dy, dx) in enumerate(TAPS):
        fw[0:62, t * 64 : t * 64 + 62] = inp["final_w"][:, 0:62, dy + 1, dx + 1].T
        fw[64:126, t * 64 : t * 64 + 62] = inp["final_w"][:, 62:124, dy + 1, dx + 1].T
    shared["finw"] = _to_bf(fw)
    fb_ = np.zeros((128, 1), np.float32)
    fb_[0:62, 0] = inp["final_b"]
    shared["finb"] = fb_

    for k in list(shared.keys()):
        if shared[k].dtype == np.float32:
            shared[k] = np.ascontiguousarray(shared[k])

    in_maps = []
    for b in range(2):
        xb, yb_, zb = inp["x"][b], inp["y"][b], inp["z"][b]
        for s in range(4):
            row0 = 64 * s - HALO
            xs, ys, zs = _slab(xb, row0), _slab(yb_, row0), _slab(zb, row0)
            m = {}
            xy = np.zeros((128, R, RS), np.float32)
            xy[0:62], xy[64:126] = xs, ys
            m["xy"] = _to_bf(xy.reshape(128, R * RS))
            zs64 = np.zeros((64, R * RS), np.float32)
            zs64[0:62] = zs.reshape(C, R * RS)
            m["zs"] = _to_bf(zs64)
            m["zc"] = _to_bf(zs[:, 5:71].reshape(62, 66 * RS))
            def pixmaj(t1, t2):
                g = np.zeros((OWN * W, 128), np.float32)
                g[:, 0:62] = t1[:, HALO : HALO + OWN, :W].reshape(C, -1).T
                g[:, 64:126] = t2[:, HALO : HALO + OWN, :W].reshape(C, -1).T
                # -> [p, k, c] flattened: partition-major chunks for direct load
                return _to_bf(
                    np.ascontiguousarray(
                        g.reshape(128, 128, 128).transpose(1, 0, 2)
                    ).reshape(128, 128 * 128)
                )
            m["gzx"] = pixmaj(zs, xs)
            m["gzy"] = pixmaj(zs, ys)
            msk = np.zeros((128, 4), np.float32)
            msk[:, 0] = 0.0 if s == 0 else 1.0
            msk[:, 1] = 0.0 if s == 3 else 1.0
            msk[:, 2] = 1.0 if b == 0 else 0.0
            msk[:, 3] = 1.0 if b == 1 else 0.0
            m["msk"] = msk
            m.update(shared)
            in_maps.append(m)
    return in_maps


def assemble_output(results):
    out = np.zeros((2, C, 256, 256), np.float32)
    for b in range(2):
        for s in range(4):
            r = results[b * 4 + s]["out"]  # [62, 64*256]
            out[b, :, 64 * s : 64 * (s + 1), :] = r.reshape(C, OWN, W)
    return out


# ---------------------------------------------------------------------------
# device IR
# ---------------------------------------------------------------------------

PAIRS = ["01", "23", "45"]
L2LEN = exr(2)[1] - exr(2)[0]  # 17544


def build_nc():
    nc = bacc.Bacc(None, target_bir_lowering=False, debug=False)

    P = {}
    P["xy"] = nc.declare_dram_parameter("xy", [128, R * RS], BF16, isOutput=False)
    P["zs"] = nc.declare_dram_parameter("zs", [64, R * RS], BF16, isOutput=False)
    P["zc"] = nc.declare_dram_parameter("zc", [C, 66 * RS], BF16, isOutput=False)
    P["gzx"] = nc.declare_dram_parameter("gzx", [128, OWN * W], BF16, isOutput=False)
    P["gzy"] = nc.declare_dram_parameter("gzy", [128, OWN * W], BF16, isOutput=False)
    P["msk"] = nc.declare_dram_parameter("msk", [128, 4], F32, isOutput=False)
    wnames_bf = (
        [f"wv{p}" for p in PAIRS]
        + ["lnones"]
        + [f"w1aug_{c}{h}" for c in "ab" for h in "01"]
        + [f"w2h_{c}{h}" for c in "ab" for h in "01"]
        + ["wfut", "wfux", "outw", "finw"]
    )
    wnames_f32 = (
        [f"pb{p}" for p in PAIRS]
        + ["rsc", "wqm", "wkm", "pwm", "ones62", "hmA", "hmB"]
        + [f"c1b_{c}{h}" for c in "ab" for h in "01"]
        + [f"c3_dw{c}_{p}" for c in "12" for p in PAIRS]
        + [f"c3_ffdw_{c}{h}" for c in "ab" for h in "01"]
        + ["fucb", "outb", "finb"]
    )
    WSHAPE = {
        "wqm": [128, 6 * 64], "wkm": [128, 6 * 64], "pwm": [128, 6 * 64],
        "ones62": [128, 1], "lnones": [128, 2], "rsc": [128, 6],
        "outw": [128, 64], "finw": [128, 9 * 64],
        "hmA": [128, 64], "hmB": [128, 64],
        "wfut": [128, 128], "wfux": [128, 128],
        "fucb": [128, 1], "outb": [128, 1], "finb": [128, 1],
    }
    for p in PAIRS:
        WSHAPE[f"wv{p}"] = [128, 128]
        WSHAPE[f"pb{p}"] = [128, 1]
        for c in "12":
            WSHAPE[f"m_dw{c}_{p}"] = [128, 7 * 128]
            WSHAPE[f"c3_dw{c}_{p}"] = [128, 3]
    for c in "ab":
        for h in "01":
            WSHAPE[f"w1aug_{c}{h}"] = [128, 124]
            WSHAPE[f"m_ffdw_{c}{h}"] = [128, 7 * 128]
            WSHAPE[f"c3_ffdw_{c}{h}"] = [128, 3]
            WSHAPE[f"c1b_{c}{h}"] = [128, 1]
            WSHAPE[f"w2h_{c}{h}"] = [128, 128]
    mnames = [f"m_dw{c}_{p}" for c in "12" for p in PAIRS] + [
        f"m_ffdw_{c}{h}" for c in "ab" for h in "01"
    ]
    for nm in wnames_bf + mnames:
        P[nm] = nc.declare_dram_parameter(nm, WSHAPE[nm], BF16, isOutput=False)
    for nm in wnames_f32:
        P[nm] = nc.declare_dram_parameter(nm, WSHAPE[nm], F32, isOutput=False)
    out_p = nc.declare_dram_parameter("out", [C, OWN * W], F32, isOutput=True)

    with tile.TileContext(nc, num_cores=8) as tc:
        with (
            tc.tile_pool(name="w", bufs=1) as wp,
            tc.tile_pool(name="small", bufs=1) as sp,
            tc.tile_pool(name="big", bufs=1) as bp,
            tc.tile_pool(name="ring", bufs=3) as rp,
            tc.tile_pool(name="psA", bufs=3, space="PSUM") as psA,
            tc.tile_pool(name="psC", bufs=2, space="PSUM") as psC,
            tc.tile_pool(name="dram", bufs=1, space="DRAM") as dp,
        ):
            # ---------- gram stacks first (they gate the first PE work) ----------
            NCK1 = OWN * W // 128  # 128 chunks
            tpgx = bp.tile([128, NCK1 * 128], BF16, tag="s2")
            tpgy = bp.tile([128, NCK1 * 128], BF16, tag="s3")
            for tp_t, pn, eng in ((tpgx, "gzx", nc.sync), (tpgy, "gzy", nc.scalar)):
                halfc = NCK1 * 128 // 2
                eng.dma_start(tp_t[:, 0:halfc], P[pn][:, 0:halfc])
                eng.dma_start(tp_t[:, halfc:], P[pn][:, halfc:])
            zz = bp.tile([128, FA], BF16, tag="s1")
            nc.sync.dma_start(zz[0:64, RS : RS + R * RS], P["zs"][:])
            xy = bp.tile([128, FA], BF16, tag="xy")
            nc.sync.dma_start(xy[0:64, RS : RS + R * RS], P["xy"][0:64, :])
            nc.sync.dma_start(xy[64:128, RS : RS + R * RS], P["xy"][64:128, :])

            def load_mdw(nm):
                t_ = rp.tile([128, 7 * 128], BF16, tag="mdw", bufs=2, name=nm + "_l")
                nc.scalar.dma_start(t_[:], P[nm][:])
                return t_

            def chunks(rng, step):
                lo, hi = rng
                out = []
                while lo < hi:
                    out.append((lo, min(lo + step, hi)))
                    lo += step
                return out

            def pad_zero(t, lo_row=0, hi_row=RA):
                v = t[:].rearrange("p (r s) -> p r s", s=RS)[:, lo_row:hi_row, W : W + 2]
                nc.vector.memset(v, 0.0)

            def mask_rows(t, e, dtype_rows=(0, 128)):
                if e <= 0:
                    return
                r0, r1 = dtype_rows
                top = t[r0:r1, (HALO - e + ROFF) * RS : (HALO + ROFF) * RS]
                bot = t[r0:r1, (HALO + OWN + ROFF) * RS : (HALO + OWN + e + ROFF) * RS]
                nc.vector.tensor_scalar_mul(top, top, MTOP[r0:r1])
                nc.vector.tensor_scalar_mul(bot, bot, MBOT[r0:r1])

            def stream_v(dst, wv_t, src, e, kp=128):
                """dst[:, rng] = (wv_t.T @ src)[:, rng] via psA, ACT copy evac."""
                for lo, hi in chunks(exr(e), 1024):
                    ps = psA.tile([128, 1024], F32, tag="psA")
                    for l2, h2 in chunks((lo, hi), NT):
                        nc.tensor.matmul(
                            ps[:, l2 - lo : h2 - lo], wv_t[0:kp, :], src[0:kp, l2:h2],
                            start=True, stop=True,
                        )
                    nc.scalar.copy(dst[:, lo:hi], ps[:, 0 : hi - lo])

            def dw_taps_split(ps, src, mats7, cv, lo, hi, start):
                """accumulate depthwise 3x3 of src into ps[:, 0:hi-lo]:
                6 dx!=0 taps as PE diag-MMs; 3 dx=0 taps on DVE into a bf16
                scratch merged by an identity diag-MM (block 6 of mats7)."""
                n = hi - lo
                scr = rp.tile([128, 1024], BF16, tag="scr", bufs=2)
                nc.vector.tensor_scalar_mul(
                    scr[:, 0:n], src[:, lo - RS : hi - RS], cv[:, 0:1]
                )
                nc.vector.scalar_tensor_tensor(
                    scr[:, 0:n], src[:, lo:hi], cv[:, 1:2], scr[:, 0:n],
                    op0=ALU.mult, op1=ALU.add,
                )
                nc.vector.scalar_tensor_tensor(
                    scr[:, 0:n], src[:, lo + RS : hi + RS], cv[:, 2:3], scr[:, 0:n],
                    op0=ALU.mult, op1=ALU.add,
                )
                for l2, h2 in chunks((lo, hi), NT):
                    for j, d in enumerate(D6):
                        nc.tensor.matmul(
                            ps[:, l2 - lo : h2 - lo],
                            mats7[:, j * 128 : (j + 1) * 128],
                            src[:, l2 + d : h2 + d],
                            start=(start and j == 0), stop=False,
                        )
                    nc.tensor.matmul(
                        ps[:, l2 - lo : h2 - lo],
                        mats7[:, 6 * 128 : 7 * 128],
                        scr[:, l2 - lo : h2 - lo],
                        start=False, stop=True,
                    )

            def conv_pe_gelu(dst, src, mats, cv, e_out, part=None):
                """dst = gelu(dwconv(src)) over extent e_out (split PE/DVE).
                part=(i, n) emits only the i-th of n chunk groups."""
                cl = chunks(exr(e_out), 1024)
                if part is not None:
                    i, n = part
                    cl = cl[i * len(cl) // n : (i + 1) * len(cl) // n]
                for lo, hi in cl:
                    ps = psA.tile([128, 1024], F32, tag="psA")
                    dw_taps_split(ps, src, mats, cv, lo, hi, start=True)
                    nc.scalar.activation(dst[:, lo:hi], ps[:, 0 : hi - lo], AF.Gelu)

            def pair_tail(out_t, v_t, g_t, WoTp, pb_t, resid, pair, e_out):
                """out_t = WoT^T v + dw2(g) + pb + resid (split PE/DVE conv)."""
                mats = load_mdw(f"m_dw2_{pair}")
                cv = WT[f"c3_dw2_{pair}"]
                for lo, hi in chunks(exr(e_out), 1024):
                    ps = psA.tile([128, 1024], F32, tag="psA")
                    for l2, h2 in chunks((lo, hi), NT):
                        nc.tensor.matmul(
                            ps[:, l2 - lo : h2 - lo], WoTp[:], v_t[:, l2:h2],
                            start=True, stop=False,
                        )
                    dw_taps_split(ps, g_t, mats, cv, lo, hi, start=False)
                    nc.vector.scalar_tensor_tensor(
                        out_t[:, lo:hi], ps[:, 0 : hi - lo], pb_t[:],
                        resid[:, lo:hi], op0=ALU.add, op1=ALU.add,
                    )

            # ---------- gram helpers (batched xbar transposes) ----------
            KSECS = [(0, 33), (33, 65), (65, 97), (97, NCK)]

            def tp3v(tp_tile):
                return tp_tile[:].rearrange("p (k c) -> p k c", c=128)

            def gram_half(tp_tile, src, p0, c0, eng):
                """tp[p, k*128 + c0 + f] = src[p0+f, span k*128+p] via xbar
                transpose DMAs, sectioned so gram MMs pipeline per section."""
                tp3 = tp3v(tp_tile)
                for k0, k1 in KSECS:
                    eng.dma_start(
                        tp3[:, k0:k1, c0 : c0 + 64],
                        src[p0 : p0 + 64,
                            GSPAN0 + k0 * 128 : GSPAN0 + k1 * 128],
                        transpose=True,
                    )

            def gram_mms(gt, tp_tile, nck=NCK):
                for ck in range(nck):
                    op = tp_tile[:, ck * 128 : (ck + 1) * 128]
                    nc.tensor.matmul(
                        gt[:], op, op, start=(ck == 0), stop=(ck == nck - 1)
                    )

            # =============== L1 Grams + AR1 ===============

            # ---------- weights via ACT hwdge queue ----------
            WT = {}
            for nm in wnames_bf:
                WT[nm] = wp.tile(WSHAPE[nm], BF16, tag=nm, name=nm)
                nc.scalar.dma_start(WT[nm][:], P[nm][:])
            for nm in wnames_f32:
                WT[nm] = wp.tile(WSHAPE[nm], F32, tag=nm, name=nm)
                nc.scalar.dma_start(WT[nm][:], P[nm][:])
            msk = sp.tile([128, 4], F32, tag="msk")
            nc.scalar.dma_start(msk[:], P["msk"][:])
            SEL0, SEL1 = msk[:, 2:3], msk[:, 3:4]
            MTOP, MBOT = msk[:, 0:1], msk[:, 1:2]


            arin = dp.tile([128, 4 * 128], F32, tag="arin")
            arout = dp.tile([128, 4 * 128], F32, tag="arout", addr_space="Shared")
            arin_sb = sp.tile([128, 4 * 128], F32, tag="arin_sb")
            g1sb = sp.tile([128, 4 * 128], F32, tag="g1sb")

            for gi, tp_t in enumerate([tpgx, tpgy]):
                gps = psC.tile([128, 128], F32, tag="psC")
                gram_mms(gps, tp_t, nck=NCK1)
                nc.vector.tensor_scalar_mul(
                    arin_sb[:, gi * 128 : (gi + 1) * 128], gps[:], SEL0
                )
                nc.vector.tensor_scalar_mul(
                    arin_sb[:, (2 + gi) * 128 : (3 + gi) * 128], gps[:], SEL1
                )
            nc.sync.dma_start(arin[:], arin_sb[:])
            nc.gpsimd.collective_compute(
                "AllReduce", ALU.add, replica_groups=[list(range(8))],
                ins=[arin.opt()], outs=[arout.opt()],
            )
            nc.sync.dma_start(g1sb[:], arout[:])
            # per-batch Gram selection
            gmy = sp.tile([128, 2 * 128], F32, tag="gmy")
            for gi in range(2):
                nc.vector.tensor_scalar_mul(
                    gmy[:, gi * 128 : (gi + 1) * 128],
                    g1sb[:, gi * 128 : (gi + 1) * 128], SEL0,
                )
                nc.vector.scalar_tensor_tensor(
                    gmy[:, gi * 128 : (gi + 1) * 128],
                    g1sb[:, (2 + gi) * 128 : (3 + gi) * 128], SEL1,
                    gmy[:, gi * 128 : (gi + 1) * 128],
                    op0=ALU.mult, op1=ALU.add,
                )

            def attn_module(m, G, qblk, kblk, WoTp, odd):
                """emit small-attn for module m from stack-Gram G [128, 128];
                writes W_oT into WoTp rows/cols r0."""
                wq_s = WT["wqm"][0:62, m * 64 : m * 64 + 62]
                wk_s = WT["wkm"][0:62, m * 64 : m * 64 + 62]
                pw_s = WT["pwm"][0:62, m * 64 : m * 64 + 62]

                def blk(tag, rblk, cblk):
                    if rblk == 0:
                        return G[0:62, cblk * 64 : cblk * 64 + 62]
                    t_ = sp.tile([128, 64], F32, tag="gblk_" + tag)
                    nc.sync.dma_start(
                        t_[0:62, 0:62], G[64:126, cblk * 64 : cblk * 64 + 62]
                    )
                    return t_[0:62, 0:62]

                gqk = blk("qk", qblk, kblk)
                gqq = blk("qq", qblk, qblk)
                gkk = blk("kk", kblk, kblk)

                def mm62(lhs, rhs, tag):
                    pp = psC.tile([128, 64], F32, tag="psC")
                    nc.tensor.matmul(pp[0:62, 0:62], lhs, rhs, start=True, stop=True)
                    ss = sp.tile([128, 64], F32, tag="t_" + tag, name="t_" + tag)
                    nc.vector.tensor_copy(ss[0:62, 0:62], pp[0:62, 0:62])
                    return ss

                T1 = mm62(gqk, wq_s, "T1")
                T2 = mm62(gqq, wq_s, "T2")
                T3 = mm62(gkk, wk_s, "T3")
                SKQ = mm62(wk_s, T1[0:62, 0:62], "SKQ")
                u2 = sp.tile([128, 64], F32, tag="u2")
                nc.vector.tensor_tensor(
                    u2[0:62, 0:62], wq_s, T2[0:62, 0:62], op=ALU.mult
                )
                u3 = sp.tile([128, 64], F32, tag="u3")
                nc.vector.tensor_tensor(
                    u3[0:62, 0:62], wk_s, T3[0:62, 0:62], op=ALU.mult
                )
                pq = psC.tile([128, 64], F32, tag="psC")
                nc.tensor.matmul(
                    pq[0:1, 0:62], WT["ones62"][0:62, 0:1], u2[0:62, 0:62],
                    start=True, stop=True,
                )
                pk = psC.tile([128, 64], F32, tag="psC")
                nc.tensor.matmul(
                    pk[0:62, 0:1], u3[0:62, 0:62], WT["ones62"][0:62, 0:1],
                    start=True, stop=True,
                )
                ik = sp.tile([128, 2], F32, tag="ik")
                nc.scalar.activation(ik[0:62, 0:1], pk[0:62, 0:1], AF.Sqrt)
                nc.vector.tensor_scalar_max(ik[0:62, 0:1], ik[0:62, 0:1], 1e-12)
                nc.vector.reciprocal(ik[0:62, 1:2], ik[0:62, 0:1])
                scd = sp.tile([128, 1], F32, tag="scd")
                nc.vector.tensor_tensor(
                    scd[0:62, 0:1], ik[0:62, 1:2], WT["rsc"][0:62, m : m + 1],
                    op=ALU.mult,
                )
                iq = sp.tile([1, 128], F32, tag="iq")
                nc.scalar.activation(iq[0:1, 0:62], pq[0:1, 0:62], AF.Sqrt)
                nc.vector.tensor_scalar_max(iq[0:1, 0:62], iq[0:1, 0:62], 1e-12)
                nc.vector.reciprocal(iq[0:1, 64:126], iq[0:1, 0:62])
                iqb = sp.tile([128, 64], F32, tag="iqb")
                nc.gpsimd.partition_broadcast(iqb[0:62, 0:62], iq[0:1, 64:126])
                L = sp.tile([128, 64], F32, tag="L")
                nc.vector.tensor_scalar_mul(L[0:62, 0:62], SKQ[0:62, 0:62], scd[0:62, 0:1])
                nc.vector.tensor_tensor(
                    L[0:62, 0:62], L[0:62, 0:62], iqb[0:62, 0:62], op=ALU.mult
                )
                A = sp.tile([128, 64], F32, tag="A")
                nc.vector.memset(A[:], 0.0)
                nc.vector.tensor_tensor(
                    L[0:62, 0:62], L[0:62, 0:62], WT["hmA"][0:62, 0:62], op=ALU.mult
                )
                nc.vector.tensor_tensor(
                    L[0:62, 0:62], L[0:62, 0:62], WT["hmB"][0:62, 0:62], op=ALU.add
                )
                mx = sp.tile([128, 2], F32, tag="mx")
                nc.vector.tensor_reduce(
                    mx[0:62, 0:1], L[0:62, 0:62], op=ALU.max, axis=mybir.AxisListType.X
                )
                nc.vector.tensor_scalar_mul(mx[0:62, 1:2], mx[0:62, 0:1], -1.0)
                nc.scalar.activation(
                    A[0:62, 0:62], L[0:62, 0:62], AF.Exp, bias=mx[0:62, 1:2]
                )
                sm = sp.tile([128, 2], F32, tag="sm")
                nc.vector.tensor_reduce(
                    sm[0:62, 0:1], A[0:62, 0:62], op=ALU.add, axis=mybir.AxisListType.X
                )
                nc.vector.reciprocal(sm[0:62, 1:2], sm[0:62, 0:1])
                nc.vector.tensor_scalar_mul(A[0:62, 0:62], A[0:62, 0:62], sm[0:62, 1:2])
                wps = psC.tile([128, 64], F32, tag="psC")
                if odd:
                    nc.tensor.matmul(
                        wps[64:126, 0:62], A[0:62, 0:62], pw_s,
                        start=True, stop=True, tile_position=(0, 64),
                    )
                    nc.vector.tensor_copy(WoTp[64:126, 64:126], wps[64:126, 0:62])
                else:
                    nc.tensor.matmul(
                        wps[0:62, 0:62], A[0:62, 0:62], pw_s, start=True, stop=True
                    )
                    nc.vector.tensor_copy(WoTp[0:62, 0:62], wps[0:62, 0:62])

            # WoT pair tiles
            WoT = {}
            for p in PAIRS:
                WoT[p] = sp.tile([128, 128], BF16, tag=f"WoT{p}", name=f"WoT{p}")
                nc.vector.memset(WoT[p][:], 0.0)

            # =============== P23: modules 2,3 (v from z) ===============
            v23 = bp.tile([128, FA], BF16, tag="s2")
            stream_v(v23, WT["wv23"], zz, 6, kp=64)
            pad_zero(v23)
            g23 = bp.tile([128, FA], BF16, tag="s3")
            conv_pe_gelu(g23, v23, load_mdw("m_dw1_23"), WT["c3_dw1_23"], 5)
            pad_zero(g23)
            mask_rows(g23, 5)
            # attn for both L1 pairs (AR1 finished during the conv above);
            # modules 0,1 emitted after pt23 so their chain latency hides there
            attn_module(2, gmy[:, 0:128], 1, 0, WoT["23"], odd=False)
            attn_module(3, gmy[:, 128:256], 1, 0, WoT["23"], odd=True)
            fafb = bp.tile([128, FA], BF16, tag="s1")  # reuses zz slot
            pair_tail(fafb, v23, g23, WoT["23"], WT["pb23"], xy, "23", 4)
            pad_zero(fafb)
            mask_rows(fafb, 4)

            # =============== P01: modules 0,1 ===============
            attn_module(0, gmy[:, 0:128], 1, 1, WoT["01"], odd=False)
            attn_module(1, gmy[:, 128:256], 1, 1, WoT["01"], odd=True)
            v01 = bp.tile([128, FA], BF16, tag="s2")
            stream_v(v01, WT["wv01"], xy, 4)
            pad_zero(v01)
            g01 = bp.tile([128, FA], BF16, tag="s3")
            conv_pe_gelu(g01, v01, load_mdw("m_dw1_01"), WT["c3_dw1_01"], 3)
            pad_zero(g01)
            mask_rows(g01, 3)
            # in-place residual: xa|yb overwrites the xy slab (resid read and
            # output write are the same STT op, chunk by chunk)
            xaYb = xy
            pair_tail(xaYb, v01, g01, WoT["01"], WT["pb01"], xy, "01", 2)
            pad_zero(xaYb)
            mask_rows(xaYb, 2)

            # =============== L2 Grams + AR2 (all sources resident) ===============
            tpa = bp.tile([128, NCK * 128], BF16, tag="s2")  # v01 dead
            gram_half(tpa, xaYb, 0, 0, nc.sync)
            gram_half(tpa, fafb, 0, 64, nc.scalar)
            tpb = bp.tile([128, NCK * 128], BF16, tag="s3")  # g01 dead
            gram_half(tpb, xaYb, 64, 0, nc.sync)
            gram_half(tpb, fafb, 64, 64, nc.scalar)
            arin2_sb = sp.tile([128, 4 * 128], F32, tag="arin_sb")
            for gi, tp_t in enumerate([tpa, tpb]):
                gps = psC.tile([128, 128], F32, tag="psC")
                gram_mms(gps, tp_t)
                nc.vector.tensor_scalar_mul(
                    arin2_sb[:, gi * 128 : (gi + 1) * 128], gps[:], SEL0
                )
                nc.vector.tensor_scalar_mul(
                    arin2_sb[:, (2 + gi) * 128 : (3 + gi) * 128], gps[:], SEL1
                )
            arin2 = dp.tile([128, 4 * 128], F32, tag="arin2")
            arout2 = dp.tile([128, 4 * 128], F32, tag="arout2", addr_space="Shared")
            nc.sync.dma_start(arin2[:], arin2_sb[:])
            nc.gpsimd.collective_compute(
                "AllReduce", ALU.add, replica_groups=[list(range(8))],
                ins=[arin2.opt()], outs=[arout2.opt()],
            )
            g2sb = sp.tile([128, 4 * 128], F32, tag="g1sb")
            nc.sync.dma_start(g2sb[:], arout2[:])
            gmy2 = sp.tile([128, 2 * 128], F32, tag="gmy")
            for gi in range(2):
                nc.vector.tensor_scalar_mul(
                    gmy2[:, gi * 128 : (gi + 1) * 128],
                    g2sb[:, gi * 128 : (gi + 1) * 128], SEL0,
                )
                nc.vector.scalar_tensor_tensor(
                    gmy2[:, gi * 128 : (gi + 1) * 128],
                    g2sb[:, (2 + gi) * 128 : (3 + gi) * 128], SEL1,
                    gmy2[:, gi * 128 : (gi + 1) * 128],
                    op0=ALU.mult, op1=ALU.add,
                )

            # =============== P45 (v45/g45 overlap AR2 latency) ===============
            v45 = bp.tile([128, FA], BF16, tag="s2")  # tpa dead after gA MMs
            stream_v(v45, WT["wv45"], fafb, 4)
            pad_zero(v45)
            g45 = bp.tile([128, FA], BF16, tag="s3")  # tpb dead after gB MMs
            mdw45 = load_mdw("m_dw1_45")
            conv_pe_gelu(g45, v45, mdw45, WT["c3_dw1_45"], 3, part=(0, 2))
            # stacks are [xa|fa]: q=xa blk0, k=fa blk1 (chain hides in conv 2nd half)
            attn_module(4, gmy2[:, 0:128], 0, 1, WoT["45"], odd=False)
            attn_module(5, gmy2[:, 128:256], 0, 1, WoT["45"], odd=True)
            conv_pe_gelu(g45, v45, mdw45, WT["c3_dw1_45"], 3, part=(1, 2))
            pad_zero(g45)
            mask_rows(g45, 3)
            t12 = bp.tile([128, FA], BF16, tag="s1")  # fafb dead after v45/gram
            pair_tail(t12, v45, g45, WoT["45"], WT["pb45"], xaYb, "45", 2)
            pad_zero(t12)
            mask_rows(t12, 2)

            # =============== LN stats (both chains) ===============
            lo2, hi2 = exr(2)
            lo1, hi1 = exr(1)
            L2p = 64 * 275  # 17600 staging per stat row
            stat_all_d = dp.tile([2, 2 * L2p], F32, tag="stat_all_d")

            sq = bp.tile([128, FA], BF16, tag="s2")  # reuses v45 slot
            for lo, hi in chunks((lo2, hi2), 4096):
                nc.scalar.activation(sq[:, lo:hi], t12[:, lo:hi], AF.Square)
            for si, srct in enumerate([t12, sq]):
                base = si * L2LEN
                for ci2, (lo, hi) in enumerate(chunks((lo2, hi2), 512)):
                    ps = psA.tile([128, 1024], F32, tag="psA")
                    nc.tensor.matmul(
                        ps[0:2, 0 : hi - lo], WT["lnones"][:],
                        srct[:, lo:hi], start=True, stop=True,
                    )
                    stch = rp.tile([2, 512], F32, tag="stch", bufs=2)
                    if ci2 % 2 == 0:
                        nc.scalar.copy(stch[:, 0 : hi - lo], ps[0:2, 0 : hi - lo])
                    else:
                        nc.vector.tensor_copy(
                            stch[:, 0 : hi - lo], ps[0:2, 0 : hi - lo]
                        )
                    qe = nc.sync if ci2 % 2 == 0 else nc.scalar
                    qe.dma_start(
                        stat_all_d[:, base + lo - lo2 : base + hi - lo2],
                        stch[:, 0 : hi - lo],
                    )

            # per-pixel inv-std / mu*inv rows for both chains, upfront
            row_ds = {}
            for ci in (0, 1):
                row_dX = dp.tile([1, 2 * L2p], BF16, tag=f"row_d{ci}")
                mu64 = sp.tile([64, 275], F32, tag="mu64")
                ms64 = sp.tile([64, 275], F32, tag="ms64")
                nc.sync.dma_start(
                    mu64[:],
                    stat_all_d[ci : ci + 1, 0:L2p].rearrange(
                        "o (p f) -> (o p) f", p=64
                    ),
                )
                nc.sync.dma_start(
                    ms64[:],
                    stat_all_d[ci : ci + 1, L2LEN : L2LEN + L2p].rearrange(
                        "o (p f) -> (o p) f", p=64
                    ),
                )
                var = sp.tile([64, 275], F32, tag="var64")
                nc.vector.tensor_tensor(var[:], mu64[:], mu64[:], op=ALU.mult)
                nc.vector.tensor_tensor(var[:], ms64[:], var[:], op=ALU.subtract)
                sd = sp.tile([64, 275], F32, tag="sd64")
                nc.vector.tensor_scalar_add(var[:], var[:], 1e-5)
                nc.scalar.activation(sd[:], var[:], AF.Sqrt)
                inv = sp.tile([64, 275], F32, tag="inv64")
                nc.vector.reciprocal(inv[:], sd[:])
                invb = sp.tile([64, 275], BF16, tag="invb64")
                nc.vector.tensor_copy(invb[:], inv[:])
                musb = sp.tile([64, 275], BF16, tag="musb64")
                nc.vector.tensor_tensor(var[:], mu64[:], inv[:], op=ALU.mult)
                nc.vector.tensor_copy(musb[:], var[:])
                nc.sync.dma_start(
                    row_dX[0:1, 0:L2p].rearrange("o (p f) -> (o p) f", p=64),
                    invb[:],
                )
                nc.sync.dma_start(
                    row_dX[0:1, L2p : 2 * L2p].rearrange("o (p f) -> (o p) f", p=64),
                    musb[:],
                )
                row_ds[ci] = row_dX

            # =============== FF per chain (ring-buffered ts) ===============
            for ci, cn in ((0, "a"), (1, "b")):
                r0 = 0 if ci == 0 else 64
                row_dX = row_ds[ci]
                th = {}
                w1ts = [WT[f"w1aug_{cn}0"], WT[f"w1aug_{cn}1"]]
                for h, hn in ((0, "0"), (1, "1")):
                    tht = bp.tile(
                        [128, FA], BF16, tag=("s3" if h == 0 else "s2"),
                        name=f"th{h}_{cn}",
                    )
                    th[h] = tht
                # w1 for both halves per chunk; ts built on the fly in a ring
                for ck2, (lo, hi) in enumerate(chunks((lo2, hi2), 1024)):
                    n = hi - lo
                    qa = nc.sync if ck2 % 2 == 0 else nc.scalar
                    qb = nc.scalar if ck2 % 2 == 0 else nc.sync
                    sbr = rp.tile([128, 1024], BF16, tag="sbr", bufs=2)
                    qa.dma_start(
                        sbr[r0 : r0 + 62, 0:n],
                        row_dX[0:1, lo - lo2 : hi - lo2].partition_broadcast(62),
                    )
                    tsr = rp.tile([128, 1024], BF16, tag="tsr", bufs=1)
                    nc.vector.tensor_tensor(
                        tsr[r0 : r0 + 62, 0:n], t12[r0 : r0 + 62, lo:hi],
                        sbr[r0 : r0 + 62, 0:n], op=ALU.mult,
                    )
                    qb.dma_start(
                        tsr[r0 + 62 : r0 + 63, 0:n],
                        row_dX[0:1, L2p + lo - lo2 : L2p + hi - lo2],
                    )
                    for h in (0, 1):
                        ps = psA.tile([128, 1024], F32, tag="psA")
                        for l2, h2 in chunks((lo, hi), NT):
                            nc.tensor.matmul(
                                ps[0:124, l2 - lo : h2 - lo],
                                w1ts[h][r0 : r0 + 63, :],
                                tsr[r0 : r0 + 63, l2 - lo : h2 - lo],
                                start=True, stop=True,
                            )
                        nc.scalar.activation(
                            th[h][0:124, lo:hi], ps[0:124, 0 : hi - lo], AF.Gelu,
                            bias=WT[f"c1b_{cn}{'0' if h == 0 else '1'}"][0:124, :],
                        )
                for h in (0, 1):
                    pad_zero(th[h])
                    mask_rows(th[h], 2, (0, 124))
                # conv h0/h1 + gelu + w2, chunk-wise (convs on PE)
                for lo, hi in chunks((lo1, hi1), 1024):
                    wps = psA.tile([128, 1024], F32, tag="psA")
                    for h, hn in ((0, "0"), (1, "1")):
                        ghc = rp.tile([128, 1024], BF16, tag="ghc", bufs=2)
                        cps = psA.tile([128, 1024], F32, tag="psA")
                        mats = load_mdw(f"m_ffdw_{cn}{hn}")
                        dw_taps_split(
                            cps, th[h], mats, WT[f"c3_ffdw_{cn}{hn}"],
                            lo, hi, start=True,
                        )
                        nc.scalar.activation(
                            ghc[0:124, 0 : hi - lo], cps[0:124, 0 : hi - lo],
                            AF.Gelu,
                        )
                        w2t = WT[f"w2h_{cn}{hn}"]
                        for l2, h2 in chunks((lo, hi), NT):
                            nc.tensor.matmul(
                                wps[r0 : r0 + 62, l2 - lo : h2 - lo],
                                w2t[0:124, 0:62],
                                ghc[0:124, l2 - lo : h2 - lo],
                                start=(h == 0), stop=(h == 1),
                                tile_position=(0, 64) if ci == 1 else None,
                            )
                    # t12 += w2 out (in place, one rounding)
                    nc.vector.scalar_tensor_tensor(
                        t12[r0 : r0 + 62, lo:hi], wps[r0 : r0 + 62, 0 : hi - lo],
                        1.0, t12[r0 : r0 + 62, lo:hi], op0=ALU.mult, op1=ALU.add,
                    )

            # =============== fusions + final ===============
            fin = bp.tile([128, FA], BF16, tag="s3")
            nc.sync.dma_start(fin[64:126, lo1:hi1], P["zc"][0:62, :])
            fab = bp.tile([128, FA], BF16, tag="s2")
            for lo, hi in chunks((lo1, hi1), 1024):
                ps = psA.tile([128, 1024], F32, tag="psA")
                for l2, h2 in chunks((lo, hi), NT):
                    nc.tensor.matmul(
                        ps[:, l2 - lo : h2 - lo], WT["wfut"][:], t12[:, l2:h2],
                        start=True, stop=False,
                    )
                    nc.tensor.matmul(
                        ps[:, l2 - lo : h2 - lo], WT["wfux"][:], xaYb[:, l2:h2],
                        start=False, stop=True,
                    )
                nc.scalar.add(fab[:, lo:hi], ps[:, 0 : hi - lo], WT["fucb"][:])
            pad_zero(fab)
            mask_rows(fab, 1)
            for lo, hi in chunks((lo1, hi1), 1024):
                ps = psA.tile([128, 1024], F32, tag="psA")
                for l2, h2 in chunks((lo, hi), NT):
                    nc.tensor.matmul(
                        ps[0:64, l2 - lo : h2 - lo], WT["outw"][:], fab[:, l2:h2],
                        start=True, stop=True,
                    )
                nc.scalar.add(fin[0:62, lo:hi], ps[0:62, 0 : hi - lo], WT["outb"][0:62, :])
            pad_zero(fin)
            mask_rows(fin, 1, (0, 62))
            # final 3x3 conv, direct DMA out in 2-row (516 col) pieces.
            # each 258-col row-piece goes to its own PSUM bank (offsets 0, 512)
            lo0, hi0 = exr(0)
            out3 = out_p[:].rearrange("c (r w) -> c r w", w=W)
            for lo, hi in chunks((lo0, hi0), 516):
                ps = psA.tile([128, 1024], F32, tag="psA")
                for t in range(9):
                    dy, dx = TAPS[t]
                    d = dy * RS + dx
                    for bi, (l2, h2) in enumerate(((lo, lo + 258), (lo + 258, hi))):
                        nc.tensor.matmul(
                            ps[0:64, bi * 512 : bi * 512 + 258],
                            WT["finw"][:, t * 64 : (t + 1) * 64],
                            fin[:, l2 + d : h2 + d],
                            start=(t == 0), stop=(t == 8),
                        )
                och = rp.tile([62, 516], F32, tag="och", bufs=1)
                ps3 = ps[:].rearrange("p (b c) -> p b c", c=512)[0:62, :, 0:258]
                nc.scalar.add(
                    och[:].rearrange("c (r s) -> c r s", s=RS), ps3,
                    WT["finb"][0:62, :],
                )
                r0 = (lo - lo0) // RS
                nc.sync.dma_start(
                    out3[:, r0 : r0 + 2, :],
                    och[:].rearrange("c (r s) -> c r s", s=RS)[:, :, 0:W],
                )

    nc.finalize()
    return nc


_NC_CACHE = {}


def _run(inputs, trace=False):
    if "nc" not in _NC_CACHE:
        _NC_CACHE["nc"] = build_nc()
    nc = _NC_CACHE["nc"]
    names = {
        a.name.removesuffix("_set")
        for a in nc.m.functions[0].allocations
        if getattr(a, "kind", None) == "ExternalInput"
    }
    in_maps = prep_host_inputs(inputs)
    in_maps = [{k: v for k, v in m.items() if k in names} for m in in_maps]
    res = run_bass_kernel_spmd(
        nc, in_maps, core_ids=list(range(8)), trace=trace
    )
    return assemble_output(res.results), res


def kernel(**inputs):
    out, _ = _run(inputs, trace=False)
    return out


# revision 27
# speedup vs baseline: 1.1097x; 1.0157x over previous
"""Bass/Trainium2 kernel for nn_Block_14010183320003 (MST++-style block).

Sharding: 8 cores = 2 batches x 4 row-quarters (64 owned rows each, halo 6).
Chain packing: partitions 0:62 = x-derived chain A, 64:126 = y-derived chain B.
Attention is computed from 124-dim input Gram matrices (no q/k tensors);
per-core partial Grams are summed with two ~256KB AllReduces.
Pixel-major Gram stacks are built ON DEVICE with batched DMA xbar transposes
(few big DMA_TRANSPOSE ops instead of per-chunk ones).
Row layout: stride 258 per row (2 zero pad cols) so 3x3 convs are free-axis
shifted reads; depthwise convs run as PE diag-matmul accumulation.
"""
import sys

sys.path.insert(0, "/opt/trn_rl_repo")
import numpy as np
import ml_dtypes

import concourse.bass as bass
import concourse.mybir as mybir
import concourse.tile as tile
import concourse.bacc as bacc
from concourse.bass_utils import run_bass_kernel_spmd

BF16, F32 = mybir.dt.bfloat16, mybir.dt.float32
AF = mybir.ActivationFunctionType
ALU = mybir.AluOpType
bf = ml_dtypes.bfloat16

C = 62
W = 256
RS = 258  # row stride (2 zero pad cols)
OWN = 64
HALO = 6
R = 76
ROFF = 1  # lead pad row at tile row 0 (zero pad for shifted reads)
RA = 78  # lead pad + 76 slab rows + 1 trailing spare
FA = RA * RS  # 20124
NT = 512  # matmul moving chunk
NH, DH = 2, 31
GSPAN0 = (HALO + ROFF) * RS  # owned-pixel span start = 1806
NCK = 129  # owned span 64*258 = 129 chunks of 128

TAPS = [(dy, dx) for dy in (-1, 0, 1) for dx in (-1, 0, 1)]
T6 = [0, 2, 3, 5, 6, 8]  # taps with dx != 0 -> PE diag matmuls
TD = [1, 4, 7]  # dx == 0 taps (dy=-1,0,+1) -> DVE scalar chains
D6 = [TAPS[t][0] * RS + TAPS[t][1] for t in T6]


def exr(e):
    """free range of extent e (slab rows [6-e, 70+e)), incl lead-row offset."""
    return ((HALO - e + ROFF) * RS, (HALO + OWN + e + ROFF) * RS)


def _slab(full, row0):
    """full [C, 256, 256] -> [C, R, RS] zero-padded slab (rows row0..row0+R)."""
    out = np.zeros((C, R, RS), np.float32)
    lo, hi = max(0, row0), min(256, row0 + R)
    out[:, lo - row0 : hi - row0, :W] = full[:, lo:hi]
    return out


def _to_bf(a):
    return np.ascontiguousarray(a.astype(bf))


def _diag_taps6(kc):
    """kc: [128, 9] -> [128, 7*128] bf16: 6 dx!=0 tap diags + identity."""
    out = np.zeros((128, 7 * 128), np.float32)
    ar = np.arange(128)
    for j, t in enumerate(T6):
        out[:, j * 128 : (j + 1) * 128][ar, ar] = kc[:, t]
    out[:, 6 * 128 : 7 * 128][ar, ar] = 1.0
    return _to_bf(out)


def _pair128(a_block, b_block):
    out = np.zeros((128, 128), np.float32)
    out[0:62, 0:62] = a_block
    out[64:126, 64:126] = b_block
    return _to_bf(out)


def prep_host_inputs(inputs):
    """Build the 8 per-core input maps."""
    inp = {k: np.asarray(v, dtype=np.float32) for k, v in inputs.items()}
    wq, wk, wv = inp["attn_wq"], inp["attn_wk"], inp["attn_wv"]
    pw, pb = inp["attn_pw"], inp["attn_pb"]
    dw1, dw2 = inp["attn_dw1"], inp["attn_dw2"]
    resc = inp["attn_rescale"]

    shared = {}
    # v-projection pair weights [128,128].
    # wv01/wv45: block-diag (K rows 0:62 -> cols 0:62 even module, K 64:126 ->
    # cols 64:126 odd module). wv23: single-z, K rows 0:62 feed BOTH col blocks.
    shared["wv01"] = _pair128(wv[0], wv[1])
    shared["wv45"] = _pair128(wv[4], wv[5])
    wv23 = np.zeros((128, 128), np.float32)
    wv23[0:62, 0:62] = wv[2]
    wv23[0:62, 64:126] = wv[3]
    shared["wv23"] = _to_bf(wv23)
    for pair, (me, mo) in {"01": (0, 1), "23": (2, 3), "45": (4, 5)}.items():
        pbv = np.zeros((128, 1), np.float32)
        pbv[0:62, 0] = pb[me]
        pbv[64:126, 0] = pb[mo]
        shared[f"pb{pair}"] = pbv
        for cn, dwk in (("1", dw1), ("2", dw2)):
            kc = np.zeros((128, 9), np.float32)
            for t, (dy, dx) in enumerate(TAPS):
                kc[0:62, t] = dwk[me][:, dy + 1, dx + 1]
                kc[64:126, t] = dwk[mo][:, dy + 1, dx + 1]
            shared[f"m_dw{cn}_{pair}"] = _diag_taps6(kc)
            shared[f"c3_dw{cn}_{pair}"] = np.ascontiguousarray(kc[:, TD])
    # attn small weights: [128, 6*64] col block m at m*64, rows 0:62
    for nm, src in (("wqm", wq), ("wkm", wk), ("pwm", pw)):
        t = np.zeros((128, 6 * 64), np.float32)
        for m in range(6):
            t[0:62, m * 64 : m * 64 + 62] = src[m]
        shared[nm] = t
    rs = np.zeros((128, 6), np.float32)
    for m in range(6):
        rs[0:62, m] = np.repeat(resc[m], DH)
    shared["rsc"] = rs
    ones62 = np.zeros((128, 1), np.float32)
    ones62[0:62, 0] = 1.0
    shared["ones62"] = ones62
    hmA = np.zeros((128, 64), np.float32)
    hmB = np.full((128, 64), -1e4, np.float32)
    for h in range(NH):
        hmA[h * DH : (h + 1) * DH, h * DH : (h + 1) * DH] = 1.0
        hmB[h * DH : (h + 1) * DH, h * DH : (h + 1) * DH] = 0.0
    shared["hmA"] = hmA
    shared["hmB"] = hmB
    lnones = np.zeros((128, 2), np.float32)
    lnones[0:62, 0] = 1.0 / C
    lnones[64:126, 1] = 1.0 / C
    shared["lnones"] = _to_bf(lnones)

    # FF weights. chain A uses ff index 0, chain B index 1.
    for ci, cname in ((0, "a"), (1, "b")):
        g, b = inp["ln_g"][ci], inp["ln_b"][ci]
        w1 = inp["ff_w1"][ci]  # [62, 248]
        w2 = inp["ff_w2"][ci]  # [248, 62]
        w1p = g[:, None] * w1
        c2 = w1p.sum(axis=0)  # [248]
        c1 = b @ w1  # [248]
        dwk = inp["ff_dw"][ci]  # [248, 3, 3]
        r0 = 0 if ci == 0 else 64
        for h in (0, 1):
            sl = slice(h * 124, (h + 1) * 124)
            wt = np.zeros((128, 124), np.float32)
            wt[r0 : r0 + 62, :] = w1p[:, sl]
            wt[r0 + 62, :] = -c2[sl]
            shared[f"w1aug_{cname}{h}"] = _to_bf(wt)
            cb = np.zeros((128, 1), np.float32)
            cb[0:124, 0] = c1[sl]
            shared[f"c1b_{cname}{h}"] = cb
            kc = np.zeros((128, 9), np.float32)
            for t, (dy, dx) in enumerate(TAPS):
                kc[0:124, t] = dwk[sl, dy + 1, dx + 1]
            shared[f"m_ffdw_{cname}{h}"] = _diag_taps6(kc)
            shared[f"c3_ffdw_{cname}{h}"] = np.ascontiguousarray(kc[:, TD])
            w2t = np.zeros((128, 128), np.float32)
            w2t[0:124, 0:62] = w2[sl, :]
            shared[f"w2h_{cname}{h}"] = _to_bf(w2t)

    # fusion weights: fab = Wfu_t^T @ t12 + Wfu_x^T @ xaYb + fucb
    wt_ = np.zeros((128, 128), np.float32)
    wt_[0:62, 0:62] = inp["fuc1_w"][:, 0:62].T
    wt_[64:126, 64:126] = inp["fuc2_w"][:, 0:62].T
    shared["wfut"] = _to_bf(wt_)
    wx_ = np.zeros((128, 128), np.float32)
    wx_[0:62, 0:62] = inp["fuc1_w"][:, 62:124].T
    wx_[64:126, 64:126] = inp["fuc2_w"][:, 62:124].T
    shared["wfux"] = _to_bf(wx_)
    fucb = np.zeros((128, 1), np.float32)
    fucb[0:62, 0] = inp["fuc1_b"]
    fucb[64:126, 0] = inp["fuc2_b"]
    shared["fucb"] = fucb
    ow = np.zeros((128, 64), np.float32)
    ow[0:62, 0:62] = inp["outc_w"][:, 0:62].T
    ow[64:126, 0:62] = inp["outc_w"][:, 62:124].T
    shared["outw"] = _to_bf(ow)
    ob = np.zeros((128, 1), np.float32)
    ob[0:62, 0] = inp["outc_b"]
    shared["outb"] = ob
    fw = np.zeros((128, 9 * 64), np.float32)
    for t, (dy, dx) in enumerate(TAPS):
        fw[0:62, t * 64 : t * 64 + 62] = inp["final_w"][:, 0:62, dy + 1, dx + 1].T
        fw[64:126, t * 64 : t * 64 + 62] = inp["final_w"][:, 62:124, dy + 1, dx + 1].T
    shared["finw"] = _to_bf(fw)
    fb_ = np.zeros((128, 1), np.float32)
    fb_[0:62, 0] = inp["final_b"]
    shared["finb"] = fb_

    for k in list(shared.keys()):
        if shared[k].dtype == np.float32:
            shared[k] = np.ascontiguousarray(shared[k])

    in_maps = []
    for b in range(2):
        xb, yb_, zb = inp["x"][b], inp["y"][b], inp["z"][b]
        for s in range(4):
            row0 = 64 * s - HALO
            xs, ys, zs = _slab(xb, row0), _slab(yb_, row0), _slab(zb, row0)
            m = {}
            xy = np.zeros((128, R, RS), np.float32)
            xy[0:62], xy[64:126] = xs, ys
            m["xy"] = _to_bf(xy.reshape(128, R * RS))
            zs64 = np.zeros((64, R * RS), np.float32)
            zs64[0:62] = zs.reshape(C, R * RS)
            m["zs"] = _to_bf(zs64)
            m["zc"] = _to_bf(zs[:, 5:71].reshape(62, 66 * RS))
            def pixmaj(t1, t2):
                g = np.zeros((OWN * W, 128), np.float32)
                g[:, 0:62] = t1[:, HALO : HALO + OWN, :W].reshape(C, -1).T
                g[:, 64:126] = t2[:, HALO : HALO + OWN, :W].reshape(C, -1).T
                # -> [p, k, c] flattened: partition-major chunks for direct load
                return _to_bf(
                    np.ascontiguousarray(
                        g.reshape(128, 128, 128).transpose(1, 0, 2)
                    ).reshape(128, 128 * 128)
                )
            m["gzx"] = pixmaj(zs, xs)
            m["gzy"] = pixmaj(zs, ys)
            msk = np.zeros((128, 4), np.float32)
            msk[:, 0] = 0.0 if s == 0 else 1.0
            msk[:, 1] = 0.0 if s == 3 else 1.0
            msk[:, 2] = 1.0 if b == 0 else 0.0
            msk[:, 3] = 1.0 if b == 1 else 0.0
            m["msk"] = msk
            m.update(shared)
            in_maps.append(m)
    return in_maps


def assemble_output(results):
    out = np.zeros((2, C, 256, 256), np.float32)
    for b in range(2):
        for s in range(4):
            r = results[b * 4 + s]["out"]  # [62, 64*256]
            out[b, :, 64 * s : 64 * (s + 1), :] = r.reshape(C, OWN, W)
    return out


# ---------------------------------------------------------------------------
# device IR
# ---------------------------------------------------------------------------

PAIRS = ["01", "23", "45"]
L2LEN = exr(2)[1] - exr(2)[0]  # 17544


def build_nc():
    nc = bacc.Bacc(None, target_bir_lowering=False, debug=False)

    P = {}
    P["xy"] = nc.declare_dram_parameter("xy", [128, R * RS], BF16, isOutput=False)
    P["zs"] = nc.declare_dram_parameter("zs", [64, R * RS], BF16, isOutput=False)
    P["zc"] = nc.declare_dram_parameter("zc", [C, 66 * RS], BF16, isOutput=False)
    P["gzx"] = nc.declare_dram_parameter("gzx", [128, OWN * W], BF16, isOutput=False)
    P["gzy"] = nc.declare_dram_parameter("gzy", [128, OWN * W], BF16, isOutput=False)
    P["msk"] = nc.declare_dram_parameter("msk", [128, 4], F32, isOutput=False)
    wnames_bf = (
        [f"wv{p}" for p in PAIRS]
        + ["lnones"]
        + [f"w1aug_{c}{h}" for c in "ab" for h in "01"]
        + [f"w2h_{c}{h}" for c in "ab" for h in "01"]
        + ["wfut", "wfux", "outw", "finw"]
    )
    wnames_f32 = (
        [f"pb{p}" for p in PAIRS]
        + ["rsc", "wqm", "wkm", "pwm", "ones62", "hmA", "hmB"]
        + [f"c1b_{c}{h}" for c in "ab" for h in "01"]
        + [f"c3_dw{c}_{p}" for c in "12" for p in PAIRS]
        + [f"c3_ffdw_{c}{h}" for c in "ab" for h in "01"]
        + ["fucb", "outb", "finb"]
    )
    WSHAPE = {
        "wqm": [128, 6 * 64], "wkm": [128, 6 * 64], "pwm": [128, 6 * 64],
        "ones62": [128, 1], "lnones": [128, 2], "rsc": [128, 6],
        "outw": [128, 64], "finw": [128, 9 * 64],
        "hmA": [128, 64], "hmB": [128, 64],
        "wfut": [128, 128], "wfux": [128, 128],
        "fucb": [128, 1], "outb": [128, 1], "finb": [128, 1],
    }
    for p in PAIRS:
        WSHAPE[f"wv{p}"] = [128, 128]
        WSHAPE[f"pb{p}"] = [128, 1]
        for c in "12":
            WSHAPE[f"m_dw{c}_{p}"] = [128, 7 * 128]
            WSHAPE[f"c3_dw{c}_{p}"] = [128, 3]
    for c in "ab":
        for h in "01":
            WSHAPE[f"w1aug_{c}{h}"] = [128, 124]
            WSHAPE[f"m_ffdw_{c}{h}"] = [128, 7 * 128]
            WSHAPE[f"c3_ffdw_{c}{h}"] = [128, 3]
            WSHAPE[f"c1b_{c}{h}"] = [128, 1]
            WSHAPE[f"w2h_{c}{h}"] = [128, 128]
    mnames = [f"m_dw{c}_{p}" for c in "12" for p in PAIRS] + [
        f"m_ffdw_{c}{h}" for c in "ab" for h in "01"
    ]
    for nm in wnames_bf + mnames:
        P[nm] = nc.declare_dram_parameter(nm, WSHAPE[nm], BF16, isOutput=False)
    for nm in wnames_f32:
        P[nm] = nc.declare_dram_parameter(nm, WSHAPE[nm], F32, isOutput=False)
    out_p = nc.declare_dram_parameter("out", [C, OWN * W], F32, isOutput=True)

    with tile.TileContext(nc, num_cores=8) as tc:
        with (
            tc.tile_pool(name="w", bufs=1) as wp,
            tc.tile_pool(name="small", bufs=1) as sp,
            tc.tile_pool(name="big", bufs=1) as bp,
            tc.tile_pool(name="ring", bufs=3) as rp,
            tc.tile_pool(name="psA", bufs=3, space="PSUM") as psA,
            tc.tile_pool(name="psC", bufs=2, space="PSUM") as psC,
            tc.tile_pool(name="dram", bufs=1, space="DRAM") as dp,
        ):
            # ---------- gram stacks first (they gate the first PE work) ----------
            NCK1 = OWN * W // 128  # 128 chunks
            halfc = NCK1 * 128 // 2
            tpgx = bp.tile([128, NCK1 * 128], BF16, tag="s2")
            tpgy = bp.tile([128, NCK1 * 128], BF16, tag="s3")
            zz = bp.tile([128, FA], BF16, tag="s1")
            xy = bp.tile([128, FA], BF16, tag="xy")
            # SP queue: first gram half, then zs (v23's input), then the rest;
            # ACT queue: gzy halves (weights follow later on ACT)
            nc.sync.dma_start(tpgx[:, 0:halfc], P["gzx"][:, 0:halfc])
            nc.sync.dma_start(zz[0:64, RS : RS + R * RS], P["zs"][:])
            nc.sync.dma_start(tpgx[:, halfc:], P["gzx"][:, halfc:])
            nc.scalar.dma_start(tpgy[:, 0:halfc], P["gzy"][:, 0:halfc])
            nc.scalar.dma_start(tpgy[:, halfc:], P["gzy"][:, halfc:])
            nc.sync.dma_start(xy[0:64, RS : RS + R * RS], P["xy"][0:64, :])
            nc.sync.dma_start(xy[64:128, RS : RS + R * RS], P["xy"][64:128, :])

            def load_mdw(nm):
                t_ = rp.tile([128, 7 * 128], BF16, tag="mdw", bufs=2, name=nm + "_l")
                nc.scalar.dma_start(t_[:], P[nm][:])
                return t_

            def chunks(rng, step):
                lo, hi = rng
                out = []
                while lo < hi:
                    out.append((lo, min(lo + step, hi)))
                    lo += step
                return out

            def pad_zero(t, lo_row=0, hi_row=RA):
                v = t[:].rearrange("p (r s) -> p r s", s=RS)[:, lo_row:hi_row, W : W + 2]
                nc.vector.memset(v, 0.0)

            def mask_rows(t, e, dtype_rows=(0, 128)):
                if e <= 0:
                    return
                r0, r1 = dtype_rows
                top = t[r0:r1, (HALO - e + ROFF) * RS : (HALO + ROFF) * RS]
                bot = t[r0:r1, (HALO + OWN + ROFF) * RS : (HALO + OWN + e + ROFF) * RS]
                nc.vector.tensor_scalar_mul(top, top, MTOP[r0:r1])
                nc.vector.tensor_scalar_mul(bot, bot, MBOT[r0:r1])

            def stream_v(dst, wv_t, src, e, kp=128):
                """dst[:, rng] = (wv_t.T @ src)[:, rng] via psA, ACT copy evac."""
                for lo, hi in chunks(exr(e), 1024):
                    ps = psA.tile([128, 1024], F32, tag="psA")
                    for l2, h2 in chunks((lo, hi), NT):
                        nc.tensor.matmul(
                            ps[:, l2 - lo : h2 - lo], wv_t[0:kp, :], src[0:kp, l2:h2],
                            start=True, stop=True,
                        )
                    nc.scalar.copy(dst[:, lo:hi], ps[:, 0 : hi - lo])

            def dw_taps_split(ps, src, mats7, cv, lo, hi, start):
                """accumulate depthwise 3x3 of src into ps[:, 0:hi-lo]:
                6 dx!=0 taps as PE diag-MMs; 3 dx=0 taps on DVE into a bf16
                scratch merged by an identity diag-MM (block 6 of mats7)."""
                n = hi - lo
                scr = rp.tile([128, 1024], BF16, tag="scr", bufs=2)
                nc.vector.tensor_scalar_mul(
                    scr[:, 0:n], src[:, lo - RS : hi - RS], cv[:, 0:1]
                )
                nc.vector.scalar_tensor_tensor(
                    scr[:, 0:n], src[:, lo:hi], cv[:, 1:2], scr[:, 0:n],
                    op0=ALU.mult, op1=ALU.add,
                )
                nc.vector.scalar_tensor_tensor(
                    scr[:, 0:n], src[:, lo + RS : hi + RS], cv[:, 2:3], scr[:, 0:n],
                    op0=ALU.mult, op1=ALU.add,
                )
                for l2, h2 in chunks((lo, hi), NT):
                    for j, d in enumerate(D6):
                        nc.tensor.matmul(
                            ps[:, l2 - lo : h2 - lo],
                            mats7[:, j * 128 : (j + 1) * 128],
                            src[:, l2 + d : h2 + d],
                            start=(start and j == 0), stop=False,
                        )
                    nc.tensor.matmul(
                        ps[:, l2 - lo : h2 - lo],
                        mats7[:, 6 * 128 : 7 * 128],
                        scr[:, l2 - lo : h2 - lo],
                        start=False, stop=True,
                    )

            def conv_pe_gelu(dst, src, mats, cv, e_out, part=None):
                """dst = gelu(dwconv(src)) over extent e_out (split PE/DVE).
                part=(i, n) emits only the i-th of n chunk groups."""
                cl = chunks(exr(e_out), 1024)
                if part is not None:
                    i, n = part
                    cl = cl[i * len(cl) // n : (i + 1) * len(cl) // n]
                for lo, hi in cl:
                    ps = psA.tile([128, 1024], F32, tag="psA")
                    dw_taps_split(ps, src, mats, cv, lo, hi, start=True)
                    nc.scalar.activation(dst[:, lo:hi], ps[:, 0 : hi - lo], AF.Gelu)

            def pair_tail(out_t, v_t, g_t, WoTp, pb_t, resid, pair, e_out):
                """out_t = WoT^T v + dw2(g) + pb + resid (split PE/DVE conv)."""
                mats = load_mdw(f"m_dw2_{pair}")
                cv = WT[f"c3_dw2_{pair}"]
                for lo, hi in chunks(exr(e_out), 1024):
                    ps = psA.tile([128, 1024], F32, tag="psA")
                    for l2, h2 in chunks((lo, hi), NT):
                        nc.tensor.matmul(
                            ps[:, l2 - lo : h2 - lo], WoTp[:], v_t[:, l2:h2],
                            start=True, stop=False,
                        )
                    dw_taps_split(ps, g_t, mats, cv, lo, hi, start=False)
                    nc.vector.scalar_tensor_tensor(
                        out_t[:, lo:hi], ps[:, 0 : hi - lo], pb_t[:],
                        resid[:, lo:hi], op0=ALU.add, op1=ALU.add,
                    )

            # ---------- gram helpers (batched xbar transposes) ----------
            KSECS = [(0, 33), (33, 65), (65, 97), (97, NCK)]

            def tp3v(tp_tile):
                return tp_tile[:].rearrange("p (k c) -> p k c", c=128)

            def gram_half(tp_tile, src, p0, c0, eng):
                """tp[p, k*128 + c0 + f] = src[p0+f, span k*128+p] via xbar
                transpose DMAs, sectioned so gram MMs pipeline per section."""
                tp3 = tp3v(tp_tile)
                for k0, k1 in KSECS:
                    eng.dma_start(
                        tp3[:, k0:k1, c0 : c0 + 64],
                        src[p0 : p0 + 64,
                            GSPAN0 + k0 * 128 : GSPAN0 + k1 * 128],
                        transpose=True,
                    )

            def gram_mms(gt, tp_tile, nck=NCK):
                for ck in range(nck):
                    op = tp_tile[:, ck * 128 : (ck + 1) * 128]
                    nc.tensor.matmul(
                        gt[:], op, op, start=(ck == 0), stop=(ck == nck - 1)
                    )

            # =============== L1 Grams + AR1 ===============

            # ---------- weights via ACT hwdge queue ----------
            WT = {}
            for nm in wnames_bf:
                WT[nm] = wp.tile(WSHAPE[nm], BF16, tag=nm, name=nm)
                nc.scalar.dma_start(WT[nm][:], P[nm][:])
            for nm in wnames_f32:
                WT[nm] = wp.tile(WSHAPE[nm], F32, tag=nm, name=nm)
                nc.scalar.dma_start(WT[nm][:], P[nm][:])
            msk = sp.tile([128, 4], F32, tag="msk")
            nc.scalar.dma_start(msk[:], P["msk"][:])
            SEL0, SEL1 = msk[:, 2:3], msk[:, 3:4]
            MTOP, MBOT = msk[:, 0:1], msk[:, 1:2]


            arin = dp.tile([128, 4 * 128], F32, tag="arin")
            arout = dp.tile([128, 4 * 128], F32, tag="arout", addr_space="Shared")
            arin_sb = sp.tile([128, 4 * 128], F32, tag="arin_sb")
            g1sb = sp.tile([128, 4 * 128], F32, tag="g1sb")

            for gi, tp_t in enumerate([tpgx, tpgy]):
                gps = psC.tile([128, 128], F32, tag="psC")
                gram_mms(gps, tp_t, nck=NCK1)
                nc.vector.tensor_scalar_mul(
                    arin_sb[:, gi * 128 : (gi + 1) * 128], gps[:], SEL0
                )
                nc.vector.tensor_scalar_mul(
                    arin_sb[:, (2 + gi) * 128 : (3 + gi) * 128], gps[:], SEL1
                )
            nc.sync.dma_start(arin[:], arin_sb[:])
            nc.gpsimd.collective_compute(
                "AllReduce", ALU.add, replica_groups=[list(range(8))],
                ins=[arin.opt()], outs=[arout.opt()],
            )
            nc.sync.dma_start(g1sb[:], arout[:])
            # per-batch Gram selection
            gmy = sp.tile([128, 2 * 128], F32, tag="gmy")
            for gi in range(2):
                nc.vector.tensor_scalar_mul(
                    gmy[:, gi * 128 : (gi + 1) * 128],
                    g1sb[:, gi * 128 : (gi + 1) * 128], SEL0,
                )
                nc.vector.scalar_tensor_tensor(
                    gmy[:, gi * 128 : (gi + 1) * 128],
                    g1sb[:, (2 + gi) * 128 : (3 + gi) * 128], SEL1,
                    gmy[:, gi * 128 : (gi + 1) * 128],
                    op0=ALU.mult, op1=ALU.add,
                )

            def attn_module(m, G, qblk, kblk, WoTp, odd):
                """emit small-attn for module m from stack-Gram G [128, 128];
                writes W_oT into WoTp rows/cols r0."""
                wq_s = WT["wqm"][0:62, m * 64 : m * 64 + 62]
                wk_s = WT["wkm"][0:62, m * 64 : m * 64 + 62]
                pw_s = WT["pwm"][0:62, m * 64 : m * 64 + 62]

                def blk(tag, rblk, cblk):
                    if rblk == 0:
                        return G[0:62, cblk * 64 : cblk * 64 + 62]
                    t_ = sp.tile([128, 64], F32, tag="gblk_" + tag)
                    nc.sync.dma_start(
                        t_[0:62, 0:62], G[64:126, cblk * 64 : cblk * 64 + 62]
                    )
                    return t_[0:62, 0:62]

                gqk = blk("qk", qblk, kblk)
                gqq = blk("qq", qblk, qblk)
                gkk = blk("kk", kblk, kblk)

                def mm62(lhs, rhs, tag):
                    pp = psC.tile([128, 64], F32, tag="psC")
                    nc.tensor.matmul(pp[0:62, 0:62], lhs, rhs, start=True, stop=True)
                    ss = sp.tile([128, 64], F32, tag="t_" + tag, name="t_" + tag)
                    nc.vector.tensor_copy(ss[0:62, 0:62], pp[0:62, 0:62])
                    return ss

                T1 = mm62(gqk, wq_s, "T1")
                T2 = mm62(gqq, wq_s, "T2")
                T3 = mm62(gkk, wk_s, "T3")
                SKQ = mm62(wk_s, T1[0:62, 0:62], "SKQ")
                u2 = sp.tile([128, 64], F32, tag="u2")
                nc.vector.tensor_tensor(
                    u2[0:62, 0:62], wq_s, T2[0:62, 0:62], op=ALU.mult
                )
                u3 = sp.tile([128, 64], F32, tag="u3")
                nc.vector.tensor_tensor(
                    u3[0:62, 0:62], wk_s, T3[0:62, 0:62], op=ALU.mult
                )
                pq = psC.tile([128, 64], F32, tag="psC")
                nc.tensor.matmul(
                    pq[0:1, 0:62], WT["ones62"][0:62, 0:1], u2[0:62, 0:62],
                    start=True, stop=True,
                )
                pk = psC.tile([128, 64], F32, tag="psC")
                nc.tensor.matmul(
                    pk[0:62, 0:1], u3[0:62, 0:62], WT["ones62"][0:62, 0:1],
                    start=True, stop=True,
                )
                ik = sp.tile([128, 2], F32, tag="ik")
                nc.scalar.activation(ik[0:62, 0:1], pk[0:62, 0:1], AF.Sqrt)
                nc.vector.tensor_scalar_max(ik[0:62, 0:1], ik[0:62, 0:1], 1e-12)
                nc.vector.reciprocal(ik[0:62, 1:2], ik[0:62, 0:1])
                scd = sp.tile([128, 1], F32, tag="scd")
                nc.vector.tensor_tensor(
                    scd[0:62, 0:1], ik[0:62, 1:2], WT["rsc"][0:62, m : m + 1],
                    op=ALU.mult,
                )
                iq = sp.tile([1, 128], F32, tag="iq")
                nc.scalar.activation(iq[0:1, 0:62], pq[0:1, 0:62], AF.Sqrt)
                nc.vector.tensor_scalar_max(iq[0:1, 0:62], iq[0:1, 0:62], 1e-12)
                nc.vector.reciprocal(iq[0:1, 64:126], iq[0:1, 0:62])
                iqb = sp.tile([128, 64], F32, tag="iqb")
                nc.gpsimd.partition_broadcast(iqb[0:62, 0:62], iq[0:1, 64:126])
                L = sp.tile([128, 64], F32, tag="L")
                nc.vector.tensor_scalar_mul(L[0:62, 0:62], SKQ[0:62, 0:62], scd[0:62, 0:1])
                nc.vector.tensor_tensor(
                    L[0:62, 0:62], L[0:62, 0:62], iqb[0:62, 0:62], op=ALU.mult
                )
                A = sp.tile([128, 64], F32, tag="A")
                nc.vector.memset(A[:], 0.0)
                nc.vector.tensor_tensor(
                    L[0:62, 0:62], L[0:62, 0:62], WT["hmA"][0:62, 0:62], op=ALU.mult
                )
                nc.vector.tensor_tensor(
                    L[0:62, 0:62], L[0:62, 0:62], WT["hmB"][0:62, 0:62], op=ALU.add
                )
                mx = sp.tile([128, 2], F32, tag="mx")
                nc.vector.tensor_reduce(
                    mx[0:62, 0:1], L[0:62, 0:62], op=ALU.max, axis=mybir.AxisListType.X
                )
                nc.vector.tensor_scalar_mul(mx[0:62, 1:2], mx[0:62, 0:1], -1.0)
                nc.scalar.activation(
                    A[0:62, 0:62], L[0:62, 0:62], AF.Exp, bias=mx[0:62, 1:2]
                )
                sm = sp.tile([128, 2], F32, tag="sm")
                nc.vector.tensor_reduce(
                    sm[0:62, 0:1], A[0:62, 0:62], op=ALU.add, axis=mybir.AxisListType.X
                )
                nc.vector.reciprocal(sm[0:62, 1:2], sm[0:62, 0:1])
                nc.vector.tensor_scalar_mul(A[0:62, 0:62], A[0:62, 0:62], sm[0:62, 1:2])
                wps = psC.tile([128, 64], F32, tag="psC")
                if odd:
                    nc.tensor.matmul(
                        wps[64:126, 0:62], A[0:62, 0:62], pw_s,
                        start=True, stop=True, tile_position=(0, 64),
                    )
                    nc.vector.tensor_copy(WoTp[64:126, 64:126], wps[64:126, 0:62])
                else:
                    nc.tensor.matmul(
                        wps[0:62, 0:62], A[0:62, 0:62], pw_s, start=True, stop=True
                    )
                    nc.vector.tensor_copy(WoTp[0:62, 0:62], wps[0:62, 0:62])

            # WoT pair tiles
            WoT = {}
            for p in PAIRS:
                WoT[p] = sp.tile([128, 128], BF16, tag=f"WoT{p}", name=f"WoT{p}")
                nc.vector.memset(WoT[p][:], 0.0)

            # =============== P23: modules 2,3 (v from z) ===============
            v23 = bp.tile([128, FA], BF16, tag="s2")
            stream_v(v23, WT["wv23"], zz, 6, kp=64)
            pad_zero(v23)
            g23 = bp.tile([128, FA], BF16, tag="s3")
            conv_pe_gelu(g23, v23, load_mdw("m_dw1_23"), WT["c3_dw1_23"], 5)
            pad_zero(g23)
            mask_rows(g23, 5)
            # attn for both L1 pairs (AR1 finished during the conv above);
            # modules 0,1 emitted after pt23 so their chain latency hides there
            attn_module(2, gmy[:, 0:128], 1, 0, WoT["23"], odd=False)
            attn_module(3, gmy[:, 128:256], 1, 0, WoT["23"], odd=True)
            fafb = bp.tile([128, FA], BF16, tag="s1")  # reuses zz slot
            pair_tail(fafb, v23, g23, WoT["23"], WT["pb23"], xy, "23", 4)
            pad_zero(fafb)
            mask_rows(fafb, 4)

            # =============== P01: modules 0,1 ===============
            attn_module(0, gmy[:, 0:128], 1, 1, WoT["01"], odd=False)
            attn_module(1, gmy[:, 128:256], 1, 1, WoT["01"], odd=True)
            v01 = bp.tile([128, FA], BF16, tag="s2")
            stream_v(v01, WT["wv01"], xy, 4)
            pad_zero(v01)
            g01 = bp.tile([128, FA], BF16, tag="s3")
            conv_pe_gelu(g01, v01, load_mdw("m_dw1_01"), WT["c3_dw1_01"], 3)
            pad_zero(g01)
            mask_rows(g01, 3)
            # in-place residual: xa|yb overwrites the xy slab (resid read and
            # output write are the same STT op, chunk by chunk)
            xaYb = xy
            pair_tail(xaYb, v01, g01, WoT["01"], WT["pb01"], xy, "01", 2)
            pad_zero(xaYb)
            mask_rows(xaYb, 2)

            # =============== L2 Grams + AR2 (all sources resident) ===============
            tpa = bp.tile([128, NCK * 128], BF16, tag="s2")  # v01 dead
            gram_half(tpa, xaYb, 0, 0, nc.sync)
            gram_half(tpa, fafb, 0, 64, nc.scalar)
            tpb = bp.tile([128, NCK * 128], BF16, tag="s3")  # g01 dead
            gram_half(tpb, xaYb, 64, 0, nc.sync)
            gram_half(tpb, fafb, 64, 64, nc.scalar)
            arin2_sb = sp.tile([128, 4 * 128], F32, tag="arin_sb")
            for gi, tp_t in enumerate([tpa, tpb]):
                gps = psC.tile([128, 128], F32, tag="psC")
                gram_mms(gps, tp_t)
                nc.vector.tensor_scalar_mul(
                    arin2_sb[:, gi * 128 : (gi + 1) * 128], gps[:], SEL0
                )
                nc.vector.tensor_scalar_mul(
                    arin2_sb[:, (2 + gi) * 128 : (3 + gi) * 128], gps[:], SEL1
                )
            arin2 = dp.tile([128, 4 * 128], F32, tag="arin2")
            arout2 = dp.tile([128, 4 * 128], F32, tag="arout2", addr_space="Shared")
            nc.sync.dma_start(arin2[:], arin2_sb[:])
            nc.gpsimd.collective_compute(
                "AllReduce", ALU.add, replica_groups=[list(range(8))],
                ins=[arin2.opt()], outs=[arout2.opt()],
            )
            g2sb = sp.tile([128, 4 * 128], F32, tag="g1sb")
            nc.sync.dma_start(g2sb[:], arout2[:])
            gmy2 = sp.tile([128, 2 * 128], F32, tag="gmy")
            for gi in range(2):
                nc.vector.tensor_scalar_mul(
                    gmy2[:, gi * 128 : (gi + 1) * 128],
                    g2sb[:, gi * 128 : (gi + 1) * 128], SEL0,
                )
                nc.vector.scalar_tensor_tensor(
                    gmy2[:, gi * 128 : (gi + 1) * 128],
                    g2sb[:, (2 + gi) * 128 : (3 + gi) * 128], SEL1,
                    gmy2[:, gi * 128 : (gi + 1) * 128],
                    op0=ALU.mult, op1=ALU.add,
                )

            # =============== P45 (v45/g45 overlap AR2 latency) ===============
            v45 = bp.tile([128, FA], BF16, tag="s2")  # tpa dead after gA MMs
            stream_v(v45, WT["wv45"], fafb, 4)
            pad_zero(v45)
            g45 = bp.tile([128, FA], BF16, tag="s3")  # tpb dead after gB MMs
            mdw45 = load_mdw("m_dw1_45")
            conv_pe_gelu(g45, v45, mdw45, WT["c3_dw1_45"], 3, part=(0, 2))
            # stacks are [xa|fa]: q=xa blk0, k=fa blk1 (chain hides in conv 2nd half)
            attn_module(4, gmy2[:, 0:128], 0, 1, WoT["45"], odd=False)
            attn_module(5, gmy2[:, 128:256], 0, 1, WoT["45"], odd=True)
            conv_pe_gelu(g45, v45, mdw45, WT["c3_dw1_45"], 3, part=(1, 2))
            pad_zero(g45)
            mask_rows(g45, 3)
            t12 = bp.tile([128, FA], BF16, tag="s1")  # fafb dead after v45/gram
            pair_tail(t12, v45, g45, WoT["45"], WT["pb45"], xaYb, "45", 2)
            pad_zero(t12)
            mask_rows(t12, 2)

            # =============== LN stats (both chains) ===============
            lo2, hi2 = exr(2)
            lo1, hi1 = exr(1)
            L2p = 64 * 275  # 17600 staging per stat row
            stat_all_d = dp.tile([2, 2 * L2p], F32, tag="stat_all_d")

            sq = bp.tile([128, FA], BF16, tag="s2")  # reuses v45 slot
            for lo, hi in chunks((lo2, hi2), 4096):
                nc.scalar.activation(sq[:, lo:hi], t12[:, lo:hi], AF.Square)
            for si, srct in enumerate([t12, sq]):
                base = si * L2LEN
                for ci2, (lo, hi) in enumerate(chunks((lo2, hi2), 512)):
                    ps = psA.tile([128, 1024], F32, tag="psA")
                    nc.tensor.matmul(
                        ps[0:2, 0 : hi - lo], WT["lnones"][:],
                        srct[:, lo:hi], start=True, stop=True,
                    )
                    stch = rp.tile([2, 512], F32, tag="stch", bufs=2)
                    if ci2 % 2 == 0:
                        nc.scalar.copy(stch[:, 0 : hi - lo], ps[0:2, 0 : hi - lo])
                    else:
                        nc.vector.tensor_copy(
                            stch[:, 0 : hi - lo], ps[0:2, 0 : hi - lo]
                        )
                    qe = nc.sync if ci2 % 2 == 0 else nc.scalar
                    qe.dma_start(
                        stat_all_d[:, base + lo - lo2 : base + hi - lo2],
                        stch[:, 0 : hi - lo],
                    )

            # per-pixel inv-std / mu*inv rows for both chains, upfront
            row_ds = {}
            for ci in (0, 1):
                row_dX = dp.tile([1, 2 * L2p], BF16, tag=f"row_d{ci}")
                mu64 = sp.tile([64, 275], F32, tag="mu64")
                ms64 = sp.tile([64, 275], F32, tag="ms64")
                nc.sync.dma_start(
                    mu64[:],
                    stat_all_d[ci : ci + 1, 0:L2p].rearrange(
                        "o (p f) -> (o p) f", p=64
                    ),
                )
                nc.sync.dma_start(
                    ms64[:],
                    stat_all_d[ci : ci + 1, L2LEN : L2LEN + L2p].rearrange(
                        "o (p f) -> (o p) f", p=64
                    ),
                )
                var = sp.tile([64, 275], F32, tag="var64")
                nc.vector.tensor_tensor(var[:], mu64[:], mu64[:], op=ALU.mult)
                nc.vector.tensor_tensor(var[:], ms64[:], var[:], op=ALU.subtract)
                sd = sp.tile([64, 275], F32, tag="sd64")
                nc.vector.tensor_scalar_add(var[:], var[:], 1e-5)
                nc.scalar.activation(sd[:], var[:], AF.Sqrt)
                inv = sp.tile([64, 275], F32, tag="inv64")
                nc.vector.reciprocal(inv[:], sd[:])
                invb = sp.tile([64, 275], BF16, tag="invb64")
                nc.vector.tensor_copy(invb[:], inv[:])
                musb = sp.tile([64, 275], BF16, tag="musb64")
                nc.vector.tensor_tensor(var[:], mu64[:], inv[:], op=ALU.mult)
                nc.vector.tensor_copy(musb[:], var[:])
                nc.sync.dma_start(
                    row_dX[0:1, 0:L2p].rearrange("o (p f) -> (o p) f", p=64),
                    invb[:],
                )
                nc.sync.dma_start(
                    row_dX[0:1, L2p : 2 * L2p].rearrange("o (p f) -> (o p) f", p=64),
                    musb[:],
                )
                row_ds[ci] = row_dX

            # =============== FF per chain (ring-buffered ts) ===============
            for ci, cn in ((0, "a"), (1, "b")):
                r0 = 0 if ci == 0 else 64
                row_dX = row_ds[ci]
                th = {}
                w1ts = [WT[f"w1aug_{cn}0"], WT[f"w1aug_{cn}1"]]
                for h, hn in ((0, "0"), (1, "1")):
                    tht = bp.tile(
                        [128, FA], BF16, tag=("s3" if h == 0 else "s2"),
                        name=f"th{h}_{cn}",
                    )
                    th[h] = tht
                # w1 for both halves per chunk; ts built on the fly in a ring
                for ck2, (lo, hi) in enumerate(chunks((lo2, hi2), 1024)):
                    n = hi - lo
                    qa = nc.sync if ck2 % 2 == 0 else nc.scalar
                    qb = nc.scalar if ck2 % 2 == 0 else nc.sync
                    sbr = rp.tile([128, 1024], BF16, tag="sbr", bufs=2)
                    qa.dma_start(
                        sbr[r0 : r0 + 62, 0:n],
                        row_dX[0:1, lo - lo2 : hi - lo2].partition_broadcast(62),
                    )
                    tsr = rp.tile([128, 1024], BF16, tag="tsr", bufs=1)
                    nc.vector.tensor_tensor(
                        tsr[r0 : r0 + 62, 0:n], t12[r0 : r0 + 62, lo:hi],
                        sbr[r0 : r0 + 62, 0:n], op=ALU.mult,
                    )
                    qb.dma_start(
                        tsr[r0 + 62 : r0 + 63, 0:n],
                        row_dX[0:1, L2p + lo - lo2 : L2p + hi - lo2],
                    )
                    for h in (0, 1):
                        ps = psA.tile([128, 1024], F32, tag="psA")
                        for l2, h2 in chunks((lo, hi), NT):
                            nc.tensor.matmul(
                                ps[0:124, l2 - lo : h2 - lo],
                                w1ts[h][r0 : r0 + 63, :],
                                tsr[r0 : r0 + 63, l2 - lo : h2 - lo],
                                start=True, stop=True,
                            )
                        nc.scalar.activation(
                            th[h][0:124, lo:hi], ps[0:124, 0 : hi - lo], AF.Gelu,
                            bias=WT[f"c1b_{cn}{'0' if h == 0 else '1'}"][0:124, :],
                        )
                for h in (0, 1):
                    pad_zero(th[h])
                    mask_rows(th[h], 2, (0, 124))
                # conv h0/h1 + gelu + w2, chunk-wise (convs on PE)
                for lo, hi in chunks((lo1, hi1), 1024):
                    wps = psA.tile([128, 1024], F32, tag="psA")
                    for h, hn in ((0, "0"), (1, "1")):
                        ghc = rp.tile([128, 1024], BF16, tag="ghc", bufs=2)
                        cps = psA.tile([128, 1024], F32, tag="psA")
                        mats = load_mdw(f"m_ffdw_{cn}{hn}")
                        dw_taps_split(
                            cps, th[h], mats, WT[f"c3_ffdw_{cn}{hn}"],
                            lo, hi, start=True,
                        )
                        nc.scalar.activation(
                            ghc[0:124, 0 : hi - lo], cps[0:124, 0 : hi - lo],
                            AF.Gelu,
                        )
                        w2t = WT[f"w2h_{cn}{hn}"]
                        for l2, h2 in chunks((lo, hi), NT):
                            nc.tensor.matmul(
                                wps[r0 : r0 + 62, l2 - lo : h2 - lo],
                                w2t[0:124, 0:62],
                                ghc[0:124, l2 - lo : h2 - lo],
                                start=(h == 0), stop=(h == 1),
                                tile_position=(0, 64) if ci == 1 else None,
                            )
                    # t12 += w2 out (in place, one rounding)
                    nc.vector.scalar_tensor_tensor(
                        t12[r0 : r0 + 62, lo:hi], wps[r0 : r0 + 62, 0 : hi - lo],
                        1.0, t12[r0 : r0 + 62, lo:hi], op0=ALU.mult, op1=ALU.add,
                    )

            # =============== fusions + final ===============
            fin = bp.tile([128, FA], BF16, tag="s3")
            nc.sync.dma_start(fin[64:126, lo1:hi1], P["zc"][0:62, :])
            fab = bp.tile([128, FA], BF16, tag="s2")
            for lo, hi in chunks((lo1, hi1), 1024):
                ps = psA.tile([128, 1024], F32, tag="psA")
                for l2, h2 in chunks((lo, hi), NT):
                    nc.tensor.matmul(
                        ps[:, l2 - lo : h2 - lo], WT["wfut"][:], t12[:, l2:h2],
                        start=True, stop=False,
                    )
                    nc.tensor.matmul(
                        ps[:, l2 - lo : h2 - lo], WT["wfux"][:], xaYb[:, l2:h2],
                        start=False, stop=True,
                    )
                nc.scalar.add(fab[:, lo:hi], ps[:, 0 : hi - lo], WT["fucb"][:])
            pad_zero(fab)
            mask_rows(fab, 1)
            for lo, hi in chunks((lo1, hi1), 1024):
                ps = psA.tile([128, 1024], F32, tag="psA")
                for l2, h2 in chunks((lo, hi), NT):
                    nc.tensor.matmul(
                        ps[0:64, l2 - lo : h2 - lo], WT["outw"][:], fab[:, l2:h2],
                        start=True, stop=True,
                    )
                nc.scalar.add(fin[0:62, lo:hi], ps[0:62, 0 : hi - lo], WT["outb"][0:62, :])
            pad_zero(fin)
            mask_rows(fin, 1, (0, 62))
            # final 3x3 conv, direct DMA out in 2-row (516 col) pieces.
            # each 258-col row-piece goes to its own PSUM bank (offsets 0, 512)
            lo0, hi0 = exr(0)
            out3 = out_p[:].rearrange("c (r w) -> c r w", w=W)
            for lo, hi in chunks((lo0, hi0), 516):
                ps = psA.tile([128, 1024], F32, tag="psA")
                for t in range(9):
                    dy, dx = TAPS[t]
                    d = dy * RS + dx
                    for bi, (l2, h2) in enumerate(((lo, lo + 258), (lo + 258, hi))):
                        nc.tensor.matmul(
                            ps[0:64, bi * 512 : bi * 512 + 258],
                            WT["finw"][:, t * 64 : (t + 1) * 64],
                            fin[:, l2 + d : h2 + d],
                            start=(t == 0), stop=(t == 8),
                        )
                och = rp.tile([62, 516], F32, tag="och", bufs=1)
                ps3 = ps[:].rearrange("p (b c) -> p b c", c=512)[0:62, :, 0:258]
                nc.scalar.add(
                    och[:].rearrange("c (r s) -> c r s", s=RS), ps3,
                    WT["finb"][0:62, :],
                )
                r0 = (lo - lo0) // RS
                nc.sync.dma_start(
                    out3[:, r0 : r0 + 2, :],
                    och[:].rearrange("c (r s) -> c r s", s=RS)[:, :, 0:W],
                )

    nc.finalize()
    return nc


_NC_CACHE = {}


def _run(inputs, trace=False):
    if "nc" not in _NC_CACHE:
        _NC_CACHE["nc"] = build_nc()
    nc = _NC_CACHE["nc"]
    names = {
        a.name.removesuffix("_set")
        for a in nc.m.functions[0].allocations
        if getattr(a, "kind", None) == "ExternalInput"
    }
    in_maps = prep_host_inputs(inputs)
    in_maps = [{k: v for k, v in m.items() if k in names} for m in in_maps]
    res = run_bass_kernel_spmd(
        nc, in_maps, core_ids=list(range(8)), trace=trace
    )
    return assemble_output(res.results), res


def kernel(**inputs):
    out, _ = _run(inputs, trace=False)
    return out
